# revision 1
# baseline (speedup 1.0000x reference)
"""Trainium2 Bass kernel for AttentionFFNBlock (B=2, L=2048, D=1024, H=16, FF=4096).

Sharding (8 cores, zero cross-core communication):
  core c -> batch b = c//4, group slot g = c%4.
  Each core owns 512 query rows of its batch, interleaved in 128-row blocks
  for causal load balance: global row = (2p+s)*512 + g*128 + i for local row
  r = p*256 + s*128 + i.  The core computes K/V for the full sequence
  (replicated inside the batch group), attention for its rows over all 16
  heads, then out-proj + LN1 + FFN + LN2 for its rows only.  Causality is
  enforced with per-core additive masks passed as input data (SPMD-safe).

Schedule: x arrives pre-transposed from the host (no DMA transposes); Q and
the first K/V chunks are projected up front; the remaining K / V projections
are interleaved into the attention head-pair loop so the PE stays dense
while the ACT engine works through the exp()s.  Scores skip dead (fully
masked) column blocks; head pairs share one exp instruction and alternate
PE row-groups (tile_position) so K=64 matmuls pack the array.  Wo/W1 are
prefetched as soon as SBUF frees up.  fc2 runs in two passes (rc pairs) so
the LN2 epilogues overlap the second pass's matmuls.

All matmuls in bf16 (fp32 PSUM accumulation); norms/softmax in fp32.
"""

import numpy as np
import ml_dtypes

import concourse.bass as bass
import concourse.mybir as mybir
import concourse.tile as tile
from concourse import bacc
from concourse.bass_utils import run_bass_kernel_spmd
from concourse.masks import make_identity

F32 = mybir.dt.float32
BF16 = mybir.dt.bfloat16
AF = mybir.ActivationFunctionType
ALU = mybir.AluOpType

N_CORES = 8
B, L, D = 2, 2048, 1024
H, HD = 16, 64
DFF = 4096
EPS = 1e-5
P = 128
NEG = -1e9

IC = D // P        # 8 contraction chunks of the model dim
TC = L // P        # 16 token chunks
FC = DFF // P      # 32 ff chunks
NPAIR = 8          # head pairs (= oc chunks)

_CACHE = {}


def _build():
    nc = bacc.Bacc("TRN2", target_bir_lowering=False, debug=False,
                   num_devices=N_CORES)

    def din(name, shape, dt=F32):
        return nc.dram_tensor(name, shape, dt, kind="ExternalInput").ap()

    io = dict(
        xT=din("xT", [D, L], BF16),               # x[b]^T (K/V source)
        xrT=din("xrT", [D, 512], BF16),           # owned rows^T (Q source)
        xr=din("xr", [512, D], F32),              # owned rows (residual)
        wq=din("wq", [D, D], BF16), wk=din("wk", [D, D], BF16),
        wv=din("wv", [D, D], BF16), wo=din("wo", [D, D], BF16),
        w1=din("w1", [D, DFF], BF16), w2=din("w2", [DFF, D], BF16),
        bq=din("bq", [D]), bk=din("bk", [D]), bv=din("bv", [D], BF16),
        bo=din("bo", [D], BF16), b1=din("b1", [DFF]), b2=din("b2", [D], BF16),
        g1=din("g1", [D], BF16), be1=din("be1", [D], BF16),
        g2=din("g2", [D], BF16), be2=din("be2", [D], BF16),
        cmask=din("cmask", [4, P, P], BF16),
        out=nc.dram_tensor("out", [512, D], BF16, kind="ExternalOutput").ap(),
    )

    with tile.TileContext(nc) as tc:
        _emit(nc, tc, io)
    nc.compile()
    return nc


def _layernorm(nc, pool, acc, eps_t, g_t, b_t, out_ap, g_eng=None,
               b_eng=None):
    """LayerNorm over the free axis (D=1024) of acc [128, 1024] -> out_ap."""
    stats = pool.tile([P, 2, 6], F32, tag="ln_stats")
    for sg in range(2):
        nc.vector.bn_stats(out=stats[:, sg, :], in_=acc[:, sg * 512:(sg + 1) * 512])
    mv = pool.tile([P, 2], F32, tag="ln_mv")
    nc.vector.bn_aggr(out=mv[:], in_=stats[:])
    rstd = pool.tile([P, 1], F32, tag="ln_rstd")
    nc.scalar.activation(out=rstd[:], in_=mv[:, 1:2], func=AF.Sqrt,
                         bias=eps_t[:], scale=1.0)
    nc.vector.reciprocal(out=rstd[:], in_=rstd[:])
    nmr = pool.tile([P, 1], F32, tag="ln_nmr")
    nc.vector.tensor_scalar(out=nmr[:], in0=mv[:, 0:1], scalar1=rstd[:],
                            scalar2=-1.0, op0=ALU.mult, op1=ALU.mult)
    u = pool.tile([P, D], BF16, tag="ln_u")
    nc.scalar.activation(out=u[:], in_=acc[:], func=AF.Identity,
                         bias=nmr[:], scale=rstd[:])
    (g_eng or nc.gpsimd).tensor_tensor(out=u[:], in0=u[:], in1=g_t[:, :],
                                       op=ALU.mult)
    (b_eng or nc.vector).tensor_tensor(out=out_ap, in0=u[:], in1=b_t[:, :],
                                       op=ALU.add)


def _emit(nc, tc, io):
    out = io["out"]

    with tc.tile_pool(name="const", bufs=1) as const:
        ao_pool = tc.alloc_tile_pool(name="ao_pool", bufs=1, side="right")
        # ---- constants / biases (tiles now; DMAs deferred past wk/xT) ----
        bq_t = const.tile([P, IC], F32)
        bk_t = const.tile([P, IC], F32)
        b1_t = const.tile([P, FC], F32)
        row_vecs = {}
        for nm in ("bv", "bo", "b2", "g1", "be1", "g2", "be2"):
            dt = F32 if nm.startswith("nope") else BF16
            rv = const.tile([P, D], dt, name=f"cv_{nm}")
            row_vecs[nm] = rv
        bv_t, bo_t, b2_t = row_vecs["bv"], row_vecs["bo"], row_vecs["b2"]
        g1_t, be1_t = row_vecs["g1"], row_vecs["be1"]
        g2_t, be2_t = row_vecs["g2"], row_vecs["be2"]
        cm_t = const.tile([P, 4, P], BF16)
        eps_t = const.tile([P, 1], F32)
        ident = const.tile([P, P], BF16)

        def early_dmas():
            nc.sync.dma_start(bq_t[:], io["bq"].rearrange("(o p) -> p o", p=P))
            nc.sync.dma_start(bk_t[:], io["bk"].rearrange("(o p) -> p o", p=P))
            nc.sync.dma_start(b1_t[:], io["b1"].rearrange("(f p) -> p f", p=P))
            nc.sync.dma_start(row_vecs["bv"][:],
                              io["bv"][None, :].to_broadcast([P, D]))
            nc.vector.memset(eps_t[:], EPS)

        def const_dmas():
            nc.sync.dma_start(cm_t[:], io["cmask"].rearrange("i p q -> p i q"))
            for nm in ("bo", "b2", "g1", "be1", "g2", "be2"):
                nc.sync.dma_start(row_vecs[nm][:],
                                  io[nm][None, :].to_broadcast([P, D]))
            make_identity(nc, ident[:])

        aoT = ao_pool.tile([P, IC, 512], BF16)   # attention output^T

        kv_pool = tc.alloc_tile_pool(name="kv_pool", bufs=1)
        ptile = tc.alloc_tile_pool(name="ptile", bufs=7)
        rtile = tc.alloc_tile_pool(name="rtile", bufs=2)
        spsum = tc.alloc_tile_pool(name="spsum", bufs=2, space="PSUM")
        avpsum = tc.alloc_tile_pool(name="avpsum", bufs=1, space="PSUM")
        if True:
            kT = kv_pool.tile([P, IC, L], BF16)
            v_all = kv_pool.tile([P, TC, H, HD + 1], BF16)
            qT = kv_pool.tile([P, IC, 512], BF16)
            nc.vector.memset(v_all[:, :, :, HD:], 1.0)

            proj_stream = []   # deferred (emit_mms, epilogue) generators

            def drain_proj(n):
                """Emit up to n deferred projection matmuls."""
                while n > 0 and proj_stream:
                    gen = proj_stream[0]
                    try:
                        next(gen)
                        n -= 1
                    except StopIteration:
                        proj_stream.pop(0)

            def attention(pair, prev_epi=None):
                oc = pair
                hA, hB = 2 * pair, 2 * pair + 1
                pavA = avpsum.tile([HD + 1, 512], F32, tag="avA")
                pavB = avpsum.tile([HD + 1, 512], F32, tag="avB")
                drain_proj(4)
                pts = []
                for kc in range(TC):
                    if kc == 2 and prev_epi is not None:
                        prev_epi()
                        prev_epi = None
                    j0 = kc // 4
                    n0 = j0 * P
                    ps = spsum.tile([P, 2, 512], F32, tag="s")
                    nc.tensor.matmul(
                        ps[:, 0, n0:512],
                        kT[0:HD, oc, kc * P:(kc + 1) * P],
                        qT[0:HD, oc, n0:512], start=True, stop=True)
                    nc.tensor.matmul(
                        ps[:, 1, n0:512],
                        kT[HD:P, oc, kc * P:(kc + 1) * P],
                        qT[HD:P, oc, n0:512], start=True, stop=True)
                    pt = ptile.tile([P, 2, 512], BF16, tag="p")
                    nc.scalar.activation(out=pt[:, :, n0:512],
                                         in_=ps[:, :, n0:512],
                                         func=AF.Exp, scale=0.125)
                    # diagonal-window causal mask on block j0 (both heads):
                    # multiply by 0/1 post-exp (gpsimd cannot touch PSUM)
                    for j in range(2):
                        nc.gpsimd.tensor_tensor(
                            out=pt[:, j, n0:n0 + P], in0=pt[:, j, n0:n0 + P],
                            in1=cm_t[:, kc % 4, :], op=ALU.mult)
                    pts.append((kc, n0, pt))
                    drain_proj(4 if pair < 3 else 2)
                    # AV lagging two chunks so the exp/mask chain never
                    # stalls the in-order PE queue
                    if len(pts) >= 7:
                        pkc, pn0, ppt = pts.pop(0)
                        for j, (h, pav) in enumerate(((hA, pavA), (hB, pavB))):
                            nc.tensor.matmul(
                                pav[:, pn0:512], v_all[:, pkc, h, :],
                                ppt[:, j, pn0:512], start=(pkc == 0),
                                stop=False, skip_group_check=True)
                while pts:
                    pkc, pn0, ppt = pts.pop(0)
                    last = not pts
                    for j, (h, pav) in enumerate(((hA, pavA), (hB, pavB))):
                        nc.tensor.matmul(
                            pav[:, pn0:512], v_all[:, pkc, h, :],
                            ppt[:, j, pn0:512], start=(pkc == 0),
                            stop=last, skip_group_check=True)
                def epi():
                    for hp, pav in ((0, pavA), (HD, pavB)):
                        rec = rtile.tile([1, 512], F32, tag="rec")
                        nc.vector.reciprocal(rec[:], pav[HD:HD + 1, :])
                        rec_b = rtile.tile([HD, 512], F32, tag="rec_b")
                        nc.gpsimd.partition_broadcast(rec_b[:], rec[0:1, :])
                        nc.vector.tensor_tensor(
                            out=aoT[hp:hp + HD, oc, :],
                            in0=pav[:HD, :], in1=rec_b[:], op=ALU.mult)
                return epi

            # ---- projections (pairs 0..5 overlap with x_pool live) ----
            with (
                tc.tile_pool(name="x_pool", bufs=1) as x_pool,
                tc.tile_pool(name="ppsum", bufs=2, space="PSUM") as ppsum,
            ):
                wk_t = x_pool.tile([P, IC, D], BF16)
                xT_t = x_pool.tile([P, IC, L], BF16)
                wq_t = x_pool.tile([P, IC, D], BF16)
                xrT_t = x_pool.tile([P, IC, 512], BF16)
                wv_t = x_pool.tile([P, IC, D], BF16)
                wkr = io["wk"].rearrange("(i p) n -> p i n", p=P)
                wqr = io["wq"].rearrange("(i p) n -> p i n", p=P)
                wvr = io["wv"].rearrange("(i p) n -> p i n", p=P)
                xTr = io["xT"].rearrange("(i p) n -> p i n", p=P)
                nc.sync.dma_start(wk_t[:, :, 0:P], wkr[:, :, 0:P])
                nc.sync.dma_start(xT_t[:, 0:4, 0:512], xTr[:, 0:4, 0:512])
                nc.sync.dma_start(xT_t[:, 4:8, 0:512], xTr[:, 4:8, 0:512])
                nc.sync.dma_start(wq_t[:, :, 0:512], wqr[:, :, 0:512])
                nc.sync.dma_start(xrT_t[:],
                                  io["xrT"].rearrange("(i p) n -> p i n", p=P))
                nc.sync.dma_start(wv_t[:, :, 0:512], wvr[:, :, 0:512])
                early_dmas()
                nc.sync.dma_start(xT_t[:, :, 512:1024], xTr[:, :, 512:1024])
                nc.sync.dma_start(wk_t[:, :, P:512], wkr[:, :, P:512])
                nc.sync.dma_start(xT_t[:, :, 1024:1536], xTr[:, :, 1024:1536])
                nc.sync.dma_start(xT_t[:, :, 1536:2048], xTr[:, :, 1536:2048])
                nc.sync.dma_start(wk_t[:, :, 512:1024], wkr[:, :, 512:1024])
                nc.sync.dma_start(wq_t[:, :, 512:1024], wqr[:, :, 512:1024])
                const_dmas()
                nc.sync.dma_start(wv_t[:, :, 512:1024], wvr[:, :, 512:1024])

                def k_proj(oc):
                    for tcc in range(4):
                        ps = ppsum.tile([P, 512], F32, tag="proj")
                        for ic in range(IC):
                            nc.tensor.matmul(
                                ps[:], wk_t[:, ic, oc * P:(oc + 1) * P],
                                xT_t[:, ic, tcc * 512:(tcc + 1) * 512],
                                start=(ic == 0), stop=(ic == IC - 1))
                            yield
                        nc.vector.tensor_scalar_add(
                            out=kT[:, oc, tcc * 512:(tcc + 1) * 512],
                            in0=ps[:], scalar1=bk_t[:, oc:oc + 1])

                def q_proj(oc):
                    ps = ppsum.tile([P, 512], F32, tag="proj")
                    for ic in range(IC):
                        nc.tensor.matmul(
                            ps[:], wq_t[:, ic, oc * P:(oc + 1) * P],
                            xrT_t[:, ic, :],
                            start=(ic == 0), stop=(ic == IC - 1))
                        yield
                    nc.vector.tensor_scalar_add(
                        out=qT[:, oc, :], in0=ps[:], scalar1=bq_t[:, oc:oc + 1])

                def v_proj(tcc, hf):
                    ps = ppsum.tile([P, 512], F32, tag="proj")
                    for ic in range(IC):
                        nc.tensor.matmul(
                            ps[:], xT_t[:, ic, tcc * P:(tcc + 1) * P],
                            wv_t[:, ic, hf * 512:(hf + 1) * 512],
                            start=(ic == 0), stop=(ic == IC - 1))
                        yield
                    nc.vector.tensor_tensor(
                        out=v_all[:, tcc, hf * 8:(hf + 1) * 8, :HD],
                        in0=ps.rearrange("p (h d) -> p h d", d=HD),
                        in1=bv_t[:, hf * 512:(hf + 1) * 512]
                        .rearrange("p (h d) -> p h d", d=HD),
                        op=ALU.add)

                # upfront, ordered to match serial DMA arrival
                def adv(gen, n):
                    for _ in range(n):
                        try:
                            next(gen)
                        except StopIteration:
                            return
                k0, k1 = k_proj(0), k_proj(1)
                qs = [q_proj(oc) for oc in range(IC)]
                v0s = [v_proj(tcc, 0) for tcc in range(TC)]
                adv(k0, 8)                       # K0.tcc0 (wk0+xT0)
                for oc in range(4):
                    adv(qs[oc], 9)               # Q0-3 (wq0+xrT)
                for tcc in range(4):
                    adv(v0s[tcc], 9)             # V0 tcc0-3 (wv0+xT0)
                adv(k0, 100)                     # K0 rest (xT1-3)
                adv(k1, 32)                      # K1 (wk1)
                for oc in range(4, IC):
                    adv(qs[oc], 9)               # Q4-7 (wq1)
                for tcc in range(4, TC):
                    adv(v0s[tcc], 9)             # V0 rest
                for g in [k0, k1] + qs + v0s:
                    adv(g, 100)
                # deferred: K2,K3, all of V1, K4..K7 — drained inside attention
                proj_stream.extend([k_proj(2), k_proj(3)])
                proj_stream.extend(v_proj(tcc, 1) for tcc in range(TC))
                proj_stream.extend(k_proj(oc) for oc in range(4, IC))

                prev_epi = None
                for pair in range(7):
                    prev_epi = attention(pair, prev_epi)
                drain_proj(1 << 30)

            # x_pool freed: prefetch xr + wo under attn 7 (right side)
            xrr_pool = tc.alloc_tile_pool(name="xrr_pool", bufs=1, side="right")
            xr_nat = xrr_pool.tile([P, 4, D], F32)
            nc.sync.dma_start(xr_nat[:],
                              io["xr"].rearrange("(rc p) d -> p rc d", p=P))
            wo_pool = tc.alloc_tile_pool(name="wo_pool", bufs=1, side="right")
            wo_t = wo_pool.tile([P, IC, D], BF16)
            wor = io["wo"].rearrange("(i p) n -> p i n", p=P)
            for h2 in range(2):
                nc.sync.dma_start(wo_t[:, :, h2 * 512:(h2 + 1) * 512],
                                  wor[:, :, h2 * 512:(h2 + 1) * 512])

            prev_epi = attention(7, prev_epi)
            prev_epi()

            # free the attention pools (non-LIFO: wo/w1a stay live)
            avpsum.release()
            spsum.release()
            rtile.release()
            ptile.release()
            kv_pool.release()

            w1_pool = tc.alloc_tile_pool(name="w1_pool", bufs=1)
            w1_t = w1_pool.tile([P, IC, DFF], BF16)
            w1r = io["w1"].rearrange("(i p) n -> p i n", p=P)
            for c in range(8):
                nc.sync.dma_start(w1_t[:, :, c * 512:(c + 1) * 512],
                                  w1r[:, :, c * 512:(c + 1) * 512])


            if True:
                # ---- out-proj + LN1 + transpose ----
                with tc.tile_pool(name="t_pool", bufs=1) as t_pool:
                    tbf = t_pool.tile([P, 4, D], BF16)    # LN1 out (residual)
                    tT = t_pool.tile([P, IC, 512], BF16)  # LN1 out transposed

                    with (
                        tc.tile_pool(name="lnt", bufs=4) as lnt,
                        tc.tile_pool(name="opsum", bufs=4, space="PSUM") as opsum,
                        tc.tile_pool(name="trpsum", bufs=4, space="PSUM") as trpsum,
                    ):
                        for rc in range(4):
                            acc = lnt.tile([P, D], F32, tag="acc")
                            for n2 in range(2):
                                ps = opsum.tile([P, 512], F32, tag="o")
                                for dc in range(IC):
                                    nc.tensor.matmul(
                                        ps[:], aoT[:, dc, rc * P:(rc + 1) * P],
                                        wo_t[:, dc, n2 * 512:(n2 + 1) * 512],
                                        start=(dc == 0), stop=(dc == IC - 1))
                                nc.vector.tensor_tensor(
                                    out=acc[:, n2 * 512:(n2 + 1) * 512],
                                    in0=ps[:],
                                    in1=xr_nat[:, rc, n2 * 512:(n2 + 1) * 512],
                                    op=ALU.add)
                            nc.vector.tensor_tensor(
                                out=acc[:], in0=acc[:], in1=bo_t[:, :],
                                op=ALU.add)
                            _layernorm(nc, lnt, acc, eps_t, g1_t, be1_t,
                                       tbf[:, rc, :])
                        for rc in range(4):
                            for ic in range(IC):
                                pst = trpsum.tile([P, P], BF16, tag="tr")
                                nc.tensor.transpose(
                                    pst[:], tbf[:, rc, ic * P:(ic + 1) * P],
                                    ident[:])
                                nc.scalar.copy(
                                    tT[:, ic, rc * P:(rc + 1) * P], pst[:])

                    wo_pool.release()
                    xrr_pool.release()
                    ao_pool.release()

                    # ================= FFN =================
                    w2_pool = tc.alloc_tile_pool(name="w2_pool", bufs=1)
                    w2_t = w2_pool.tile([P, FC, D], BF16)
                    w2r = io["w2"].rearrange("(f p) n -> p f n", p=P)
                    for grp in range(8):
                        nc.sync.dma_start(w2_t[:, grp * 4:(grp + 1) * 4, :],
                                          w2r[:, grp * 4:(grp + 1) * 4, :])
                    with (
                        tc.tile_pool(name="h_pool", bufs=1) as h_pool,
                        tc.tile_pool(name="fpsum", bufs=2, space="PSUM") as fpsum,
                        tc.tile_pool(name="ypsum", bufs=3, space="PSUM") as ypsum,
                    ):
                        hT = h_pool.tile([P, FC, 512], BF16)
                        psy = {}

                        def fc2_mms(fc, rcs):
                            for rc in rcs:
                                for n2 in range(2):
                                    nc.tensor.matmul(
                                        psy[rc][:, n2, :],
                                        hT[:, fc, rc * P:(rc + 1) * P],
                                        w2_t[:, fc, n2 * 512:(n2 + 1) * 512],
                                        start=(fc == 0), stop=(fc == FC - 1))

                        finbox = {}

                        def epilogue(rc):
                            fin = finbox["p"]
                            acc = fin.tile([P, D], F32, tag="acc2", bufs=2)
                            for n2 in range(2):
                                nc.vector.tensor_tensor(
                                    out=acc[:, n2 * 512:(n2 + 1) * 512],
                                    in0=psy[rc][:, n2, :],
                                    in1=tbf[:, rc, n2 * 512:(n2 + 1) * 512],
                                    op=ALU.add)
                            nc.vector.tensor_tensor(
                                out=acc[:], in0=acc[:], in1=b2_t[:, :],
                                op=ALU.add)
                            res = fin.tile([P, D], BF16, tag="res", bufs=2)
                            _layernorm(nc, fin, acc, eps_t, g2_t, be2_t,
                                       res[:], g_eng=nc.vector,
                                       b_eng=nc.vector)
                            nc.sync.dma_start(
                                out.rearrange("(rc p) d -> p rc d", p=P)[:, rc, :],
                                res[:])

                        # pass 1: fc1 + fc2 for rc 0,1,2 interleaved per fc
                        psy[0] = ypsum.tile([P, 2, 512], F32, tag="y", name="psy0")
                        psy[1] = ypsum.tile([P, 2, 512], F32, tag="y", name="psy1")
                        psy[2] = ypsum.tile([P, 2, 512], F32, tag="y", name="psy2")
                        for grp in range(8):
                            for k in range(4):
                                fc = grp * 4 + k
                                ps = fpsum.tile([P, 512], F32, tag="f1")
                                for ic in range(IC):
                                    nc.tensor.matmul(
                                        ps[:],
                                        w1_t[:, ic, fc * P:(fc + 1) * P],
                                        tT[:, ic, :],
                                        start=(ic == 0), stop=(ic == IC - 1))
                                nc.scalar.activation(out=hT[:, fc, :], in_=ps[:],
                                                     func=AF.Gelu,
                                                     bias=b1_t[:, fc:fc + 1],
                                                     scale=1.0)
                                fc2_mms(fc, (0, 1, 2))
                        finbox["p"] = tc.alloc_tile_pool(name="fin", bufs=1)
                        epilogue(0)
                        epilogue(1)
                        epilogue(2)
                        # pass 2: fc2 for rc3 (w2 already prefetched)
                        psy[3] = ypsum.tile([P, 2, 512], F32, tag="y", name="psy3")
                        for fc in range(FC):
                            fc2_mms(fc, (3,))
                        epilogue(3)
                        finbox["p"].release()
                    w2_pool.release()

            w1_pool.release()


def _row_index(g):
    idx = np.empty(512, dtype=np.int64)
    r = 0
    for p in range(2):
        for s in range(2):
            j = 2 * p + s
            base = j * 512 + g * 128
            idx[r:r + 128] = np.arange(base, base + 128)
            r += 128
    return idx


def _causal_masks(g):
    kj = np.arange(P)[:, None]
    qi = np.arange(P)[None, :]
    m = np.empty((4, P, P), dtype=np.float32)
    for i in range(4):
        m[i] = np.where(kj <= qi + (g - i) * P, 1.0, 0.0)
    return m


def kernel(**inputs):
    if "nc" not in _CACHE:
        _CACHE["nc"] = _build()
    nc = _CACHE["nc"]

    bf = ml_dtypes.bfloat16
    x = np.asarray(inputs["x"], dtype=np.float32)
    w_bf = {k: np.ascontiguousarray(
        np.asarray(inputs[k], dtype=np.float32).astype(bf))
        for k in ("Wq", "Wk", "Wv", "Wo", "W1", "W2")}
    vecs = {k: np.ascontiguousarray(np.asarray(inputs[k], dtype=np.float32))
            for k in ("bq", "bk", "bv", "bo", "b1", "b2", "g1", "be1", "g2",
                      "be2")}

    in_maps = []
    for c in range(N_CORES):
        b, g = c // 4, c % 4
        idx = _row_index(g)
        xb = x[b]
        xrows = xb[idx]
        in_maps.append({
            "xT": np.ascontiguousarray(xb.T.astype(bf)),
            "xrT": np.ascontiguousarray(xrows.T.astype(bf)),
            "xr": np.ascontiguousarray(xrows),
            "wq": w_bf["Wq"], "wk": w_bf["Wk"], "wv": w_bf["Wv"],
            "wo": w_bf["Wo"], "w1": w_bf["W1"], "w2": w_bf["W2"],
            "bq": vecs["bq"], "bk": vecs["bk"],
            "bv": vecs["bv"].astype(bf), "bo": vecs["bo"].astype(bf),
            "b1": vecs["b1"], "b2": vecs["b2"].astype(bf),
            "g1": vecs["g1"].astype(bf), "be1": vecs["be1"].astype(bf),
            "g2": vecs["g2"].astype(bf), "be2": vecs["be2"].astype(bf),
            "cmask": _causal_masks(g).astype(bf),
        })

    res = run_bass_kernel_spmd(nc, in_maps, core_ids=list(range(N_CORES)))
    _CACHE["last_result"] = res

    outp = np.empty((B, L, D), dtype=np.float32)
    for c in range(N_CORES):
        b, g = c // 4, c % 4
        outp[b][_row_index(g)] = res.results[c]["out"].astype(np.float32)
    return outp



# revision 14
# speedup vs baseline: 1.2999x; 1.2999x over previous
"""Trainium2 Bass kernel for AttentionFFNBlock (B=2, L=2048, D=1024, H=16, FF=4096).

Sharding (8 cores, zero cross-core communication):
  core c -> batch b = c//4, group slot g = c%4.
  Each core owns 512 query rows of its batch, interleaved in 128-row blocks
  for causal load balance: global row = (2p+s)*512 + g*128 + i for local row
  r = p*256 + s*128 + i.  The core computes K/V for the full sequence
  (replicated inside the batch group), attention for its rows over all 16
  heads, then out-proj + LN1 + FFN + LN2 for its rows only.

FP8 design (cost model: DoubleRow fp8 matmul = 0.5 cycles/row with 2x128
contraction -> 4x bf16 MAC throughput):
  - Q/K/V/out projections run as fp8e4m3 DoubleRow matmuls. wq/wk are scaled
    16x host-side (their sigma=1/32 sits in e4m3's subnormal range); the
    1/16 descale folds into the psum-drain tensor_scalar for free.
  - Scores stay bf16 (kT/qT bf16).  Causality is enforced PRE-exp by one
    extra bf16 matmul per (pair, kc): a static lower-triangular [k>=m]*-240
    stationary against a per-core indicator moving operand adds -240 exactly
    where key > query.  No per-element mask multiplies on DVE/Pool at all.
  - Softmax: pt = exp(s/8 - 2) written by ACT directly as fp8e4m3 (max logit
    ~6.5 -> max pt ~95 < 240).  The denominator comes from the ones column of
    v8 through the same AV matmul, so quantization of pt largely cancels.
  - AV and out-proj are fp8 DoubleRow (v8 / aoT8 in e4m3).
  - FFN is 3-term split fp8: W ~ (Wh + Wl)/s with Wh=e4m3(s*W) and
    Wl=e5m2(s*W - Wh) (s=16 for W1, 64 for W2 - avoids e4m3 subnormal
    flush), activations split hi=e4m3(a), lo=e5m2(a - hi). Terms
    ah@Wh + al@Wh + ah@Wl accumulate in one psum group: 0.75x the bf16
    cost with ~bf16 accuracy.  Descale 1/16 folds into the Gelu activation
    scale; 1/64 into the fc2 drain tensor_scalar.

Measured numpy end-to-end rel err of this exact scheme: 7.8e-3 (gate 2e-2).
"""

import numpy as np
import ml_dtypes

import concourse.bass as bass
import concourse.mybir as mybir
import concourse.tile as tile
from concourse import bacc
from concourse.bass_utils import run_bass_kernel_spmd
from concourse.masks import make_identity

F32 = mybir.dt.float32
BF16 = mybir.dt.bfloat16
F8E4 = mybir.dt.float8e4
F8E5 = mybir.dt.float8e5
AF = mybir.ActivationFunctionType
ALU = mybir.AluOpType
DR = mybir.MatmulPerfMode.DoubleRow

N_CORES = 8
B, L, D = 2, 2048, 1024
H, HD = 16, 64
DFF = 4096
EPS = 1e-5
P = 128

IC = D // P        # 8 contraction chunks of the model dim
ICH = IC // 2      # 4 DoubleRow chunks (256 contraction each)
TC = L // P        # 16 token chunks
FC = DFF // P      # 32 ff chunks
FCH = FC // 2      # 16 DoubleRow ff chunks
NPAIR = 8          # head pairs (= oc chunks)

WQK_SCALE = 16.0   # wq/wk quantized from 16*W
W1_SCALE = 16.0
W2_SCALE = 64.0

_CACHE = {}


def _build():
    nc = bacc.Bacc("TRN2", target_bir_lowering=False, debug=False,
                   num_devices=N_CORES)

    def din(name, shape, dt=F32):
        return nc.dram_tensor(name, shape, dt, kind="ExternalInput").ap()

    io = dict(
        xT=din("xT", [D, L], F8E4),               # x[b]^T (K/V source)
        xrT=din("xrT", [D, 512], F8E4),           # owned rows^T (Q source)
        xr=din("xr", [512, D], F32),              # owned rows (residual)
        wq=din("wq", [D, D], F8E4), wk=din("wk", [D, D], F8E4),
        wv=din("wv", [D, D], F8E4), wo=din("wo", [D, D], F8E4),
        w1h=din("w1h", [D, DFF], F8E4), w1l=din("w1l", [D, DFF], F8E5),
        w2h=din("w2h", [DFF, D], F8E4), w2l=din("w2l", [DFF, D], F8E5),
        bq=din("bq", [D]), bk=din("bk", [D]), bv=din("bv", [D], BF16),
        bo=din("bo", [D], BF16), b1=din("b1", [DFF]), b2=din("b2", [D], BF16),
        g1=din("g1", [D], BF16), be1=din("be1", [D], BF16),
        g2=din("g2", [D], BF16), be2=din("be2", [D], BF16),
        mq=din("mq", [P, 4, 2, P], BF16),         # causal indicator (per-core)
        mk=din("mk", [P, P], BF16),               # static -240 * [k >= m]
        out=nc.dram_tensor("out", [512, D], BF16, kind="ExternalOutput").ap(),
    )

    with tile.TileContext(nc) as tc:
        _emit(nc, tc, io)
    nc.compile()
    return nc


def _layernorm(nc, pool, acc, eps_t, g_t, b_t, out_ap, g_eng=None,
               b_eng=None):
    """LayerNorm over the free axis (D=1024) of acc [128, 1024] -> out_ap."""
    stats = pool.tile([P, 2, 6], F32, tag="ln_stats")
    for sg in range(2):
        nc.vector.bn_stats(out=stats[:, sg, :], in_=acc[:, sg * 512:(sg + 1) * 512])
    mv = pool.tile([P, 2], F32, tag="ln_mv")
    nc.vector.bn_aggr(out=mv[:], in_=stats[:])
    rstd = pool.tile([P, 1], F32, tag="ln_rstd")
    nc.scalar.activation(out=rstd[:], in_=mv[:, 1:2], func=AF.Sqrt,
                         bias=eps_t[:], scale=1.0)
    nc.vector.reciprocal(out=rstd[:], in_=rstd[:])
    nmr = pool.tile([P, 1], F32, tag="ln_nmr")
    nc.vector.tensor_scalar(out=nmr[:], in0=mv[:, 0:1], scalar1=rstd[:],
                            scalar2=-1.0, op0=ALU.mult, op1=ALU.mult)
    u = pool.tile([P, D], BF16, tag="ln_u")
    nc.scalar.activation(out=u[:], in_=acc[:], func=AF.Identity,
                         bias=nmr[:], scale=rstd[:])
    (g_eng or nc.gpsimd).tensor_tensor(out=u[:], in0=u[:], in1=g_t[:, :],
                                       op=ALU.mult)
    (b_eng or nc.vector).tensor_tensor(out=out_ap, in0=u[:], in1=b_t[:, :],
                                       op=ALU.add)


def _emit(nc, tc, io):
    out = io["out"]

    with tc.tile_pool(name="const", bufs=1) as const:
        ao_pool = tc.alloc_tile_pool(name="ao_pool", bufs=1, side="right")
        # ---- constants / biases (tiles now; DMAs deferred past wk/xT) ----
        bq_t = const.tile([P, IC], F32)
        bk_t = const.tile([P, IC], F32)
        b1_t = const.tile([P, FC], F32)
        row_vecs = {}
        for nm in ("bv", "bo", "b2", "g1", "be1", "g2", "be2"):
            rv = const.tile([P, D], BF16, name=f"cv_{nm}")
            row_vecs[nm] = rv
        bv_t, bo_t, b2_t = row_vecs["bv"], row_vecs["bo"], row_vecs["b2"]
        g1_t, be1_t = row_vecs["g1"], row_vecs["be1"]
        g2_t, be2_t = row_vecs["g2"], row_vecs["be2"]
        mq_t = const.tile([P, 4, 2, P], BF16)
        mk_t = const.tile([P, P], BF16)
        eps_t = const.tile([P, 1], F32)
        neg2_t = const.tile([P, 1], F32)
        ident = const.tile([P, P], BF16)

        def early_dmas():
            nc.sync.dma_start(bq_t[:], io["bq"].rearrange("(o p) -> p o", p=P))
            nc.sync.dma_start(bk_t[:], io["bk"].rearrange("(o p) -> p o", p=P))
            nc.sync.dma_start(b1_t[:], io["b1"].rearrange("(f p) -> p f", p=P))
            nc.sync.dma_start(row_vecs["bv"][:],
                              io["bv"][None, :].to_broadcast([P, D]))
            nc.sync.dma_start(mq_t[:], io["mq"])
            nc.sync.dma_start(mk_t[:], io["mk"])
            nc.vector.memset(eps_t[:], EPS)
            nc.vector.memset(neg2_t[:], -2.0)

        def const_dmas():
            for nm in ("bo", "b2", "g1", "be1", "g2", "be2"):
                nc.sync.dma_start(row_vecs[nm][:],
                                  io[nm][None, :].to_broadcast([P, D]))
            make_identity(nc, ident[:])

        aoT8 = ao_pool.tile([P, IC, 512], F8E4)   # attention output^T (fp8)

        kv_pool = tc.alloc_tile_pool(name="kv_pool", bufs=1)
        ptile = tc.alloc_tile_pool(name="ptile", bufs=4)
        rtile = tc.alloc_tile_pool(name="rtile", bufs=2)
        spsum = tc.alloc_tile_pool(name="spsum", bufs=2, space="PSUM")
        avpsum = tc.alloc_tile_pool(name="avpsum", bufs=1, space="PSUM")
        if True:
            kT = kv_pool.tile([P, IC, L], BF16)
            v8 = kv_pool.tile([P, TC, H, HD + 1], F8E4)
            qT = kv_pool.tile([P, IC, 512], BF16)
            nc.vector.memset(v8[:, :, :, HD:], 1.0)

            proj_stream = []   # deferred (emit_mms, epilogue) generators

            def drain_proj(n):
                """Emit up to n deferred projection matmuls."""
                while n > 0 and proj_stream:
                    gen = proj_stream[0]
                    try:
                        next(gen)
                        n -= 1
                    except StopIteration:
                        proj_stream.pop(0)

            # prefetch pools for FFN weights, allocated mid-attention
            late_pools = {}

            def attention(pair, prev_epi=None):
                hA, hB = 2 * pair, 2 * pair + 1
                pavA = avpsum.tile([HD + 1, 512], F32, tag="avA")
                pavB = avpsum.tile([HD + 1, 512], F32, tag="avB")
                drain_proj(4)
                pts = []

                def emit_av(ent, last):
                    pkcp, pn0, ppt = ent
                    for j, (h, pav) in enumerate(((hA, pavA), (hB, pavB))):
                        nc.tensor.matmul(
                            pav[:, pn0:512],
                            v8[:, 2 * pkcp:2 * pkcp + 2, h, :],
                            ppt[:, j, :, pn0:512],
                            start=(pkcp == 0), stop=last,
                            perf_mode=DR, skip_group_check=True)

                for kcp in range(8):
                    if kcp == 1 and prev_epi is not None:
                        prev_epi()
                        prev_epi = None
                    j0 = kcp // 2
                    n0 = j0 * P
                    pt = ptile.tile([P, 2, 2, 512], F8E4, tag="p")
                    for t in range(2):
                        kc = 2 * kcp + t
                        ps = spsum.tile([P, 2, 512], F32, tag="s")
                        nc.tensor.matmul(
                            ps[:, 0, n0:512],
                            kT[0:HD, pair, kc * P:(kc + 1) * P],
                            qT[0:HD, pair, n0:512], start=True, stop=True)
                        nc.tensor.matmul(
                            ps[:, 1, n0:512],
                            kT[HD:P, pair, kc * P:(kc + 1) * P],
                            qT[HD:P, pair, n0:512], start=True, stop=True)
                        for j in range(2):
                            nc.tensor.matmul(
                                ps[:, j, n0:n0 + P], mk_t[:],
                                mq_t[:, kc % 4, j, :], start=False,
                                stop=False, skip_group_check=True)
                        nc.scalar.activation(out=pt[:, :, t, n0:512],
                                             in_=ps[:, :, n0:512],
                                             func=AF.Exp, scale=0.125,
                                             bias=neg2_t[:])
                        drain_proj(3 if pair < 4 else 1)
                    pts.append((kcp, n0, pt))
                    drain_proj(3 if pair < 4 else 1)
                    if len(pts) >= 3:
                        emit_av(pts.pop(0), last=False)

                while pts:
                    ent = pts.pop(0)
                    emit_av(ent, last=(not pts))

                def epi():
                    for hp, pav in ((0, pavA), (HD, pavB)):
                        rec = rtile.tile([1, 512], F32, tag="rec")
                        nc.vector.reciprocal(rec[:], pav[HD:HD + 1, :])
                        rec_b = rtile.tile([HD, 512], F32, tag="rec_b")
                        nc.gpsimd.partition_broadcast(rec_b[:], rec[0:1, :])
                        nc.vector.tensor_tensor(
                            out=aoT8[hp:hp + HD, pair, :],
                            in0=pav[:HD, :], in1=rec_b[:], op=ALU.mult)
                return epi

            # ---- projections (pairs 0..6 overlap with x_pool live) ----
            with (
                tc.tile_pool(name="x_pool", bufs=1) as x_pool,
                tc.tile_pool(name="ppsum", bufs=2, space="PSUM") as ppsum,
            ):
                wk_t = x_pool.tile([P, IC, D], F8E4)
                xT_t = x_pool.tile([P, IC, L], F8E4)
                wq_t = x_pool.tile([P, IC, D], F8E4)
                xrT_t = x_pool.tile([P, IC, 512], F8E4)
                wv_t = x_pool.tile([P, IC, D], F8E4)
                wkr = io["wk"].rearrange("(i p) n -> p i n", p=P)
                wqr = io["wq"].rearrange("(i p) n -> p i n", p=P)
                wvr = io["wv"].rearrange("(i p) n -> p i n", p=P)
                xTr = io["xT"].rearrange("(i p) n -> p i n", p=P)
                nc.sync.dma_start(wk_t[:, :, 0:P], wkr[:, :, 0:P])
                nc.sync.dma_start(xT_t[:, 0:4, 0:512], xTr[:, 0:4, 0:512])
                nc.sync.dma_start(xT_t[:, 4:8, 0:512], xTr[:, 4:8, 0:512])
                nc.sync.dma_start(wq_t[:, :, 0:512], wqr[:, :, 0:512])
                nc.sync.dma_start(xrT_t[:],
                                  io["xrT"].rearrange("(i p) n -> p i n", p=P))
                nc.sync.dma_start(wv_t[:, :, 0:512], wvr[:, :, 0:512])
                early_dmas()
                nc.sync.dma_start(xT_t[:, :, 512:1024], xTr[:, :, 512:1024])
                nc.sync.dma_start(wk_t[:, :, P:512], wkr[:, :, P:512])
                nc.sync.dma_start(xT_t[:, :, 1024:1536], xTr[:, :, 1024:1536])
                nc.sync.dma_start(xT_t[:, :, 1536:2048], xTr[:, :, 1536:2048])
                nc.sync.dma_start(wk_t[:, :, 512:1024], wkr[:, :, 512:1024])
                nc.sync.dma_start(wq_t[:, :, 512:1024], wqr[:, :, 512:1024])
                const_dmas()
                nc.sync.dma_start(wv_t[:, :, 512:1024], wvr[:, :, 512:1024])

                def k_proj(oc):
                    for tcc in range(4):
                        ps = ppsum.tile([P, 512], F32, tag="proj")
                        for i2 in range(ICH):
                            nc.tensor.matmul(
                                ps[:],
                                wk_t[:, 2 * i2:2 * i2 + 2, oc * P:(oc + 1) * P],
                                xT_t[:, 2 * i2:2 * i2 + 2,
                                     tcc * 512:(tcc + 1) * 512],
                                start=(i2 == 0), stop=(i2 == ICH - 1),
                                perf_mode=DR)
                            yield
                        nc.vector.tensor_scalar(
                            out=kT[:, oc, tcc * 512:(tcc + 1) * 512],
                            in0=ps[:], scalar1=1.0 / WQK_SCALE,
                            scalar2=bk_t[:, oc:oc + 1],
                            op0=ALU.mult, op1=ALU.add)

                def q_proj(oc):
                    ps = ppsum.tile([P, 512], F32, tag="proj")
                    for i2 in range(ICH):
                        nc.tensor.matmul(
                            ps[:],
                            wq_t[:, 2 * i2:2 * i2 + 2, oc * P:(oc + 1) * P],
                            xrT_t[:, 2 * i2:2 * i2 + 2, :],
                            start=(i2 == 0), stop=(i2 == ICH - 1),
                            perf_mode=DR)
                        yield
                    nc.vector.tensor_scalar(
                        out=qT[:, oc, :], in0=ps[:], scalar1=1.0 / WQK_SCALE,
                        scalar2=bq_t[:, oc:oc + 1], op0=ALU.mult, op1=ALU.add)

                def v_proj(tcc, hf):
                    ps = ppsum.tile([P, 512], F32, tag="proj")
                    for i2 in range(ICH):
                        nc.tensor.matmul(
                            ps[:],
                            xT_t[:, 2 * i2:2 * i2 + 2, tcc * P:(tcc + 1) * P],
                            wv_t[:, 2 * i2:2 * i2 + 2,
                                 hf * 512:(hf + 1) * 512],
                            start=(i2 == 0), stop=(i2 == ICH - 1),
                            perf_mode=DR)
                        yield
                    nc.vector.tensor_tensor(
                        out=v8[:, tcc, hf * 8:(hf + 1) * 8, :HD],
                        in0=ps.rearrange("p (h d) -> p h d", d=HD),
                        in1=bv_t[:, hf * 512:(hf + 1) * 512]
                        .rearrange("p (h d) -> p h d", d=HD),
                        op=ALU.add)

                def adv(gen, n):
                    for _ in range(n):
                        try:
                            next(gen)
                        except StopIteration:
                            return

                ks = [k_proj(oc) for oc in range(IC)]
                qs = [q_proj(oc) for oc in range(IC)]
                v0s = [v_proj(tcc, 0) for tcc in range(TC)]
                v1s = [v_proj(tcc, 1) for tcc in range(TC)]
                # upfront, ordered to match serial DMA arrival.  All of V0
                # must be EMITTED before pair 0's AV flush (tile deps track
                # emission order), so V0 is not deferred.
                adv(ks[0], 4)                    # K0.tcc0 (wk0+xT0)
                for oc in range(4):
                    adv(qs[oc], 5)               # Q0-3 (wq0+xrT)
                adv(ks[0], 100)                  # K0 rest (xT1-3)
                for tcc in range(TC):
                    adv(v0s[tcc], 5)             # V0 (wv0+xT)
                adv(ks[1], 100)                  # K1 (wk1)
                for oc in range(4, IC):
                    adv(qs[oc], 5)               # Q4-7 (wq1)
                # deferred: rest drained inside the attention pair loop
                proj_stream.append(ks[2])
                proj_stream.extend(v1s[0:4])
                proj_stream.append(ks[3])
                proj_stream.extend(v1s[4:8])
                proj_stream.append(ks[4])
                proj_stream.extend(v1s[8:12])
                proj_stream.append(ks[5])
                proj_stream.extend(v1s[12:16])
                proj_stream.extend([ks[6], ks[7]])

                prev_epi = None
                for pair in range(4):
                    prev_epi = attention(pair, prev_epi)
                # mid-attention: prefetch fc1 weights (SBUF freed by Q release
                # is modest; w1h/w1l fit alongside the attention working set)
                w1_pool = tc.alloc_tile_pool(name="w1_pool", bufs=1,
                                             side="right")
                w1h_t = w1_pool.tile([P, IC, DFF], F8E4)
                w1r_h = io["w1h"].rearrange("(i p) n -> p i n", p=P)
                for c in range(4):
                    nc.sync.dma_start(
                        w1h_t[:, :, c * 1024:(c + 1) * 1024],
                        w1r_h[:, :, c * 1024:(c + 1) * 1024])
                late_pools["w1h"] = (w1_pool, w1h_t)
                for pair in range(4, 6):
                    prev_epi = attention(pair, prev_epi)
                w1l_pool = tc.alloc_tile_pool(name="w1l_pool", bufs=1,
                                              side="right")
                w1l_t = w1l_pool.tile([P, IC, DFF], F8E5)
                w1r_l = io["w1l"].rearrange("(i p) n -> p i n", p=P)
                for c in range(4):
                    nc.sync.dma_start(
                        w1l_t[:, :, c * 1024:(c + 1) * 1024],
                        w1r_l[:, :, c * 1024:(c + 1) * 1024])
                late_pools["w1l"] = (w1l_pool, w1l_t)
                prev_epi = attention(6, prev_epi)
                drain_proj(1 << 30)

            # x_pool freed: prefetch xr + wo + w2h under attn 7 (right side)
            xrr_pool = tc.alloc_tile_pool(name="xrr_pool", bufs=1, side="right")
            xr_nat = xrr_pool.tile([P, 4, D], F32)
            nc.sync.dma_start(xr_nat[:],
                              io["xr"].rearrange("(rc p) d -> p rc d", p=P))
            wo_pool = tc.alloc_tile_pool(name="wo_pool", bufs=1, side="right")
            wo_t = wo_pool.tile([P, IC, D], F8E4)
            wor = io["wo"].rearrange("(i p) n -> p i n", p=P)
            nc.sync.dma_start(wo_t[:], wor[:])

            prev_epi = attention(7, prev_epi)
            prev_epi()

            # free the attention pools (non-LIFO: wo/w1 stay live)
            avpsum.release()
            spsum.release()
            rtile.release()
            ptile.release()
            kv_pool.release()

            w1h_t = late_pools["w1h"][1]
            w1l_t = late_pools["w1l"][1]

            if True:
                # ---- out-proj + LN1 + transpose (hi/lo split) ----
                with tc.tile_pool(name="t_pool", bufs=1) as t_pool:
                    tbf = t_pool.tile([P, 4, D], BF16)     # LN1 out (residual)
                    tTh = t_pool.tile([P, IC, 512], F8E4)  # LN1 out^T hi
                    tTl = t_pool.tile([P, IC, 512], F8E5)  # LN1 out^T lo

                    # fc2 weights fit once the attention tiles are gone;
                    # DMA'd in fcp order so fc2 matmuls chase the transfers
                    w2h_pool = tc.alloc_tile_pool(name="w2h_pool", bufs=1)
                    w2h_t = w2h_pool.tile([P, FC, D], F8E4)
                    w2r_h = io["w2h"].rearrange("(f p) n -> p f n", p=P)
                    for grp in range(4):
                        nc.sync.dma_start(
                            w2h_t[:, grp * 8:(grp + 1) * 8, :],
                            w2r_h[:, grp * 8:(grp + 1) * 8, :])
                    w2l_pool = tc.alloc_tile_pool(name="w2l_pool", bufs=1)
                    w2l_t = w2l_pool.tile([P, FC, D], F8E5)
                    w2r_l = io["w2l"].rearrange("(f p) n -> p f n", p=P)
                    for grp in range(4):
                        nc.sync.dma_start(
                            w2l_t[:, grp * 8:(grp + 1) * 8, :],
                            w2r_l[:, grp * 8:(grp + 1) * 8, :])

                    with (
                        tc.tile_pool(name="lnt", bufs=2) as lnt,
                        tc.tile_pool(name="opsum", bufs=4, space="PSUM") as opsum,
                        tc.tile_pool(name="trpsum", bufs=4, space="PSUM") as trpsum,
                    ):
                        for rc in range(4):
                            acc = lnt.tile([P, D], F32, tag="acc")
                            for n2 in range(2):
                                pso = opsum.tile([P, 512], F32, tag="o")
                                for i2 in range(ICH):
                                    nc.tensor.matmul(
                                        pso[:],
                                        aoT8[:, 2 * i2:2 * i2 + 2,
                                             rc * P:(rc + 1) * P],
                                        wo_t[:, 2 * i2:2 * i2 + 2,
                                             n2 * 512:(n2 + 1) * 512],
                                        start=(i2 == 0), stop=(i2 == ICH - 1),
                                        perf_mode=DR)
                                nc.vector.tensor_tensor(
                                    out=acc[:, n2 * 512:(n2 + 1) * 512],
                                    in0=pso[:],
                                    in1=xr_nat[:, rc, n2 * 512:(n2 + 1) * 512],
                                    op=ALU.add)
                            nc.vector.tensor_tensor(
                                out=acc[:], in0=acc[:], in1=bo_t[:, :],
                                op=ALU.add)
                            _layernorm(nc, lnt, acc, eps_t, g1_t, be1_t,
                                       tbf[:, rc, :])
                        for rc in range(4):
                            for ic in range(IC):
                                pst = trpsum.tile([P, P], BF16, tag="tr")
                                nc.tensor.transpose(
                                    pst[:], tbf[:, rc, ic * P:(ic + 1) * P],
                                    ident[:])
                                nc.scalar.copy(
                                    tTh[:, ic, rc * P:(rc + 1) * P], pst[:])
                                nc.vector.tensor_tensor(
                                    out=tTl[:, ic, rc * P:(rc + 1) * P],
                                    in0=pst[:],
                                    in1=tTh[:, ic, rc * P:(rc + 1) * P],
                                    op=ALU.subtract)

                    wo_pool.release()
                    xrr_pool.release()

                    # ================= FFN =================
                    with (
                        tc.tile_pool(name="h_pool", bufs=1) as h_pool,
                        tc.tile_pool(name="tb_pool", bufs=2) as tb_pool,
                        tc.tile_pool(name="fpsum", bufs=2, space="PSUM") as fpsum,
                        tc.tile_pool(name="ypsum", bufs=3, space="PSUM") as ypsum,
                    ):
                        hh = h_pool.tile([P, FC, 512], F8E4)
                        hl = h_pool.tile([P, FC, 512], F8E5)
                        psy = {}
                        stop_tracker = {}

                        def fc2_mms(fcp, rcs, hx, wx, term):
                            for rc in rcs:
                                for n2 in range(2):
                                    key = (rc, n2)
                                    start = key not in stop_tracker
                                    stop_tracker[key] = True
                                    nc.tensor.matmul(
                                        psy[rc][:, n2, :],
                                        hx[:, 2 * fcp:2 * fcp + 2,
                                           rc * P:(rc + 1) * P],
                                        wx[:, 2 * fcp:2 * fcp + 2,
                                           n2 * 512:(n2 + 1) * 512],
                                        start=start, stop=False,
                                        perf_mode=DR, skip_group_check=True)

                        def fc2_mms_last(fcp, rcs):
                            for rc in rcs:
                                for n2 in range(2):
                                    nc.tensor.matmul(
                                        psy[rc][:, n2, :],
                                        hh[:, 2 * fcp:2 * fcp + 2,
                                           rc * P:(rc + 1) * P],
                                        w2l_t[:, 2 * fcp:2 * fcp + 2,
                                              n2 * 512:(n2 + 1) * 512],
                                        start=False, stop=True,
                                        perf_mode=DR, skip_group_check=True)

                        finbox = {}

                        def epilogue(rc):
                            fin = finbox["p"]
                            acc = fin.tile([P, D], F32, tag="acc2", bufs=2)
                            for n2 in range(2):
                                nc.vector.tensor_scalar(
                                    out=acc[:, n2 * 512:(n2 + 1) * 512],
                                    in0=psy[rc][:, n2, :],
                                    scalar1=1.0 / W2_SCALE, scalar2=None,
                                    op0=ALU.mult)
                            nc.gpsimd.tensor_tensor(
                                out=acc[:], in0=acc[:], in1=tbf[:, rc, :],
                                op=ALU.add)
                            nc.vector.tensor_tensor(
                                out=acc[:], in0=acc[:], in1=b2_t[:, :],
                                op=ALU.add)
                            res = fin.tile([P, D], BF16, tag="res", bufs=2)
                            _layernorm(nc, fin, acc, eps_t, g2_t, be2_t,
                                       res[:], g_eng=nc.vector,
                                       b_eng=nc.vector)
                            nc.sync.dma_start(
                                out.rearrange("(rc p) d -> p rc d", p=P)[:, rc, :],
                                res[:])

                        # pass 1: fc1 + fc2 for rc 0,1,2 interleaved per fc;
                        # term3 (hh @ w2l) lags 6 fcp behind so the w2l DMA
                        # (which only starts after the attention pools free)
                        # has landed.
                        psy[0] = ypsum.tile([P, 2, 512], F32, tag="y", name="psy0")
                        psy[1] = ypsum.tile([P, 2, 512], F32, tag="y", name="psy1")
                        psy[2] = ypsum.tile([P, 2, 512], F32, tag="y", name="psy2")
                        LAG = 6
                        for fc in range(FC):
                            ps = fpsum.tile([P, 512], F32, tag="f1")
                            for i2 in range(ICH):
                                nc.tensor.matmul(
                                    ps[:],
                                    w1h_t[:, 2 * i2:2 * i2 + 2,
                                          fc * P:(fc + 1) * P],
                                    tTh[:, 2 * i2:2 * i2 + 2, :],
                                    start=(i2 == 0), stop=False, perf_mode=DR)
                            for i2 in range(ICH):
                                nc.tensor.matmul(
                                    ps[:],
                                    w1h_t[:, 2 * i2:2 * i2 + 2,
                                          fc * P:(fc + 1) * P],
                                    tTl[:, 2 * i2:2 * i2 + 2, :],
                                    start=False, stop=False, perf_mode=DR)
                            for i2 in range(ICH):
                                nc.tensor.matmul(
                                    ps[:],
                                    w1l_t[:, 2 * i2:2 * i2 + 2,
                                          fc * P:(fc + 1) * P],
                                    tTh[:, 2 * i2:2 * i2 + 2, :],
                                    start=False, stop=(i2 == ICH - 1),
                                    perf_mode=DR)
                            tb = tb_pool.tile([P, 512], BF16, tag="tb")
                            nc.scalar.activation(out=tb[:], in_=ps[:],
                                                 func=AF.Gelu,
                                                 bias=b1_t[:, fc:fc + 1],
                                                 scale=1.0 / W1_SCALE)
                            nc.gpsimd.tensor_copy(out=hh[:, fc, :], in_=tb[:])
                            nc.vector.tensor_tensor(out=hl[:, fc, :],
                                                    in0=tb[:],
                                                    in1=hh[:, fc, :],
                                                    op=ALU.subtract)
                            if fc % 2 == 1:
                                fcp = fc // 2
                                fc2_mms(fcp, (0, 1, 2), hh, w2h_t, 1)
                                fc2_mms(fcp, (0, 1, 2), hl, w2h_t, 2)
                                if fcp >= LAG:
                                    if fcp - LAG == FCH - 1:
                                        fc2_mms_last(fcp - LAG, (0, 1, 2))
                                    else:
                                        fc2_mms(fcp - LAG, (0, 1, 2), hh,
                                                w2l_t, 3)
                        # fc1 weights are done with: free before the LN2
                        # epilogue scratch allocates (LIFO on the right
                        # stack: w1l, then w1h, then ao)
                        late_pools["w1l"][0].release()
                        late_pools["w1h"][0].release()
                        ao_pool.release()
                        for fcp in range(FCH - LAG, FCH):
                            if fcp == FCH - 1:
                                fc2_mms_last(fcp, (0, 1, 2))
                            else:
                                fc2_mms(fcp, (0, 1, 2), hh, w2l_t, 3)
                        finbox["p"] = tc.alloc_tile_pool(name="fin", bufs=1)
                        epilogue(0)
                        epilogue(1)
                        epilogue(2)
                        # pass 2: fc2 for rc3 (everything resident now)
                        psy[3] = ypsum.tile([P, 2, 512], F32, tag="y", name="psy3")
                        for fcp in range(FCH):
                            fc2_mms(fcp, (3,), hh, w2h_t, 1)
                            fc2_mms(fcp, (3,), hl, w2h_t, 2)
                            if fcp == FCH - 1:
                                fc2_mms_last(fcp, (3,))
                            else:
                                fc2_mms(fcp, (3,), hh, w2l_t, 3)
                        epilogue(3)
                        finbox["p"].release()
                    w2l_pool.release()
                    w2h_pool.release()


def _row_index(g):
    idx = np.empty(512, dtype=np.int64)
    r = 0
    for p in range(2):
        for s in range(2):
            j = 2 * p + s
            base = j * 512 + g * 128
            idx[r:r + 128] = np.arange(base, base + 128)
            r += 128
    return idx


def _mask_mq(g):
    """Causal indicator for the mask matmul: mq[m, i, d, q] = 1 iff the
    static -240*[k >= m] stationary, contracted against this column, yields
    -240*[k > q + (g - i)*128] (the masked region of the n0 block)."""
    mq = np.zeros((P, 4, 2, P), dtype=np.float32)
    for i in range(4):
        t = (g - i) * 128
        for q in range(P):
            tgt = q + t + 1
            if tgt < 0:
                tgt = 0
            if tgt <= P - 1:
                mq[tgt, i, :, q] = 1.0
    return mq


def _mask_mk():
    m = np.arange(P)[:, None]
    k = np.arange(P)[None, :]
    return np.where(k >= m, -240.0, 0.0).astype(np.float32)


def kernel(**inputs):
    if "nc" not in _CACHE:
        _CACHE["nc"] = _build()
    nc = _CACHE["nc"]

    bf = ml_dtypes.bfloat16
    e4 = ml_dtypes.float8_e4m3
    e5 = ml_dtypes.float8_e5m2
    x = np.asarray(inputs["x"], dtype=np.float32)

    def f32(k):
        return np.asarray(inputs[k], dtype=np.float32)

    wq8 = np.ascontiguousarray((WQK_SCALE * f32("Wq")).astype(e4))
    wk8 = np.ascontiguousarray((WQK_SCALE * f32("Wk")).astype(e4))
    wv8 = np.ascontiguousarray(f32("Wv").astype(e4))
    wo8 = np.ascontiguousarray(f32("Wo").astype(e4))
    w1s = W1_SCALE * f32("W1")
    w1h = w1s.astype(e4)
    w1l = (w1s - w1h.astype(np.float32)).astype(e5)
    w1h, w1l = np.ascontiguousarray(w1h), np.ascontiguousarray(w1l)
    w2s = W2_SCALE * f32("W2")
    w2h = w2s.astype(e4)
    w2l = (w2s - w2h.astype(np.float32)).astype(e5)
    w2h, w2l = np.ascontiguousarray(w2h), np.ascontiguousarray(w2l)
    vecs = {k: f32(k) for k in ("bq", "bk", "bv", "bo", "b1", "b2", "g1",
                                "be1", "g2", "be2")}
    mk = _mask_mk().astype(bf)

    in_maps = []
    for c in range(N_CORES):
        b, g = c // 4, c % 4
        idx = _row_index(g)
        xb = x[b]
        xrows = xb[idx]
        in_maps.append({
            "xT": np.ascontiguousarray(xb.T.astype(e4)),
            "xrT": np.ascontiguousarray(xrows.T.astype(e4)),
            "xr": np.ascontiguousarray(xrows),
            "wq": wq8, "wk": wk8, "wv": wv8, "wo": wo8,
            "w1h": w1h, "w1l": w1l, "w2h": w2h, "w2l": w2l,
            "bq": vecs["bq"], "bk": vecs["bk"],
            "bv": vecs["bv"].astype(bf), "bo": vecs["bo"].astype(bf),
            "b1": vecs["b1"], "b2": vecs["b2"].astype(bf),
            "g1": vecs["g1"].astype(bf), "be1": vecs["be1"].astype(bf),
            "g2": vecs["g2"].astype(bf), "be2": vecs["be2"].astype(bf),
            "mq": _mask_mq(g).astype(bf),
            "mk": mk,
        })

    res = run_bass_kernel_spmd(nc, in_maps, core_ids=list(range(N_CORES)))
    _CACHE["last_result"] = res

    outp = np.empty((B, L, D), dtype=np.float32)
    for c in range(N_CORES):
        b, g = c // 4, c % 4
        outp[b][_row_index(g)] = res.results[c]["out"].astype(np.float32)
    return outp


# revision 30
# speedup vs baseline: 1.3712x; 1.0548x over previous
"""Trainium2 Bass kernel for AttentionFFNBlock (B=2, L=2048, D=1024, H=16, FF=4096).

Sharding (8 cores, zero cross-core communication):
  core c -> batch b = c//4, group slot g = c%4.
  Each core owns 512 query rows of its batch, interleaved in 128-row blocks
  for causal load balance: global row = (2p+s)*512 + g*128 + i for local row
  r = p*256 + s*128 + i.  The core computes K/V for the full sequence
  (replicated inside the batch group), attention for its rows over all 16
  heads, then out-proj + LN1 + FFN + LN2 for its rows only.

FP8 design (cost model: DoubleRow fp8 matmul = 0.5 cycles/row with 2x128
contraction -> 4x bf16 MAC throughput):
  - Q/K/V/out projections run as fp8e4m3 DoubleRow matmuls. wq/wk are scaled
    16x host-side (their sigma=1/32 sits in e4m3's subnormal range); the
    1/16 descale folds into the psum-drain tensor_scalar for free.
  - Scores stay bf16 (kT/qT bf16).  Causality is enforced PRE-exp by one
    extra bf16 matmul per (pair, kc): a static lower-triangular [k>=m]*-240
    stationary against a per-core indicator moving operand adds -240 exactly
    where key > query.  No per-element mask multiplies on DVE/Pool at all.
  - Softmax: pt = exp(s/8 - 2) written by ACT directly as fp8e4m3 (max logit
    ~6.5 -> max pt ~95 < 240).  The denominator comes from the ones column of
    v8 through the same AV matmul, so quantization of pt largely cancels.
  - AV and out-proj are fp8 DoubleRow (v8 / aoT8 in e4m3).
  - FFN is 3-term split fp8: W ~ (Wh + Wl)/s with Wh=e4m3(s*W) and
    Wl=e5m2(s*W - Wh) (s=16 for W1, 64 for W2 - avoids e4m3 subnormal
    flush), activations split hi=e4m3(a), lo=e5m2(a - hi). Terms
    ah@Wh + al@Wh + ah@Wl accumulate in one psum group: 0.75x the bf16
    cost with ~bf16 accuracy.  Descale 1/16 folds into the Gelu activation
    scale; 1/64 into the fc2 drain tensor_scalar.

Measured numpy end-to-end rel err of this exact scheme: 7.8e-3 (gate 2e-2).
"""

import numpy as np
import ml_dtypes

import concourse.bass as bass
import concourse.mybir as mybir
import concourse.tile as tile
from concourse import bacc
from concourse.bass_utils import run_bass_kernel_spmd
from concourse.masks import make_identity

F32 = mybir.dt.float32
BF16 = mybir.dt.bfloat16
F8E4 = mybir.dt.float8e4
F8E5 = mybir.dt.float8e5
AF = mybir.ActivationFunctionType
ALU = mybir.AluOpType
DR = mybir.MatmulPerfMode.DoubleRow

N_CORES = 8
B, L, D = 2, 2048, 1024
H, HD = 16, 64
DFF = 4096
EPS = 1e-5
P = 128

IC = D // P        # 8 contraction chunks of the model dim
ICH = IC // 2      # 4 DoubleRow chunks (256 contraction each)
TC = L // P        # 16 token chunks
FC = DFF // P      # 32 ff chunks
FCH = FC // 2      # 16 DoubleRow ff chunks
NPAIR = 8          # head pairs (= oc chunks)

WQK_SCALE = 16.0   # wq/wk quantized from 16*W
W1_SCALE = 16.0
W2_SCALE = 64.0

_CACHE = {}


def _build():
    nc = bacc.Bacc("TRN2", target_bir_lowering=False, debug=False,
                   num_devices=N_CORES)

    def din(name, shape, dt=F32):
        return nc.dram_tensor(name, shape, dt, kind="ExternalInput").ap()

    io = dict(
        xT=din("xT", [D, L], F8E4),               # x[b]^T (K/V source)
        xrT=din("xrT", [D, 512], F8E4),           # owned rows^T (Q source)
        xr=din("xr", [512, D], F32),              # owned rows (residual)
        wq=din("wq", [D, D], F8E4), wk=din("wk", [D, D], F8E4),
        wv=din("wv", [D, D], F8E4), wo=din("wo", [D, D], F8E4),
        w1h=din("w1h", [D, DFF], F8E4), w1l=din("w1l", [D, DFF], F8E5),
        w2h=din("w2h", [DFF, D], F8E4), w2l=din("w2l", [DFF, D], F8E5),
        bq=din("bq", [D]), bk=din("bk", [D]), bv=din("bv", [D], BF16),
        b1=din("b1", [DFF]),
        g1=din("g1", [D], BF16), be1=din("be1", [D], BF16),
        g2=din("g2", [D], BF16), be2=din("be2", [D], BF16),
        mq=din("mq", [P, 4, 2, P], BF16),         # causal indicator (per-core)
        mk=din("mk", [P, P], BF16),               # static -240 * [k >= m]
        out=nc.dram_tensor("out", [512, D], BF16, kind="ExternalOutput").ap(),
    )

    with tile.TileContext(nc) as tc:
        _emit(nc, tc, io)
    nc.compile()
    return nc


def _ln_u(nc, pool, acc, eps_t, out_u):
    """Normalize (no affine) over the free axis of acc [128, 1024] -> out_u."""
    stats = pool.tile([P, 2, 6], F32, tag="ln_stats")
    for sg in range(2):
        nc.vector.bn_stats(out=stats[:, sg, :], in_=acc[:, sg * 512:(sg + 1) * 512])
    mv = pool.tile([P, 2], F32, tag="ln_mv")
    nc.vector.bn_aggr(out=mv[:], in_=stats[:])
    rstd = pool.tile([P, 1], F32, tag="ln_rstd")
    nc.scalar.activation(out=rstd[:], in_=mv[:, 1:2], func=AF.Sqrt,
                         bias=eps_t[:], scale=1.0)
    nc.vector.reciprocal(out=rstd[:], in_=rstd[:])
    nmr = pool.tile([P, 1], F32, tag="ln_nmr")
    nc.vector.tensor_scalar(out=nmr[:], in0=mv[:, 0:1], scalar1=rstd[:],
                            scalar2=-1.0, op0=ALU.mult, op1=ALU.mult)
    nc.scalar.activation(out=out_u, in_=acc[:], func=AF.Identity,
                         bias=nmr[:], scale=rstd[:])


def _layernorm(nc, pool, acc, eps_t, g_t, b_t, out_ap, g_eng=None,
               b_eng=None):
    """LayerNorm over the free axis (D=1024) of acc [128, 1024] -> out_ap."""
    u = pool.tile([P, D], BF16, tag="ln_u")
    _ln_u(nc, pool, acc, eps_t, u[:])
    (g_eng or nc.gpsimd).tensor_tensor(out=u[:], in0=u[:], in1=g_t[:, :],
                                       op=ALU.mult)
    (b_eng or nc.vector).tensor_tensor(out=out_ap, in0=u[:], in1=b_t[:, :],
                                       op=ALU.add)


def _emit(nc, tc, io):
    out = io["out"]

    with tc.tile_pool(name="const", bufs=1) as const:
        ao_pool = tc.alloc_tile_pool(name="ao_pool", bufs=1, side="right")
        # ---- constants / biases (tiles now; DMAs deferred past wk/xT) ----
        bq_t = const.tile([P, IC], F32)
        bk_t = const.tile([P, IC], F32)
        b1_t = const.tile([P, FC], F32)
        # bo is folded into xr host-side; b2 into be1 (tbf = x1 + b2);
        # g1/be1 into W1h/W1l/b1 for the fc1 path.
        row_vecs = {}
        for nm in ("bv", "g1", "be1", "g2", "be2"):
            rv = const.tile([P, D], BF16, name=f"cv_{nm}")
            row_vecs[nm] = rv
        bv_t = row_vecs["bv"]
        g1_t, be1_t = row_vecs["g1"], row_vecs["be1"]
        g2_t, be2_t = row_vecs["g2"], row_vecs["be2"]
        mq_t = const.tile([P, 4, 2, P], BF16)
        mk_t = const.tile([P, P], BF16)
        eps_t = const.tile([P, 1], F32)
        neg2_t = const.tile([P, 1], F32)
        ident = const.tile([P, P], BF16)

        def tiny_dmas():
            nc.sync.dma_start(bk_t[:], io["bk"].rearrange("(o p) -> p o", p=P))
            nc.sync.dma_start(bq_t[:], io["bq"].rearrange("(o p) -> p o", p=P))
            nc.sync.dma_start(mq_t[:], io["mq"])
            nc.sync.dma_start(mk_t[:], io["mk"])
            nc.vector.memset(eps_t[:], EPS)
            nc.vector.memset(neg2_t[:], -2.0)

        def early_dmas():
            nc.sync.dma_start(b1_t[:], io["b1"].rearrange("(f p) -> p f", p=P))
            nc.sync.dma_start(row_vecs["bv"][:],
                              io["bv"][None, :].to_broadcast([P, D]))

        def const_dmas():
            for nm in ("g1", "be1", "g2", "be2"):
                nc.sync.dma_start(row_vecs[nm][:],
                                  io[nm][None, :].to_broadcast([P, D]))
            make_identity(nc, ident[:])

        aoT8 = ao_pool.tile([P, IC, 512], F8E4)   # attention output^T (fp8)

        kv_pool = tc.alloc_tile_pool(name="kv_pool", bufs=1)
        ptile = tc.alloc_tile_pool(name="ptile", bufs=4)
        rtile = tc.alloc_tile_pool(name="rtile", bufs=2)
        spsum = tc.alloc_tile_pool(name="spsum", bufs=2, space="PSUM")
        avpsum = tc.alloc_tile_pool(name="avpsum", bufs=1, space="PSUM")
        if True:
            kT = kv_pool.tile([P, IC, L], BF16)
            v8 = kv_pool.tile([P, TC, H, HD + 1], F8E4)
            qT = kv_pool.tile([P, IC, 512], BF16)
            nc.vector.memset(v8[:, :, :, HD:], 1.0)

            proj_stream = []   # deferred (emit_mms, epilogue) generators

            def drain_proj(n):
                """Emit up to n deferred projection matmuls."""
                while n > 0 and proj_stream:
                    gen = proj_stream[0]
                    try:
                        next(gen)
                        n -= 1
                    except StopIteration:
                        proj_stream.pop(0)

            # prefetch pools for FFN weights, allocated mid-attention
            late_pools = {}

            def attention(pair, prev_epi=None):
                hA, hB = 2 * pair, 2 * pair + 1
                pavA = avpsum.tile([HD + 1, 512], F32, tag="avA")
                pavB = avpsum.tile([HD + 1, 512], F32, tag="avB")
                drain_proj(4)
                pts = []

                def emit_av(ent, last):
                    pkcp, pn0, ppt = ent
                    for j, (h, pav) in enumerate(((hA, pavA), (hB, pavB))):
                        nc.tensor.matmul(
                            pav[:, pn0:512],
                            v8[:, 2 * pkcp:2 * pkcp + 2, h, :],
                            ppt[:, j, :, pn0:512],
                            start=(pkcp == 0), stop=last,
                            perf_mode=DR, skip_group_check=True)

                for kcp in range(8):
                    if kcp == 1 and prev_epi is not None:
                        prev_epi()
                        prev_epi = None
                    j0 = kcp // 2
                    n0 = j0 * P
                    pt = ptile.tile([P, 2, 2, 512], F8E4, tag="p")
                    for t in range(2):
                        kc = 2 * kcp + t
                        ps = spsum.tile([P, 2, 512], F32, tag="s")
                        nc.tensor.matmul(
                            ps[:, 0, n0:512],
                            kT[0:HD, pair, kc * P:(kc + 1) * P],
                            qT[0:HD, pair, n0:512], start=True, stop=True)
                        nc.tensor.matmul(
                            ps[:, 1, n0:512],
                            kT[HD:P, pair, kc * P:(kc + 1) * P],
                            qT[HD:P, pair, n0:512], start=True, stop=True)
                        for j in range(2):
                            nc.tensor.matmul(
                                ps[:, j, n0:n0 + P], mk_t[:],
                                mq_t[:, kc % 4, j, :], start=False,
                                stop=False, skip_group_check=True)
                        nc.scalar.activation(out=pt[:, :, t, n0:512],
                                             in_=ps[:, :, n0:512],
                                             func=AF.Exp, scale=0.125,
                                             bias=neg2_t[:])
                        drain_proj(2 if pair < 6 else 1)
                    pts.append((kcp, n0, pt))
                    drain_proj(2 if pair < 6 else 1)
                    if len(pts) >= 3:
                        emit_av(pts.pop(0), last=False)

                while pts:
                    ent = pts.pop(0)
                    emit_av(ent, last=(not pts))

                def epi():
                    for hp, pav in ((0, pavA), (HD, pavB)):
                        rec = rtile.tile([1, 512], F32, tag="rec")
                        nc.vector.reciprocal(rec[:], pav[HD:HD + 1, :])
                        rec_b = rtile.tile([HD, 512], F32, tag="rec_b")
                        nc.gpsimd.partition_broadcast(rec_b[:], rec[0:1, :])
                        nc.vector.tensor_tensor(
                            out=aoT8[hp:hp + HD, pair, :],
                            in0=pav[:HD, :], in1=rec_b[:], op=ALU.mult)
                return epi

            # ---- projections (pairs 0..6 overlap with x_pool live) ----
            with (
                tc.tile_pool(name="x_pool", bufs=1) as x_pool,
                tc.tile_pool(name="ppsum", bufs=2, space="PSUM") as ppsum,
            ):
                wk_t = x_pool.tile([P, IC, D], F8E4)
                xT_t = x_pool.tile([P, IC, L], F8E4)
                wq_t = x_pool.tile([P, IC, D], F8E4)
                xrT_t = x_pool.tile([P, IC, 512], F8E4)
                wv_t = x_pool.tile([P, IC, D], F8E4)
                wkr = io["wk"].rearrange("(i p) n -> p i n", p=P)
                wqr = io["wq"].rearrange("(i p) n -> p i n", p=P)
                wvr = io["wv"].rearrange("(i p) n -> p i n", p=P)
                xTr = io["xT"].rearrange("(i p) n -> p i n", p=P)
                nc.sync.dma_start(wk_t[:, :, 0:P], wkr[:, :, 0:P])
                tiny_dmas()
                nc.sync.dma_start(xT_t[:, 0:4, 0:512], xTr[:, 0:4, 0:512])
                nc.sync.dma_start(xT_t[:, 4:8, 0:512], xTr[:, 4:8, 0:512])
                nc.sync.dma_start(wq_t[:, :, 0:P], wqr[:, :, 0:P])
                nc.sync.dma_start(xrT_t[:],
                                  io["xrT"].rearrange("(i p) n -> p i n", p=P))
                nc.sync.dma_start(wq_t[:, :, P:512], wqr[:, :, P:512])
                nc.sync.dma_start(wv_t[:, :, 0:512], wvr[:, :, 0:512])
                early_dmas()
                nc.sync.dma_start(xT_t[:, :, 512:1024], xTr[:, :, 512:1024])
                nc.sync.dma_start(wk_t[:, :, P:512], wkr[:, :, P:512])
                nc.sync.dma_start(xT_t[:, :, 1024:1536], xTr[:, :, 1024:1536])
                nc.sync.dma_start(xT_t[:, :, 1536:2048], xTr[:, :, 1536:2048])
                nc.sync.dma_start(wk_t[:, :, 512:1024], wkr[:, :, 512:1024])
                nc.sync.dma_start(wq_t[:, :, 512:1024], wqr[:, :, 512:1024])
                const_dmas()
                nc.sync.dma_start(wv_t[:, :, 512:1024], wvr[:, :, 512:1024])

                def k_proj(oc):
                    for tcc in range(4):
                        ps = ppsum.tile([P, 512], F32, tag="proj")
                        for i2 in range(ICH):
                            nc.tensor.matmul(
                                ps[:],
                                wk_t[:, 2 * i2:2 * i2 + 2, oc * P:(oc + 1) * P],
                                xT_t[:, 2 * i2:2 * i2 + 2,
                                     tcc * 512:(tcc + 1) * 512],
                                start=(i2 == 0), stop=(i2 == ICH - 1),
                                perf_mode=DR)
                            yield
                        nc.vector.tensor_scalar(
                            out=kT[:, oc, tcc * 512:(tcc + 1) * 512],
                            in0=ps[:], scalar1=1.0 / WQK_SCALE,
                            scalar2=bk_t[:, oc:oc + 1],
                            op0=ALU.mult, op1=ALU.add)

                def q_proj(oc):
                    ps = ppsum.tile([P, 512], F32, tag="proj")
                    for i2 in range(ICH):
                        nc.tensor.matmul(
                            ps[:],
                            wq_t[:, 2 * i2:2 * i2 + 2, oc * P:(oc + 1) * P],
                            xrT_t[:, 2 * i2:2 * i2 + 2, :],
                            start=(i2 == 0), stop=(i2 == ICH - 1),
                            perf_mode=DR)
                        yield
                    nc.vector.tensor_scalar(
                        out=qT[:, oc, :], in0=ps[:], scalar1=1.0 / WQK_SCALE,
                        scalar2=bq_t[:, oc:oc + 1], op0=ALU.mult, op1=ALU.add)

                def v_proj(tcc, hf):
                    ps = ppsum.tile([P, 512], F32, tag="proj")
                    for i2 in range(ICH):
                        nc.tensor.matmul(
                            ps[:],
                            xT_t[:, 2 * i2:2 * i2 + 2, tcc * P:(tcc + 1) * P],
                            wv_t[:, 2 * i2:2 * i2 + 2,
                                 hf * 512:(hf + 1) * 512],
                            start=(i2 == 0), stop=(i2 == ICH - 1),
                            perf_mode=DR)
                        yield
                    nc.vector.tensor_tensor(
                        out=v8[:, tcc, hf * 8:(hf + 1) * 8, :HD],
                        in0=ps.rearrange("p (h d) -> p h d", d=HD),
                        in1=bv_t[:, hf * 512:(hf + 1) * 512]
                        .rearrange("p (h d) -> p h d", d=HD),
                        op=ALU.add)

                def adv(gen, n):
                    for _ in range(n):
                        try:
                            next(gen)
                        except StopIteration:
                            return

                ks = [k_proj(oc) for oc in range(IC)]
                qs = [q_proj(oc) for oc in range(IC)]
                v0s = [v_proj(tcc, 0) for tcc in range(TC)]
                v1s = [v_proj(tcc, 1) for tcc in range(TC)]
                # upfront, ordered to match serial DMA arrival.  All of V0
                # must be EMITTED before pair 0's AV flush (tile deps track
                # emission order), so V0 is not deferred.
                adv(ks[0], 4)                    # K0.tcc0 (wk0+xT0)
                for oc in range(4):
                    adv(qs[oc], 5)               # Q0-3 (wq0+xrT)
                adv(ks[0], 100)                  # K0 rest (xT1-3)
                for tcc in range(TC):
                    adv(v0s[tcc], 5)             # V0 (wv0+xT)
                # deferred: rest drained inside the attention pair loop.
                # Deadlines (6 drains/kcp, 48/pair): k1 by pair 1, v1 fully
                # emitted before pair 4's AV flush, k6/k7 by pairs 6/7.
                proj_stream.append(ks[1])
                proj_stream.extend(qs[4:8])
                proj_stream.append(ks[2])
                proj_stream.extend(v1s[0:4])
                proj_stream.append(ks[3])
                proj_stream.extend(v1s[4:8])
                proj_stream.append(ks[4])
                proj_stream.extend(v1s[8:12])
                proj_stream.append(ks[5])
                proj_stream.extend(v1s[12:16])
                proj_stream.extend([ks[6], ks[7]])

                prev_epi = None
                for pair in range(4):
                    prev_epi = attention(pair, prev_epi)
                # mid-attention: prefetch fc1 weights (SBUF freed by Q release
                # is modest; w1h/w1l fit alongside the attention working set)
                w1_pool = tc.alloc_tile_pool(name="w1_pool", bufs=1,
                                             side="right")
                w1h_t = w1_pool.tile([P, IC, DFF], F8E4)
                w1r_h = io["w1h"].rearrange("(i p) n -> p i n", p=P)
                for c in range(4):
                    nc.sync.dma_start(
                        w1h_t[:, :, c * 1024:(c + 1) * 1024],
                        w1r_h[:, :, c * 1024:(c + 1) * 1024])
                late_pools["w1h"] = (w1_pool, w1h_t)
                for pair in range(4, 6):
                    prev_epi = attention(pair, prev_epi)
                w1l_pool = tc.alloc_tile_pool(name="w1l_pool", bufs=1,
                                              side="right")
                w1l_t = w1l_pool.tile([P, IC, DFF], F8E5)
                w1r_l = io["w1l"].rearrange("(i p) n -> p i n", p=P)
                for c in range(4):
                    nc.sync.dma_start(
                        w1l_t[:, :, c * 1024:(c + 1) * 1024],
                        w1r_l[:, :, c * 1024:(c + 1) * 1024])
                late_pools["w1l"] = (w1l_pool, w1l_t)
                prev_epi = attention(6, prev_epi)
                drain_proj(1 << 30)

            # x_pool freed: prefetch xr + wo + w2h under attn 7 (right side)
            xrr_pool = tc.alloc_tile_pool(name="xrr_pool", bufs=1, side="right")
            xr_nat = xrr_pool.tile([P, 4, D], F32)
            nc.sync.dma_start(xr_nat[:],
                              io["xr"].rearrange("(rc p) d -> p rc d", p=P))
            wo_pool = tc.alloc_tile_pool(name="wo_pool", bufs=1, side="right")
            wo_t = wo_pool.tile([P, IC, D], F8E4)
            wor = io["wo"].rearrange("(i p) n -> p i n", p=P)
            nc.sync.dma_start(wo_t[:], wor[:])

            prev_epi = attention(7, prev_epi)
            prev_epi()

            # free the attention pools (non-LIFO: wo/w1 stay live)
            avpsum.release()
            spsum.release()
            rtile.release()
            ptile.release()
            kv_pool.release()

            w1h_t = late_pools["w1h"][1]
            w1l_t = late_pools["w1l"][1]

            if True:
                # ---- out-proj + LN1 + transpose (hi/lo split) ----
                # The critical path transposes the RAW normalized u (g1/be1
                # are folded into W1h/W1l/b1 host-side); the affine tbf
                # (= x1 + b2, the LN2 residual) is computed off-path.
                with tc.tile_pool(name="t_pool", bufs=1) as t_pool:
                    ubf = t_pool.tile([P, 4, D], BF16)     # LN1 u (pre-affine)
                    tbf = t_pool.tile([P, 4, D], BF16)     # x1 + b2 (residual)
                    tTh = t_pool.tile([P, IC, 512], F8E4)  # u^T hi
                    tTl = t_pool.tile([P, IC, 512], F8E5)  # u^T lo

                    # fc2 weights fit once the attention tiles are gone;
                    # DMA'd in fcp order so fc2 matmuls chase the transfers
                    w2h_pool = tc.alloc_tile_pool(name="w2h_pool", bufs=1)
                    w2h_t = w2h_pool.tile([P, FC, D], F8E4)
                    w2r_h = io["w2h"].rearrange("(f p) n -> p f n", p=P)
                    for grp in range(4):
                        nc.sync.dma_start(
                            w2h_t[:, grp * 8:(grp + 1) * 8, :],
                            w2r_h[:, grp * 8:(grp + 1) * 8, :])
                    w2l_pool = tc.alloc_tile_pool(name="w2l_pool", bufs=1)
                    w2l_t = w2l_pool.tile([P, FC, D], F8E5)
                    w2r_l = io["w2l"].rearrange("(f p) n -> p f n", p=P)
                    for grp in range(4):
                        nc.sync.dma_start(
                            w2l_t[:, grp * 8:(grp + 1) * 8, :],
                            w2r_l[:, grp * 8:(grp + 1) * 8, :])

                    with (
                        tc.tile_pool(name="lnt", bufs=2) as lnt,
                        tc.tile_pool(name="opsum", bufs=4, space="PSUM") as opsum,
                        tc.tile_pool(name="trpsum", bufs=4, space="PSUM") as trpsum,
                    ):
                        for rc in range(4):
                            acc = lnt.tile([P, D], F32, tag="acc")
                            for n2 in range(2):
                                pso = opsum.tile([P, 512], F32, tag="o")
                                for i2 in range(ICH):
                                    nc.tensor.matmul(
                                        pso[:],
                                        aoT8[:, 2 * i2:2 * i2 + 2,
                                             rc * P:(rc + 1) * P],
                                        wo_t[:, 2 * i2:2 * i2 + 2,
                                             n2 * 512:(n2 + 1) * 512],
                                        start=(i2 == 0), stop=(i2 == ICH - 1),
                                        perf_mode=DR)
                                nc.vector.tensor_tensor(
                                    out=acc[:, n2 * 512:(n2 + 1) * 512],
                                    in0=pso[:],
                                    in1=xr_nat[:, rc, n2 * 512:(n2 + 1) * 512],
                                    op=ALU.add)
                            _ln_u(nc, lnt, acc, eps_t, ubf[:, rc, :])
                            # critical path: transpose + hi/lo split of u
                            for ic in range(IC):
                                pst = trpsum.tile([P, P], BF16, tag="tr")
                                nc.tensor.transpose(
                                    pst[:], ubf[:, rc, ic * P:(ic + 1) * P],
                                    ident[:])
                                nc.scalar.copy(
                                    tTh[:, ic, rc * P:(rc + 1) * P], pst[:])
                                nc.vector.tensor_tensor(
                                    out=tTl[:, ic, rc * P:(rc + 1) * P],
                                    in0=pst[:],
                                    in1=tTh[:, ic, rc * P:(rc + 1) * P],
                                    op=ALU.subtract)
                        # off-path: residual tbf = u*g1 + (be1 + b2)
                        for rc in range(4):
                            nc.gpsimd.tensor_tensor(
                                out=tbf[:, rc, :], in0=ubf[:, rc, :],
                                in1=g1_t[:, :], op=ALU.mult)
                            nc.vector.tensor_tensor(
                                out=tbf[:, rc, :], in0=tbf[:, rc, :],
                                in1=be1_t[:, :], op=ALU.add)

                    wo_pool.release()
                    xrr_pool.release()

                    # ================= FFN =================
                    with (
                        tc.tile_pool(name="h_pool", bufs=1) as h_pool,
                        tc.tile_pool(name="tb_pool", bufs=2) as tb_pool,
                        tc.tile_pool(name="fpsum", bufs=2, space="PSUM") as fpsum,
                        tc.tile_pool(name="ypsum", bufs=3, space="PSUM") as ypsum,
                    ):
                        hh = h_pool.tile([P, FC, 512], F8E4)
                        hl = h_pool.tile([P, FC, 512], F8E5)
                        psy = {}
                        stop_tracker = {}

                        def fc2_mms(fcp, rcs, hx, wx, term):
                            for rc in rcs:
                                for n2 in range(2):
                                    key = (rc, n2)
                                    start = key not in stop_tracker
                                    stop_tracker[key] = True
                                    nc.tensor.matmul(
                                        psy[rc][:, n2, :],
                                        hx[:, 2 * fcp:2 * fcp + 2,
                                           rc * P:(rc + 1) * P],
                                        wx[:, 2 * fcp:2 * fcp + 2,
                                           n2 * 512:(n2 + 1) * 512],
                                        start=start, stop=False,
                                        perf_mode=DR, skip_group_check=True)

                        def fc2_mms_last(fcp, rcs):
                            for rc in rcs:
                                for n2 in range(2):
                                    nc.tensor.matmul(
                                        psy[rc][:, n2, :],
                                        hh[:, 2 * fcp:2 * fcp + 2,
                                           rc * P:(rc + 1) * P],
                                        w2l_t[:, 2 * fcp:2 * fcp + 2,
                                              n2 * 512:(n2 + 1) * 512],
                                        start=False, stop=True,
                                        perf_mode=DR, skip_group_check=True)

                        finbox = {}

                        def epilogue(rc):
                            fin = finbox["p"]
                            acc = fin.tile([P, D], F32, tag="acc2", bufs=2)
                            for n2 in range(2):
                                nc.vector.scalar_tensor_tensor(
                                    out=acc[:, n2 * 512:(n2 + 1) * 512],
                                    in0=psy[rc][:, n2, :],
                                    scalar=1.0 / W2_SCALE,
                                    in1=tbf[:, rc, n2 * 512:(n2 + 1) * 512],
                                    op0=ALU.mult, op1=ALU.add)
                            res = fin.tile([P, D], BF16, tag="res", bufs=2)
                            _layernorm(nc, fin, acc, eps_t, g2_t, be2_t,
                                       res[:], g_eng=nc.vector,
                                       b_eng=nc.vector)
                            nc.sync.dma_start(
                                out.rearrange("(rc p) d -> p rc d", p=P)[:, rc, :],
                                res[:])

                        # pass 1: fc1 + fc2 for rc 0,1,2 interleaved per fc;
                        # term3 (hh @ w2l) lags 6 fcp behind so the w2l DMA
                        # (which only starts after the attention pools free)
                        # has landed.
                        psy[0] = ypsum.tile([P, 2, 512], F32, tag="y", name="psy0")
                        psy[1] = ypsum.tile([P, 2, 512], F32, tag="y", name="psy1")
                        psy[2] = ypsum.tile([P, 2, 512], F32, tag="y", name="psy2")
                        LAG = 6
                        for fc in range(FC):
                            ps = fpsum.tile([P, 512], F32, tag="f1")
                            # rc-halves: the first half's operands (tT cols
                            # 0:256 = row chunks 0-1) are ready before the
                            # second, so fc1 can start while LN1/transpose
                            # of rc2-3 is still in flight.
                            for rh in range(2):
                                cols = slice(rh * 256, rh * 256 + 256)
                                h_first = True
                                for wt, xt in ((w1h_t, tTh), (w1h_t, tTl),
                                               (w1l_t, tTh)):
                                    for i2 in range(ICH):
                                        last = (xt is tTh and wt is w1l_t
                                                and i2 == ICH - 1)
                                        nc.tensor.matmul(
                                            ps[:, cols],
                                            wt[:, 2 * i2:2 * i2 + 2,
                                               fc * P:(fc + 1) * P],
                                            xt[:, 2 * i2:2 * i2 + 2, cols],
                                            start=h_first, stop=last,
                                            perf_mode=DR)
                                        h_first = False
                            tb = tb_pool.tile([P, 512], BF16, tag="tb")
                            nc.scalar.activation(out=tb[:], in_=ps[:],
                                                 func=AF.Gelu,
                                                 bias=b1_t[:, fc:fc + 1],
                                                 scale=1.0 / W1_SCALE)
                            nc.gpsimd.tensor_copy(out=hh[:, fc, :], in_=tb[:])
                            nc.vector.tensor_tensor(out=hl[:, fc, :],
                                                    in0=tb[:],
                                                    in1=hh[:, fc, :],
                                                    op=ALU.subtract)
                            if fc % 2 == 1:
                                fcp = fc // 2
                                fc2_mms(fcp, (0, 1, 2), hh, w2h_t, 1)
                                fc2_mms(fcp, (0, 1, 2), hl, w2h_t, 2)
                                if fcp >= LAG:
                                    if fcp - LAG == FCH - 1:
                                        fc2_mms_last(fcp - LAG, (0, 1, 2))
                                    else:
                                        fc2_mms(fcp - LAG, (0, 1, 2), hh,
                                                w2l_t, 3)
                        # fc1 weights are done with: free before the LN2
                        # epilogue scratch allocates (LIFO on the right
                        # stack: w1l, then w1h, then ao)
                        late_pools["w1l"][0].release()
                        late_pools["w1h"][0].release()
                        ao_pool.release()
                        for fcp in range(FCH - LAG, FCH):
                            if fcp == FCH - 1:
                                fc2_mms_last(fcp, (0, 1, 2))
                            else:
                                fc2_mms(fcp, (0, 1, 2), hh, w2l_t, 3)
                        finbox["p"] = tc.alloc_tile_pool(name="fin", bufs=1)
                        epilogue(0)
                        epilogue(1)
                        epilogue(2)
                        # pass 2: fc2 for rc3 (everything resident now)
                        psy[3] = ypsum.tile([P, 2, 512], F32, tag="y", name="psy3")
                        for fcp in range(FCH):
                            fc2_mms(fcp, (3,), hh, w2h_t, 1)
                            fc2_mms(fcp, (3,), hl, w2h_t, 2)
                            if fcp == FCH - 1:
                                fc2_mms_last(fcp, (3,))
                            else:
                                fc2_mms(fcp, (3,), hh, w2l_t, 3)
                        epilogue(3)
                        finbox["p"].release()
                    w2l_pool.release()
                    w2h_pool.release()


def _row_index(g):
    idx = np.empty(512, dtype=np.int64)
    r = 0
    for p in range(2):
        for s in range(2):
            j = 2 * p + s
            base = j * 512 + g * 128
            idx[r:r + 128] = np.arange(base, base + 128)
            r += 128
    return idx


def _mask_mq(g):
    """Causal indicator for the mask matmul: mq[m, i, d, q] = 1 iff the
    static -240*[k >= m] stationary, contracted against this column, yields
    -240*[k > q + (g - i)*128] (the masked region of the n0 block)."""
    mq = np.zeros((P, 4, 2, P), dtype=np.float32)
    for i in range(4):
        t = (g - i) * 128
        for q in range(P):
            tgt = q + t + 1
            if tgt < 0:
                tgt = 0
            if tgt <= P - 1:
                mq[tgt, i, :, q] = 1.0
    return mq


def _mask_mk():
    m = np.arange(P)[:, None]
    k = np.arange(P)[None, :]
    return np.where(k >= m, -240.0, 0.0).astype(np.float32)


def kernel(**inputs):
    if "nc" not in _CACHE:
        _CACHE["nc"] = _build()
    nc = _CACHE["nc"]

    bf = ml_dtypes.bfloat16
    e4 = ml_dtypes.float8_e4m3
    e5 = ml_dtypes.float8_e5m2
    x = np.asarray(inputs["x"], dtype=np.float32)

    def f32(k):
        return np.asarray(inputs[k], dtype=np.float32)

    wq8 = np.ascontiguousarray((WQK_SCALE * f32("Wq")).astype(e4))
    wk8 = np.ascontiguousarray((WQK_SCALE * f32("Wk")).astype(e4))
    wv8 = np.ascontiguousarray(f32("Wv").astype(e4))
    wo8 = np.ascontiguousarray(f32("Wo").astype(e4))
    vecs = {k: f32(k) for k in ("bq", "bk", "bv", "bo", "b1", "b2", "g1",
                                "be1", "g2", "be2")}
    # fold LN1's affine into the fc1 weights/bias: x1 @ W1 = u @ (g1*W1)
    # + be1 @ W1 (the raw normalized u is what gets transposed on-chip)
    w1s = W1_SCALE * (vecs["g1"][:, None] * f32("W1"))
    w1h = w1s.astype(e4)
    w1l = (w1s - w1h.astype(np.float32)).astype(e5)
    w1h, w1l = np.ascontiguousarray(w1h), np.ascontiguousarray(w1l)
    b1f = vecs["b1"] + vecs["be1"] @ f32("W1")
    w2s = W2_SCALE * f32("W2")
    w2h = w2s.astype(e4)
    w2l = (w2s - w2h.astype(np.float32)).astype(e5)
    w2h, w2l = np.ascontiguousarray(w2h), np.ascontiguousarray(w2l)
    # tbf on-chip computes u*g1 + be1f where be1f = be1 + b2 (the fc2 bias
    # rides along with the LN2 residual)
    be1f = vecs["be1"] + vecs["b2"]
    mk = _mask_mk().astype(bf)

    in_maps = []
    for c in range(N_CORES):
        b, g = c // 4, c % 4
        idx = _row_index(g)
        xb = x[b]
        xrows = xb[idx]
        in_maps.append({
            "xT": np.ascontiguousarray(xb.T.astype(e4)),
            "xrT": np.ascontiguousarray(xrows.T.astype(e4)),
            "xr": np.ascontiguousarray(xrows + vecs["bo"][None, :]),
            "wq": wq8, "wk": wk8, "wv": wv8, "wo": wo8,
            "w1h": w1h, "w1l": w1l, "w2h": w2h, "w2l": w2l,
            "bq": vecs["bq"], "bk": vecs["bk"],
            "bv": vecs["bv"].astype(bf),
            "b1": b1f,
            "g1": vecs["g1"].astype(bf), "be1": be1f.astype(bf),
            "g2": vecs["g2"].astype(bf), "be2": vecs["be2"].astype(bf),
            "mq": _mask_mq(g).astype(bf),
            "mk": mk,
        })

    res = run_bass_kernel_spmd(nc, in_maps, core_ids=list(range(N_CORES)))
    _CACHE["last_result"] = res

    outp = np.empty((B, L, D), dtype=np.float32)
    for c in range(N_CORES):
        b, g = c // 4, c % 4
        outp[b][_row_index(g)] = res.results[c]["out"].astype(np.float32)
    return outp


# revision 36
# speedup vs baseline: 1.3788x; 1.0055x over previous
"""Trainium2 Bass kernel for AttentionFFNBlock (B=2, L=2048, D=1024, H=16, FF=4096).

Sharding (8 cores, zero cross-core communication):
  core c -> batch b = c//4, group slot g = c%4.
  Each core owns 512 query rows of its batch, interleaved in 128-row blocks
  for causal load balance: global row = (2p+s)*512 + g*128 + i for local row
  r = p*256 + s*128 + i.  The core computes K/V for the full sequence
  (replicated inside the batch group), attention for its rows over all 16
  heads, then out-proj + LN1 + FFN + LN2 for its rows only.

FP8 design (cost model: DoubleRow fp8 matmul = 0.5 cycles/row with 2x128
contraction -> 4x bf16 MAC throughput):
  - Q/K/V/out projections run as fp8e4m3 DoubleRow matmuls. wq/wk are scaled
    16x host-side (their sigma=1/32 sits in e4m3's subnormal range); the
    1/16 descale folds into the psum-drain tensor_scalar for free.
  - Scores stay bf16 (kT/qT bf16).  Causality is enforced PRE-exp by one
    extra bf16 matmul per (pair, kc): a static lower-triangular [k>=m]*-240
    stationary against a per-core indicator moving operand adds -240 exactly
    where key > query.  No per-element mask multiplies on DVE/Pool at all.
  - Softmax: pt = exp(s/8 - 2) written by ACT directly as fp8e4m3 (max logit
    ~6.5 -> max pt ~95 < 240).  The denominator comes from the ones column of
    v8 through the same AV matmul, so quantization of pt largely cancels.
  - AV and out-proj are fp8 DoubleRow (v8 / aoT8 in e4m3).
  - FFN is 3-term split fp8: W ~ (Wh + Wl)/s with Wh=e4m3(s*W) and
    Wl=e5m2(s*W - Wh) (s=16 for W1, 64 for W2 - avoids e4m3 subnormal
    flush), activations split hi=e4m3(a), lo=e5m2(a - hi). Terms
    ah@Wh + al@Wh + ah@Wl accumulate in one psum group: 0.75x the bf16
    cost with ~bf16 accuracy.  Descale 1/16 folds into the Gelu activation
    scale; 1/64 into the fc2 drain tensor_scalar.

Measured numpy end-to-end rel err of this exact scheme: 7.8e-3 (gate 2e-2).
"""

import numpy as np
import ml_dtypes

import concourse.bass as bass
import concourse.mybir as mybir
import concourse.tile as tile
from concourse import bacc
from concourse.bass_utils import run_bass_kernel_spmd
from concourse.masks import make_identity

F32 = mybir.dt.float32
BF16 = mybir.dt.bfloat16
F8E4 = mybir.dt.float8e4
F8E5 = mybir.dt.float8e5
AF = mybir.ActivationFunctionType
ALU = mybir.AluOpType
DR = mybir.MatmulPerfMode.DoubleRow

N_CORES = 8
B, L, D = 2, 2048, 1024
H, HD = 16, 64
DFF = 4096
EPS = 1e-5
P = 128

IC = D // P        # 8 contraction chunks of the model dim
ICH = IC // 2      # 4 DoubleRow chunks (256 contraction each)
TC = L // P        # 16 token chunks
FC = DFF // P      # 32 ff chunks
FCH = FC // 2      # 16 DoubleRow ff chunks
NPAIR = 8          # head pairs (= oc chunks)

WQK_SCALE = 16.0   # wq/wk quantized from 16*W
W1_SCALE = 16.0
W2_SCALE = 64.0

_CACHE = {}


def _build():
    nc = bacc.Bacc("TRN2", target_bir_lowering=False, debug=False,
                   num_devices=N_CORES)

    def din(name, shape, dt=F32):
        return nc.dram_tensor(name, shape, dt, kind="ExternalInput").ap()

    io = dict(
        xT=din("xT", [D, L], F8E4),               # x[b]^T (K/V source)
        xrT=din("xrT", [D, 512], F8E4),           # owned rows^T (Q source)
        xr=din("xr", [512, D], F32),              # owned rows (residual)
        wq=din("wq", [D, D], F8E4), wk=din("wk", [D, D], F8E4),
        wv=din("wv", [D, D], F8E4), wo=din("wo", [D, D], F8E4),
        w1h=din("w1h", [D, DFF], F8E4), w1l=din("w1l", [D, DFF], F8E5),
        w2h=din("w2h", [DFF, D], F8E4), w2l=din("w2l", [DFF, D], F8E5),
        bq=din("bq", [D]), bk=din("bk", [D]), bv=din("bv", [D], BF16),
        b1=din("b1", [DFF]),
        g1=din("g1", [D], BF16), be1=din("be1", [D], BF16),
        g2=din("g2", [D], BF16), be2=din("be2", [D], BF16),
        mq=din("mq", [P, 4, 2, P], BF16),         # causal indicator (per-core)
        mk=din("mk", [P, P], BF16),               # static -240 * [k >= m]
        out=nc.dram_tensor("out", [512, D], BF16, kind="ExternalOutput").ap(),
    )

    with tile.TileContext(nc) as tc:
        _emit(nc, tc, io)
    nc.compile()
    return nc


def _ln_u(nc, pool, acc, eps_t, out_u):
    """Normalize (no affine) over the free axis of acc [128, 1024] -> out_u."""
    stats = pool.tile([P, 2, 6], F32, tag="ln_stats")
    for sg in range(2):
        nc.vector.bn_stats(out=stats[:, sg, :], in_=acc[:, sg * 512:(sg + 1) * 512])
    mv = pool.tile([P, 2], F32, tag="ln_mv")
    nc.vector.bn_aggr(out=mv[:], in_=stats[:])
    rstd = pool.tile([P, 1], F32, tag="ln_rstd")
    nc.scalar.activation(out=rstd[:], in_=mv[:, 1:2], func=AF.Sqrt,
                         bias=eps_t[:], scale=1.0)
    nc.vector.reciprocal(out=rstd[:], in_=rstd[:])
    nmr = pool.tile([P, 1], F32, tag="ln_nmr")
    nc.vector.tensor_scalar(out=nmr[:], in0=mv[:, 0:1], scalar1=rstd[:],
                            scalar2=-1.0, op0=ALU.mult, op1=ALU.mult)
    nc.scalar.activation(out=out_u, in_=acc[:], func=AF.Identity,
                         bias=nmr[:], scale=rstd[:])


def _layernorm(nc, pool, acc, eps_t, g_t, b_t, out_ap, g_eng=None,
               b_eng=None):
    """LayerNorm over the free axis (D=1024) of acc [128, 1024] -> out_ap."""
    u = pool.tile([P, D], BF16, tag="ln_u")
    _ln_u(nc, pool, acc, eps_t, u[:])
    (g_eng or nc.gpsimd).tensor_tensor(out=u[:], in0=u[:], in1=g_t[:, :],
                                       op=ALU.mult)
    (b_eng or nc.vector).tensor_tensor(out=out_ap, in0=u[:], in1=b_t[:, :],
                                       op=ALU.add)


def _emit(nc, tc, io):
    out = io["out"]

    with tc.tile_pool(name="const", bufs=1) as const:
        ao_pool = tc.alloc_tile_pool(name="ao_pool", bufs=1, side="right")
        # ---- constants / biases (tiles now; DMAs deferred past wk/xT) ----
        bq_t = const.tile([P, IC], F32)
        bk_t = const.tile([P, IC], F32)
        b1_t = const.tile([P, FC], F32)
        # bo is folded into xr host-side; b2 into be1 (tbf = x1 + b2);
        # g1/be1 into W1h/W1l/b1 for the fc1 path.
        row_vecs = {}
        for nm in ("bv", "g1", "be1", "g2", "be2"):
            rv = const.tile([P, D], BF16, name=f"cv_{nm}")
            row_vecs[nm] = rv
        bv_t = row_vecs["bv"]
        g1_t, be1_t = row_vecs["g1"], row_vecs["be1"]
        g2_t, be2_t = row_vecs["g2"], row_vecs["be2"]
        mq_t = const.tile([P, 4, 2, P], BF16)
        mk_t = const.tile([P, P], BF16)
        eps_t = const.tile([P, 1], F32)
        neg2_t = const.tile([P, 1], F32)
        ident = const.tile([P, P], BF16)

        def tiny_dmas():
            nc.sync.dma_start(bk_t[:], io["bk"].rearrange("(o p) -> p o", p=P))
            nc.sync.dma_start(bq_t[:], io["bq"].rearrange("(o p) -> p o", p=P))
            nc.sync.dma_start(mq_t[:], io["mq"])
            nc.sync.dma_start(mk_t[:], io["mk"])
            nc.vector.memset(eps_t[:], EPS)
            nc.vector.memset(neg2_t[:], -2.0)

        def early_dmas():
            nc.sync.dma_start(b1_t[:], io["b1"].rearrange("(f p) -> p f", p=P))
            nc.sync.dma_start(row_vecs["bv"][:],
                              io["bv"][None, :].to_broadcast([P, D]))

        def const_dmas():
            for nm in ("g1", "be1", "g2", "be2"):
                nc.sync.dma_start(row_vecs[nm][:],
                                  io[nm][None, :].to_broadcast([P, D]))
            make_identity(nc, ident[:])

        aoT8 = ao_pool.tile([P, IC, 512], F8E4)   # attention output^T (fp8)

        kv_pool = tc.alloc_tile_pool(name="kv_pool", bufs=1)
        ptile = tc.alloc_tile_pool(name="ptile", bufs=7)
        rtile = tc.alloc_tile_pool(name="rtile", bufs=2)
        spsum = tc.alloc_tile_pool(name="spsum", bufs=2, space="PSUM")
        avpsum = tc.alloc_tile_pool(name="avpsum", bufs=1, space="PSUM")
        if True:
            kT = kv_pool.tile([P, IC, L], BF16)
            v8 = kv_pool.tile([P, TC, H, HD + 1], F8E4)
            qT = kv_pool.tile([P, IC, 512], BF16)
            nc.vector.memset(v8[:, :, :, HD:], 1.0)

            proj_stream = []   # deferred (emit_mms, epilogue) generators

            def drain_proj(n):
                """Emit up to n deferred projection matmuls."""
                while n > 0 and proj_stream:
                    gen = proj_stream[0]
                    try:
                        next(gen)
                        n -= 1
                    except StopIteration:
                        proj_stream.pop(0)

            # prefetch pools for FFN weights, allocated mid-attention
            late_pools = {}

            def attention(pair, prev_epi=None, prev_flush=None):
                hA, hB = 2 * pair, 2 * pair + 1
                pavA = avpsum.tile([HD + 1, 512], F32, tag="avA")
                pavB = avpsum.tile([HD + 1, 512], F32, tag="avB")
                pts = []

                def emit_av(ent, last):
                    pkcp, pn0, ppt = ent
                    for j, (h, pav) in enumerate(((hA, pavA), (hB, pavB))):
                        nc.tensor.matmul(
                            pav[:, pn0:512],
                            v8[:, 2 * pkcp:2 * pkcp + 2, h, :],
                            ppt[:, j, :, pn0:512],
                            start=(pkcp == 0), stop=last,
                            perf_mode=DR, skip_group_check=True)

                # exp-feeding matmuls are emitted densely (scores+masks for
                # both kc of the pair back to back) so ACT never waits on
                # drain/AV filler sitting in the in-order PE queue.
                for kcp in range(8):
                    j0 = kcp // 2
                    n0 = j0 * P
                    pt = ptile.tile([P, 2, 2, 512], F8E4, tag="p")
                    for t in range(2):
                        kc = 2 * kcp + t
                        ps = spsum.tile([P, 2, 512], F32, tag="s")
                        nc.tensor.matmul(
                            ps[:, 0, n0:512],
                            kT[0:HD, pair, kc * P:(kc + 1) * P],
                            qT[0:HD, pair, n0:512], start=True, stop=True)
                        nc.tensor.matmul(
                            ps[:, 1, n0:512],
                            kT[HD:P, pair, kc * P:(kc + 1) * P],
                            qT[HD:P, pair, n0:512], start=True, stop=True)
                        for j in range(2):
                            nc.tensor.matmul(
                                ps[:, j, n0:n0 + P], mk_t[:],
                                mq_t[:, kc % 4, j, :], start=False,
                                stop=False, skip_group_check=True)
                        nc.scalar.activation(out=pt[:, :, t, n0:512],
                                             in_=ps[:, :, n0:512],
                                             func=AF.Exp, scale=0.125,
                                             bias=neg2_t[:])
                    pts.append((kcp, n0, pt))
                    if kcp == 0 and prev_flush is not None:
                        prev_flush()
                    if kcp == 1 and prev_epi is not None:
                        prev_epi()
                    if len(pts) >= 4:
                        emit_av(pts.pop(0), last=False)
                    drain_proj(6 if pair < 6 else 2)

                def flush():
                    while pts:
                        emit_av(pts.pop(0), last=(not pts))

                def epi():
                    for hp, pav in ((0, pavA), (HD, pavB)):
                        rec = rtile.tile([1, 512], F32, tag="rec")
                        nc.vector.reciprocal(rec[:], pav[HD:HD + 1, :])
                        rec_b = rtile.tile([HD, 512], F32, tag="rec_b")
                        nc.gpsimd.partition_broadcast(rec_b[:], rec[0:1, :])
                        nc.vector.tensor_tensor(
                            out=aoT8[hp:hp + HD, pair, :],
                            in0=pav[:HD, :], in1=rec_b[:], op=ALU.mult)
                return epi, flush

            # ---- projections (pairs 0..6 overlap with x_pool live) ----
            with (
                tc.tile_pool(name="x_pool", bufs=1) as x_pool,
                tc.tile_pool(name="ppsum", bufs=2, space="PSUM") as ppsum,
            ):
                wk_t = x_pool.tile([P, IC, D], F8E4)
                xT_t = x_pool.tile([P, IC, L], F8E4)
                wq_t = x_pool.tile([P, IC, D], F8E4)
                xrT_t = x_pool.tile([P, IC, 512], F8E4)
                wv_t = x_pool.tile([P, IC, D], F8E4)
                wkr = io["wk"].rearrange("(i p) n -> p i n", p=P)
                wqr = io["wq"].rearrange("(i p) n -> p i n", p=P)
                wvr = io["wv"].rearrange("(i p) n -> p i n", p=P)
                xTr = io["xT"].rearrange("(i p) n -> p i n", p=P)
                nc.sync.dma_start(wk_t[:, :, 0:P], wkr[:, :, 0:P])
                tiny_dmas()
                nc.sync.dma_start(xT_t[:, 0:4, 0:512], xTr[:, 0:4, 0:512])
                nc.sync.dma_start(xT_t[:, 4:8, 0:512], xTr[:, 4:8, 0:512])
                nc.sync.dma_start(wq_t[:, :, 0:P], wqr[:, :, 0:P])
                nc.sync.dma_start(xrT_t[:],
                                  io["xrT"].rearrange("(i p) n -> p i n", p=P))
                nc.sync.dma_start(wq_t[:, :, P:512], wqr[:, :, P:512])
                nc.sync.dma_start(wv_t[:, :, 0:512], wvr[:, :, 0:512])
                early_dmas()
                nc.sync.dma_start(xT_t[:, :, 512:1024], xTr[:, :, 512:1024])
                nc.sync.dma_start(wk_t[:, :, P:512], wkr[:, :, P:512])
                nc.sync.dma_start(xT_t[:, :, 1024:1536], xTr[:, :, 1024:1536])
                nc.sync.dma_start(xT_t[:, :, 1536:2048], xTr[:, :, 1536:2048])
                nc.sync.dma_start(wk_t[:, :, 512:1024], wkr[:, :, 512:1024])
                nc.sync.dma_start(wq_t[:, :, 512:1024], wqr[:, :, 512:1024])
                const_dmas()
                nc.sync.dma_start(wv_t[:, :, 512:1024], wvr[:, :, 512:1024])

                def k_proj(oc):
                    for tcc in range(4):
                        ps = ppsum.tile([P, 512], F32, tag="proj")
                        for i2 in range(ICH):
                            nc.tensor.matmul(
                                ps[:],
                                wk_t[:, 2 * i2:2 * i2 + 2, oc * P:(oc + 1) * P],
                                xT_t[:, 2 * i2:2 * i2 + 2,
                                     tcc * 512:(tcc + 1) * 512],
                                start=(i2 == 0), stop=(i2 == ICH - 1),
                                perf_mode=DR)
                            yield
                        nc.vector.tensor_scalar(
                            out=kT[:, oc, tcc * 512:(tcc + 1) * 512],
                            in0=ps[:], scalar1=1.0 / WQK_SCALE,
                            scalar2=bk_t[:, oc:oc + 1],
                            op0=ALU.mult, op1=ALU.add)

                def q_proj(oc):
                    ps = ppsum.tile([P, 512], F32, tag="proj")
                    for i2 in range(ICH):
                        nc.tensor.matmul(
                            ps[:],
                            wq_t[:, 2 * i2:2 * i2 + 2, oc * P:(oc + 1) * P],
                            xrT_t[:, 2 * i2:2 * i2 + 2, :],
                            start=(i2 == 0), stop=(i2 == ICH - 1),
                            perf_mode=DR)
                        yield
                    nc.vector.tensor_scalar(
                        out=qT[:, oc, :], in0=ps[:], scalar1=1.0 / WQK_SCALE,
                        scalar2=bq_t[:, oc:oc + 1], op0=ALU.mult, op1=ALU.add)

                def v_proj(tcc, hf):
                    ps = ppsum.tile([P, 512], F32, tag="proj")
                    for i2 in range(ICH):
                        nc.tensor.matmul(
                            ps[:],
                            xT_t[:, 2 * i2:2 * i2 + 2, tcc * P:(tcc + 1) * P],
                            wv_t[:, 2 * i2:2 * i2 + 2,
                                 hf * 512:(hf + 1) * 512],
                            start=(i2 == 0), stop=(i2 == ICH - 1),
                            perf_mode=DR)
                        yield
                    nc.vector.tensor_tensor(
                        out=v8[:, tcc, hf * 8:(hf + 1) * 8, :HD],
                        in0=ps.rearrange("p (h d) -> p h d", d=HD),
                        in1=bv_t[:, hf * 512:(hf + 1) * 512]
                        .rearrange("p (h d) -> p h d", d=HD),
                        op=ALU.add)

                def adv(gen, n):
                    for _ in range(n):
                        try:
                            next(gen)
                        except StopIteration:
                            return

                ks = [k_proj(oc) for oc in range(IC)]
                qs = [q_proj(oc) for oc in range(IC)]
                v0s = [v_proj(tcc, 0) for tcc in range(TC)]
                v1s = [v_proj(tcc, 1) for tcc in range(TC)]
                # upfront, ordered to match serial DMA arrival.  All of V0
                # must be EMITTED before pair 0's AV flush (tile deps track
                # emission order), so V0 is not deferred.
                adv(ks[0], 4)                    # K0.tcc0 (wk0+xT0)
                for oc in range(4):
                    adv(qs[oc], 5)               # Q0-3 (wq0+xrT)
                adv(ks[0], 100)                  # K0 rest (xT1-3)
                for tcc in range(TC):
                    adv(v0s[tcc], 5)             # V0 (wv0+xT)
                # deferred: rest drained inside the attention pair loop.
                # Deadlines (6 drains/kcp, 48/pair): k1 by pair 1, v1 fully
                # emitted before pair 4's AV flush, k6/k7 by pairs 6/7.
                proj_stream.append(ks[1])
                proj_stream.extend(qs[4:8])
                proj_stream.append(ks[2])
                proj_stream.extend(v1s[0:4])
                proj_stream.append(ks[3])
                proj_stream.extend(v1s[4:8])
                proj_stream.append(ks[4])
                proj_stream.extend(v1s[8:12])
                proj_stream.append(ks[5])
                proj_stream.extend(v1s[12:16])
                proj_stream.extend([ks[6], ks[7]])

                prev_epi = prev_flush = None
                for pair in range(4):
                    prev_epi, prev_flush = attention(pair, prev_epi,
                                                     prev_flush)
                # mid-attention: prefetch fc1 weights (SBUF freed by Q release
                # is modest; w1h/w1l fit alongside the attention working set)
                w1_pool = tc.alloc_tile_pool(name="w1_pool", bufs=1,
                                             side="right")
                w1h_t = w1_pool.tile([P, IC, DFF], F8E4)
                w1r_h = io["w1h"].rearrange("(i p) n -> p i n", p=P)
                for c in range(4):
                    nc.sync.dma_start(
                        w1h_t[:, :, c * 1024:(c + 1) * 1024],
                        w1r_h[:, :, c * 1024:(c + 1) * 1024])
                late_pools["w1h"] = (w1_pool, w1h_t)
                for pair in range(4, 6):
                    prev_epi, prev_flush = attention(pair, prev_epi,
                                                     prev_flush)
                w1l_pool = tc.alloc_tile_pool(name="w1l_pool", bufs=1,
                                              side="right")
                w1l_t = w1l_pool.tile([P, IC, DFF], F8E5)
                w1r_l = io["w1l"].rearrange("(i p) n -> p i n", p=P)
                for c in range(4):
                    nc.sync.dma_start(
                        w1l_t[:, :, c * 1024:(c + 1) * 1024],
                        w1r_l[:, :, c * 1024:(c + 1) * 1024])
                late_pools["w1l"] = (w1l_pool, w1l_t)
                prev_epi, prev_flush = attention(6, prev_epi, prev_flush)
                drain_proj(1 << 30)

            # x_pool freed: prefetch xr + wo + w2h under attn 7 (right side)
            xrr_pool = tc.alloc_tile_pool(name="xrr_pool", bufs=1, side="right")
            xr_nat = xrr_pool.tile([P, 4, D], F32)
            nc.sync.dma_start(xr_nat[:],
                              io["xr"].rearrange("(rc p) d -> p rc d", p=P))
            wo_pool = tc.alloc_tile_pool(name="wo_pool", bufs=1, side="right")
            wo_t = wo_pool.tile([P, IC, D], F8E4)
            wor = io["wo"].rearrange("(i p) n -> p i n", p=P)
            nc.sync.dma_start(wo_t[:], wor[:])

            prev_epi, prev_flush = attention(7, prev_epi, prev_flush)
            prev_flush()
            prev_epi()

            # free the attention pools (non-LIFO: wo/w1 stay live)
            avpsum.release()
            spsum.release()
            rtile.release()
            ptile.release()
            kv_pool.release()

            w1h_t = late_pools["w1h"][1]
            w1l_t = late_pools["w1l"][1]

            if True:
                # ---- out-proj + LN1 + transpose (hi/lo split) ----
                # The critical path transposes the RAW normalized u (g1/be1
                # are folded into W1h/W1l/b1 host-side); the affine tbf
                # (= x1 + b2, the LN2 residual) is computed off-path.
                with tc.tile_pool(name="t_pool", bufs=1) as t_pool:
                    ubf = t_pool.tile([P, 4, D], BF16)     # LN1 u (pre-affine)
                    tbf = t_pool.tile([P, 4, D], BF16)     # x1 + b2 (residual)
                    tTh = t_pool.tile([P, IC, 512], F8E4)  # u^T hi
                    tTl = t_pool.tile([P, IC, 512], F8E5)  # u^T lo

                    # fc2 weights fit once the attention tiles are gone;
                    # DMA'd in fcp order so fc2 matmuls chase the transfers
                    w2h_pool = tc.alloc_tile_pool(name="w2h_pool", bufs=1)
                    w2h_t = w2h_pool.tile([P, FC, D], F8E4)
                    w2r_h = io["w2h"].rearrange("(f p) n -> p f n", p=P)
                    for grp in range(4):
                        nc.sync.dma_start(
                            w2h_t[:, grp * 8:(grp + 1) * 8, :],
                            w2r_h[:, grp * 8:(grp + 1) * 8, :])
                    w2l_pool = tc.alloc_tile_pool(name="w2l_pool", bufs=1)
                    w2l_t = w2l_pool.tile([P, FC, D], F8E5)
                    w2r_l = io["w2l"].rearrange("(f p) n -> p f n", p=P)
                    for grp in range(4):
                        nc.sync.dma_start(
                            w2l_t[:, grp * 8:(grp + 1) * 8, :],
                            w2r_l[:, grp * 8:(grp + 1) * 8, :])

                    with (
                        tc.tile_pool(name="lnt", bufs=2) as lnt,
                        tc.tile_pool(name="opsum", bufs=4, space="PSUM") as opsum,
                        tc.tile_pool(name="trpsum", bufs=4, space="PSUM") as trpsum,
                    ):
                        for rc in range(4):
                            acc = lnt.tile([P, D], F32, tag="acc")
                            for n2 in range(2):
                                pso = opsum.tile([P, 512], F32, tag="o")
                                for i2 in range(ICH):
                                    nc.tensor.matmul(
                                        pso[:],
                                        aoT8[:, 2 * i2:2 * i2 + 2,
                                             rc * P:(rc + 1) * P],
                                        wo_t[:, 2 * i2:2 * i2 + 2,
                                             n2 * 512:(n2 + 1) * 512],
                                        start=(i2 == 0), stop=(i2 == ICH - 1),
                                        perf_mode=DR)
                                nc.vector.tensor_tensor(
                                    out=acc[:, n2 * 512:(n2 + 1) * 512],
                                    in0=pso[:],
                                    in1=xr_nat[:, rc, n2 * 512:(n2 + 1) * 512],
                                    op=ALU.add)
                            _ln_u(nc, lnt, acc, eps_t, ubf[:, rc, :])
                            # critical path: transpose + hi/lo split of u
                            for ic in range(IC):
                                pst = trpsum.tile([P, P], BF16, tag="tr")
                                nc.tensor.transpose(
                                    pst[:], ubf[:, rc, ic * P:(ic + 1) * P],
                                    ident[:])
                                nc.scalar.copy(
                                    tTh[:, ic, rc * P:(rc + 1) * P], pst[:])
                                nc.vector.tensor_tensor(
                                    out=tTl[:, ic, rc * P:(rc + 1) * P],
                                    in0=pst[:],
                                    in1=tTh[:, ic, rc * P:(rc + 1) * P],
                                    op=ALU.subtract)
                        # off-path: residual tbf = u*g1 + (be1 + b2)
                        for rc in range(4):
                            nc.gpsimd.tensor_tensor(
                                out=tbf[:, rc, :], in0=ubf[:, rc, :],
                                in1=g1_t[:, :], op=ALU.mult)
                            nc.vector.tensor_tensor(
                                out=tbf[:, rc, :], in0=tbf[:, rc, :],
                                in1=be1_t[:, :], op=ALU.add)

                    wo_pool.release()
                    xrr_pool.release()

                    # ================= FFN =================
                    with (
                        tc.tile_pool(name="h_pool", bufs=1) as h_pool,
                        tc.tile_pool(name="tb_pool", bufs=2) as tb_pool,
                        tc.tile_pool(name="fpsum", bufs=2, space="PSUM") as fpsum,
                        tc.tile_pool(name="ypsum", bufs=3, space="PSUM") as ypsum,
                    ):
                        hh = h_pool.tile([P, FC, 512], F8E4)
                        hl = h_pool.tile([P, FC, 512], F8E5)
                        psy = {}
                        stop_tracker = {}

                        def fc2_mms(fcp, rcs, hx, wx, term):
                            for rc in rcs:
                                for n2 in range(2):
                                    key = (rc, n2)
                                    start = key not in stop_tracker
                                    stop_tracker[key] = True
                                    nc.tensor.matmul(
                                        psy[rc][:, n2, :],
                                        hx[:, 2 * fcp:2 * fcp + 2,
                                           rc * P:(rc + 1) * P],
                                        wx[:, 2 * fcp:2 * fcp + 2,
                                           n2 * 512:(n2 + 1) * 512],
                                        start=start, stop=False,
                                        perf_mode=DR, skip_group_check=True)

                        def fc2_mms_last(fcp, rcs):
                            for rc in rcs:
                                for n2 in range(2):
                                    nc.tensor.matmul(
                                        psy[rc][:, n2, :],
                                        hh[:, 2 * fcp:2 * fcp + 2,
                                           rc * P:(rc + 1) * P],
                                        w2l_t[:, 2 * fcp:2 * fcp + 2,
                                              n2 * 512:(n2 + 1) * 512],
                                        start=False, stop=True,
                                        perf_mode=DR, skip_group_check=True)

                        finbox = {}

                        def epilogue(rc):
                            fin = finbox["p"]
                            acc = fin.tile([P, D], F32, tag="acc2", bufs=2)
                            for n2 in range(2):
                                nc.vector.scalar_tensor_tensor(
                                    out=acc[:, n2 * 512:(n2 + 1) * 512],
                                    in0=psy[rc][:, n2, :],
                                    scalar=1.0 / W2_SCALE,
                                    in1=tbf[:, rc, n2 * 512:(n2 + 1) * 512],
                                    op0=ALU.mult, op1=ALU.add)
                            res = fin.tile([P, D], BF16, tag="res", bufs=2)
                            _layernorm(nc, fin, acc, eps_t, g2_t, be2_t,
                                       res[:], g_eng=nc.vector,
                                       b_eng=nc.vector)
                            nc.sync.dma_start(
                                out.rearrange("(rc p) d -> p rc d", p=P)[:, rc, :],
                                res[:])

                        # pass 1: fc1 + fc2 for rc 0,1,2 interleaved per fc;
                        # term3 (hh @ w2l) lags 6 fcp behind so the w2l DMA
                        # (which only starts after the attention pools free)
                        # has landed.
                        psy[0] = ypsum.tile([P, 2, 512], F32, tag="y", name="psy0")
                        psy[1] = ypsum.tile([P, 2, 512], F32, tag="y", name="psy1")
                        psy[2] = ypsum.tile([P, 2, 512], F32, tag="y", name="psy2")
                        LAG = 6
                        for fc in range(FC):
                            ps = fpsum.tile([P, 512], F32, tag="f1")
                            # rc-halves: the first half's operands (tT cols
                            # 0:256 = row chunks 0-1) are ready before the
                            # second, so fc1 can start while LN1/transpose
                            # of rc2-3 is still in flight.
                            for rh in range(2):
                                cols = slice(rh * 256, rh * 256 + 256)
                                h_first = True
                                for wt, xt in ((w1h_t, tTh), (w1h_t, tTl),
                                               (w1l_t, tTh)):
                                    for i2 in range(ICH):
                                        last = (xt is tTh and wt is w1l_t
                                                and i2 == ICH - 1)
                                        nc.tensor.matmul(
                                            ps[:, cols],
                                            wt[:, 2 * i2:2 * i2 + 2,
                                               fc * P:(fc + 1) * P],
                                            xt[:, 2 * i2:2 * i2 + 2, cols],
                                            start=h_first, stop=last,
                                            perf_mode=DR)
                                        h_first = False
                            tb = tb_pool.tile([P, 512], BF16, tag="tb")
                            nc.scalar.activation(out=tb[:], in_=ps[:],
                                                 func=AF.Gelu,
                                                 bias=b1_t[:, fc:fc + 1],
                                                 scale=1.0 / W1_SCALE)
                            nc.gpsimd.tensor_copy(out=hh[:, fc, :], in_=tb[:])
                            nc.vector.tensor_tensor(out=hl[:, fc, :],
                                                    in0=tb[:],
                                                    in1=hh[:, fc, :],
                                                    op=ALU.subtract)
                            if fc % 2 == 1:
                                fcp = fc // 2
                                fc2_mms(fcp, (0, 1, 2), hh, w2h_t, 1)
                                fc2_mms(fcp, (0, 1, 2), hl, w2h_t, 2)
                                if fcp >= LAG:
                                    if fcp - LAG == FCH - 1:
                                        fc2_mms_last(fcp - LAG, (0, 1, 2))
                                    else:
                                        fc2_mms(fcp - LAG, (0, 1, 2), hh,
                                                w2l_t, 3)
                        # fc1 weights are done with: free before the LN2
                        # epilogue scratch allocates (LIFO on the right
                        # stack: w1l, then w1h, then ao)
                        late_pools["w1l"][0].release()
                        late_pools["w1h"][0].release()
                        ao_pool.release()
                        for fcp in range(FCH - LAG, FCH):
                            if fcp == FCH - 1:
                                fc2_mms_last(fcp, (0, 1, 2))
                            else:
                                fc2_mms(fcp, (0, 1, 2), hh, w2l_t, 3)
                        finbox["p"] = tc.alloc_tile_pool(name="fin", bufs=1)
                        epilogue(0)
                        epilogue(1)
                        epilogue(2)
                        # pass 2: fc2 for rc3 (everything resident now)
                        psy[3] = ypsum.tile([P, 2, 512], F32, tag="y", name="psy3")
                        for fcp in range(FCH):
                            fc2_mms(fcp, (3,), hh, w2h_t, 1)
                            fc2_mms(fcp, (3,), hl, w2h_t, 2)
                            if fcp == FCH - 1:
                                fc2_mms_last(fcp, (3,))
                            else:
                                fc2_mms(fcp, (3,), hh, w2l_t, 3)
                        epilogue(3)
                        finbox["p"].release()
                    w2l_pool.release()
                    w2h_pool.release()


def _row_index(g):
    idx = np.empty(512, dtype=np.int64)
    r = 0
    for p in range(2):
        for s in range(2):
            j = 2 * p + s
            base = j * 512 + g * 128
            idx[r:r + 128] = np.arange(base, base + 128)
            r += 128
    return idx


def _mask_mq(g):
    """Causal indicator for the mask matmul: mq[m, i, d, q] = 1 iff the
    static -240*[k >= m] stationary, contracted against this column, yields
    -240*[k > q + (g - i)*128] (the masked region of the n0 block)."""
    mq = np.zeros((P, 4, 2, P), dtype=np.float32)
    for i in range(4):
        t = (g - i) * 128
        for q in range(P):
            tgt = q + t + 1
            if tgt < 0:
                tgt = 0
            if tgt <= P - 1:
                mq[tgt, i, :, q] = 1.0
    return mq


def _mask_mk():
    m = np.arange(P)[:, None]
    k = np.arange(P)[None, :]
    return np.where(k >= m, -240.0, 0.0).astype(np.float32)


def kernel(**inputs):
    if "nc" not in _CACHE:
        _CACHE["nc"] = _build()
    nc = _CACHE["nc"]

    bf = ml_dtypes.bfloat16
    e4 = ml_dtypes.float8_e4m3
    e5 = ml_dtypes.float8_e5m2
    x = np.asarray(inputs["x"], dtype=np.float32)

    def f32(k):
        return np.asarray(inputs[k], dtype=np.float32)

    wq8 = np.ascontiguousarray((WQK_SCALE * f32("Wq")).astype(e4))
    wk8 = np.ascontiguousarray((WQK_SCALE * f32("Wk")).astype(e4))
    wv8 = np.ascontiguousarray(f32("Wv").astype(e4))
    wo8 = np.ascontiguousarray(f32("Wo").astype(e4))
    vecs = {k: f32(k) for k in ("bq", "bk", "bv", "bo", "b1", "b2", "g1",
                                "be1", "g2", "be2")}
    # fold LN1's affine into the fc1 weights/bias: x1 @ W1 = u @ (g1*W1)
    # + be1 @ W1 (the raw normalized u is what gets transposed on-chip)
    w1s = W1_SCALE * (vecs["g1"][:, None] * f32("W1"))
    w1h = w1s.astype(e4)
    w1l = (w1s - w1h.astype(np.float32)).astype(e5)
    w1h, w1l = np.ascontiguousarray(w1h), np.ascontiguousarray(w1l)
    b1f = vecs["b1"] + vecs["be1"] @ f32("W1")
    w2s = W2_SCALE * f32("W2")
    w2h = w2s.astype(e4)
    w2l = (w2s - w2h.astype(np.float32)).astype(e5)
    w2h, w2l = np.ascontiguousarray(w2h), np.ascontiguousarray(w2l)
    # tbf on-chip computes u*g1 + be1f where be1f = be1 + b2 (the fc2 bias
    # rides along with the LN2 residual)
    be1f = vecs["be1"] + vecs["b2"]
    mk = _mask_mk().astype(bf)

    in_maps = []
    for c in range(N_CORES):
        b, g = c // 4, c % 4
        idx = _row_index(g)
        xb = x[b]
        xrows = xb[idx]
        in_maps.append({
            "xT": np.ascontiguousarray(xb.T.astype(e4)),
            "xrT": np.ascontiguousarray(xrows.T.astype(e4)),
            "xr": np.ascontiguousarray(xrows + vecs["bo"][None, :]),
            "wq": wq8, "wk": wk8, "wv": wv8, "wo": wo8,
            "w1h": w1h, "w1l": w1l, "w2h": w2h, "w2l": w2l,
            "bq": vecs["bq"], "bk": vecs["bk"],
            "bv": vecs["bv"].astype(bf),
            "b1": b1f,
            "g1": vecs["g1"].astype(bf), "be1": be1f.astype(bf),
            "g2": vecs["g2"].astype(bf), "be2": vecs["be2"].astype(bf),
            "mq": _mask_mq(g).astype(bf),
            "mk": mk,
        })

    res = run_bass_kernel_spmd(nc, in_maps, core_ids=list(range(N_CORES)))
    _CACHE["last_result"] = res

    outp = np.empty((B, L, D), dtype=np.float32)
    for c in range(N_CORES):
        b, g = c // 4, c % 4
        outp[b][_row_index(g)] = res.results[c]["out"].astype(np.float32)
    return outp


# revision 40
# speedup vs baseline: 1.3839x; 1.0037x over previous
"""Trainium2 Bass kernel for AttentionFFNBlock (B=2, L=2048, D=1024, H=16, FF=4096).

Sharding (8 cores, zero cross-core communication):
  core c -> batch b = c//4, group slot g = c%4.
  Each core owns 512 query rows of its batch, interleaved in 128-row blocks
  for causal load balance: global row = (2p+s)*512 + g*128 + i for local row
  r = p*256 + s*128 + i.  The core computes K/V for the full sequence
  (replicated inside the batch group), attention for its rows over all 16
  heads, then out-proj + LN1 + FFN + LN2 for its rows only.

FP8 design (cost model: DoubleRow fp8 matmul = 0.5 cycles/row with 2x128
contraction -> 4x bf16 MAC throughput):
  - Q/K/V/out projections run as fp8e4m3 DoubleRow matmuls. wq/wk are scaled
    16x host-side (their sigma=1/32 sits in e4m3's subnormal range); the
    1/16 descale folds into the psum-drain tensor_scalar for free.
  - Scores stay bf16 (kT/qT bf16).  Causality is enforced PRE-exp by one
    extra bf16 matmul per (pair, kc): a static lower-triangular [k>=m]*-240
    stationary against a per-core indicator moving operand adds -240 exactly
    where key > query.  No per-element mask multiplies on DVE/Pool at all.
  - Softmax: pt = exp(s/8 - 2) written by ACT directly as fp8e4m3 (max logit
    ~6.5 -> max pt ~95 < 240).  The denominator comes from the ones column of
    v8 through the same AV matmul, so quantization of pt largely cancels.
  - AV and out-proj are fp8 DoubleRow (v8 / aoT8 in e4m3).
  - FFN is 3-term split fp8: W ~ (Wh + Wl)/s with Wh=e4m3(s*W) and
    Wl=e5m2(s*W - Wh) (s=16 for W1, 64 for W2 - avoids e4m3 subnormal
    flush), activations split hi=e4m3(a), lo=e5m2(a - hi). Terms
    ah@Wh + al@Wh + ah@Wl accumulate in one psum group: 0.75x the bf16
    cost with ~bf16 accuracy.  Descale 1/16 folds into the Gelu activation
    scale; 1/64 into the fc2 drain tensor_scalar.

Measured numpy end-to-end rel err of this exact scheme: 7.8e-3 (gate 2e-2).
"""

import numpy as np
import ml_dtypes

import concourse.bass as bass
import concourse.mybir as mybir
import concourse.tile as tile
from concourse import bacc
from concourse.bass_utils import run_bass_kernel_spmd
from concourse.masks import make_identity

F32 = mybir.dt.float32
BF16 = mybir.dt.bfloat16
F8E4 = mybir.dt.float8e4
F8E5 = mybir.dt.float8e5
AF = mybir.ActivationFunctionType
ALU = mybir.AluOpType
DR = mybir.MatmulPerfMode.DoubleRow

N_CORES = 8
B, L, D = 2, 2048, 1024
H, HD = 16, 64
DFF = 4096
EPS = 1e-5
P = 128

IC = D // P        # 8 contraction chunks of the model dim
ICH = IC // 2      # 4 DoubleRow chunks (256 contraction each)
TC = L // P        # 16 token chunks
FC = DFF // P      # 32 ff chunks
FCH = FC // 2      # 16 DoubleRow ff chunks
NPAIR = 8          # head pairs (= oc chunks)

WQK_SCALE = 16.0   # wq/wk quantized from 16*W
W1_SCALE = 16.0
W2_SCALE = 64.0

_CACHE = {}


def _build():
    nc = bacc.Bacc("TRN2", target_bir_lowering=False, debug=False,
                   num_devices=N_CORES)

    def din(name, shape, dt=F32):
        return nc.dram_tensor(name, shape, dt, kind="ExternalInput").ap()

    io = dict(
        xT=din("xT", [D, L], F8E4),               # x[b]^T (K/V source)
        xrT=din("xrT", [D, 512], F8E4),           # owned rows^T (Q source)
        xr=din("xr", [512, D], F32),              # owned rows (residual)
        wq=din("wq", [D, D], F8E4), wk=din("wk", [D, D], F8E4),
        wv=din("wv", [D, D], F8E4), wo=din("wo", [D, D], F8E4),
        w1h=din("w1h", [D, DFF], F8E4), w1l=din("w1l", [D, DFF], F8E5),
        w2h=din("w2h", [DFF, D], F8E4), w2l=din("w2l", [DFF, D], F8E5),
        bq=din("bq", [D]), bk=din("bk", [D]), bv=din("bv", [D], BF16),
        b1=din("b1", [DFF]),
        g1=din("g1", [D], BF16), be1=din("be1", [D], BF16),
        g2=din("g2", [D], BF16), be2=din("be2", [D], BF16),
        mq=din("mq", [P, 4, 2, P], BF16),         # causal indicator (per-core)
        mk=din("mk", [P, P], BF16),               # static -240 * [k >= m]
        out=nc.dram_tensor("out", [512, D], BF16, kind="ExternalOutput").ap(),
    )

    with tile.TileContext(nc) as tc:
        _emit(nc, tc, io)
    nc.compile()
    return nc


def _ln_u(nc, pool, acc, eps_t, out_u):
    """Normalize (no affine) over the free axis of acc [128, 1024] -> out_u."""
    stats = pool.tile([P, 2, 6], F32, tag="ln_stats")
    for sg in range(2):
        nc.vector.bn_stats(out=stats[:, sg, :], in_=acc[:, sg * 512:(sg + 1) * 512])
    mv = pool.tile([P, 2], F32, tag="ln_mv")
    nc.vector.bn_aggr(out=mv[:], in_=stats[:])
    rstd = pool.tile([P, 1], F32, tag="ln_rstd")
    nc.scalar.activation(out=rstd[:], in_=mv[:, 1:2], func=AF.Sqrt,
                         bias=eps_t[:], scale=1.0)
    nc.vector.reciprocal(out=rstd[:], in_=rstd[:])
    nmr = pool.tile([P, 1], F32, tag="ln_nmr")
    nc.vector.tensor_scalar(out=nmr[:], in0=mv[:, 0:1], scalar1=rstd[:],
                            scalar2=-1.0, op0=ALU.mult, op1=ALU.mult)
    nc.scalar.activation(out=out_u, in_=acc[:], func=AF.Identity,
                         bias=nmr[:], scale=rstd[:])


def _layernorm(nc, pool, acc, eps_t, g_t, b_t, out_ap, g_eng=None,
               b_eng=None):
    """LayerNorm over the free axis (D=1024) of acc [128, 1024] -> out_ap."""
    u = pool.tile([P, D], BF16, tag="ln_u")
    _ln_u(nc, pool, acc, eps_t, u[:])
    (g_eng or nc.gpsimd).tensor_tensor(out=u[:], in0=u[:], in1=g_t[:, :],
                                       op=ALU.mult)
    (b_eng or nc.vector).tensor_tensor(out=out_ap, in0=u[:], in1=b_t[:, :],
                                       op=ALU.add)


def _emit(nc, tc, io):
    out = io["out"]

    with tc.tile_pool(name="const", bufs=1) as const:
        ao_pool = tc.alloc_tile_pool(name="ao_pool", bufs=1, side="right")
        # ---- constants / biases (tiles now; DMAs deferred past wk/xT) ----
        bq_t = const.tile([P, IC], F32)
        bk_t = const.tile([P, IC], F32)
        b1_t = const.tile([P, FC], F32)
        # bo is folded into xr host-side; b2 into be1 (tbf = x1 + b2);
        # g1/be1 into W1h/W1l/b1 for the fc1 path.
        row_vecs = {}
        for nm in ("bv", "g1", "be1", "g2", "be2"):
            rv = const.tile([P, D], BF16, name=f"cv_{nm}")
            row_vecs[nm] = rv
        bv_t = row_vecs["bv"]
        g1_t, be1_t = row_vecs["g1"], row_vecs["be1"]
        g2_t, be2_t = row_vecs["g2"], row_vecs["be2"]
        mq_t = const.tile([P, 4, 2, P], BF16)
        mk_t = const.tile([P, P], BF16)
        eps_t = const.tile([P, 1], F32)
        neg2_t = const.tile([P, 1], F32)
        ident = const.tile([P, P], BF16)

        def tiny_dmas():
            nc.sync.dma_start(bk_t[:], io["bk"].rearrange("(o p) -> p o", p=P))
            nc.sync.dma_start(bq_t[:], io["bq"].rearrange("(o p) -> p o", p=P))
            nc.sync.dma_start(mq_t[:], io["mq"])
            nc.sync.dma_start(mk_t[:], io["mk"])
            nc.vector.memset(eps_t[:], EPS)
            nc.vector.memset(neg2_t[:], -2.0)

        def early_dmas():
            nc.sync.dma_start(b1_t[:], io["b1"].rearrange("(f p) -> p f", p=P))
            nc.sync.dma_start(row_vecs["bv"][:],
                              io["bv"][None, :].to_broadcast([P, D]))

        def const_dmas():
            for nm in ("g1", "be1", "g2", "be2"):
                nc.sync.dma_start(row_vecs[nm][:],
                                  io[nm][None, :].to_broadcast([P, D]))
            make_identity(nc, ident[:])

        aoT8 = ao_pool.tile([P, IC, 512], F8E4)   # attention output^T (fp8)

        kv_pool = tc.alloc_tile_pool(name="kv_pool", bufs=1)
        ptile = tc.alloc_tile_pool(name="ptile", bufs=7)
        rtile = tc.alloc_tile_pool(name="rtile", bufs=2)
        spsum = tc.alloc_tile_pool(name="spsum", bufs=2, space="PSUM")
        avpsum = tc.alloc_tile_pool(name="avpsum", bufs=1, space="PSUM")
        if True:
            kT = kv_pool.tile([P, IC, L], BF16)
            v8 = kv_pool.tile([P, TC, H, HD + 1], F8E4)
            qT = kv_pool.tile([P, IC, 512], BF16)
            nc.vector.memset(v8[:, :, :, HD:], 1.0)

            proj_stream = []   # deferred (emit_mms, epilogue) generators

            def drain_proj(n):
                """Emit up to n deferred projection matmuls."""
                while n > 0 and proj_stream:
                    gen = proj_stream[0]
                    try:
                        next(gen)
                        n -= 1
                    except StopIteration:
                        proj_stream.pop(0)

            # prefetch pools for FFN weights, allocated mid-attention
            late_pools = {}

            def attention(pair, prev_epi=None, prev_flush=None):
                hA, hB = 2 * pair, 2 * pair + 1
                pavA = avpsum.tile([HD + 1, 512], F32, tag="avA")
                pavB = avpsum.tile([HD + 1, 512], F32, tag="avB")
                pts = []

                def emit_av(ent, last):
                    pkcp, pn0, ppt = ent
                    for j, (h, pav) in enumerate(((hA, pavA), (hB, pavB))):
                        nc.tensor.matmul(
                            pav[:, pn0:512],
                            v8[:, 2 * pkcp:2 * pkcp + 2, h, :],
                            ppt[:, j, :, pn0:512],
                            start=(pkcp == 0), stop=last,
                            perf_mode=DR, skip_group_check=True)

                # exp-feeding matmuls are emitted densely (scores+masks for
                # both kc of the pair back to back) so ACT never waits on
                # drain/AV filler sitting in the in-order PE queue.
                for kcp in range(8):
                    j0 = kcp // 2
                    n0 = j0 * P
                    pt = ptile.tile([P, 2, 2, 512], F8E4, tag="p")
                    for t in range(2):
                        kc = 2 * kcp + t
                        ps = spsum.tile([P, 2, 512], F32, tag="s")
                        nc.tensor.matmul(
                            ps[:, 0, n0:512],
                            kT[0:HD, pair, kc * P:(kc + 1) * P],
                            qT[0:HD, pair, n0:512], start=True, stop=True)
                        nc.tensor.matmul(
                            ps[:, 1, n0:512],
                            kT[HD:P, pair, kc * P:(kc + 1) * P],
                            qT[HD:P, pair, n0:512], start=True, stop=True)
                        for j in range(2):
                            nc.tensor.matmul(
                                ps[:, j, n0:n0 + P], mk_t[:],
                                mq_t[:, kc % 4, j, :], start=False,
                                stop=False, skip_group_check=True)
                        nc.scalar.activation(out=pt[:, :, t, n0:512],
                                             in_=ps[:, :, n0:512],
                                             func=AF.Exp, scale=0.125,
                                             bias=neg2_t[:])
                    pts.append((kcp, n0, pt))
                    if kcp == 0 and prev_flush is not None:
                        prev_flush()
                    if kcp == 1 and prev_epi is not None:
                        prev_epi()
                    if len(pts) >= 4:
                        emit_av(pts.pop(0), last=False)
                    drain_proj(5 if pair < 6 else 2)

                def flush():
                    while pts:
                        emit_av(pts.pop(0), last=(not pts))

                def epi():
                    for hp, pav in ((0, pavA), (HD, pavB)):
                        rec = rtile.tile([1, 512], F32, tag="rec")
                        nc.vector.reciprocal(rec[:], pav[HD:HD + 1, :])
                        rec_b = rtile.tile([HD, 512], F32, tag="rec_b")
                        nc.gpsimd.partition_broadcast(rec_b[:], rec[0:1, :])
                        nc.vector.tensor_tensor(
                            out=aoT8[hp:hp + HD, pair, :],
                            in0=pav[:HD, :], in1=rec_b[:], op=ALU.mult)
                return epi, flush

            # ---- projections (pairs 0..6 overlap with x_pool live) ----
            with (
                tc.tile_pool(name="x_pool", bufs=1) as x_pool,
                tc.tile_pool(name="ppsum", bufs=2, space="PSUM") as ppsum,
            ):
                wk_t = x_pool.tile([P, IC, D], F8E4)
                xT_t = x_pool.tile([P, IC, L], F8E4)
                wq_t = x_pool.tile([P, IC, D], F8E4)
                xrT_t = x_pool.tile([P, IC, 512], F8E4)
                wv_t = x_pool.tile([P, IC, D], F8E4)
                wkr = io["wk"].rearrange("(i p) n -> p i n", p=P)
                wqr = io["wq"].rearrange("(i p) n -> p i n", p=P)
                wvr = io["wv"].rearrange("(i p) n -> p i n", p=P)
                xTr = io["xT"].rearrange("(i p) n -> p i n", p=P)
                nc.sync.dma_start(wk_t[:, :, 0:P], wkr[:, :, 0:P])
                tiny_dmas()
                nc.sync.dma_start(xT_t[:, 0:4, 0:512], xTr[:, 0:4, 0:512])
                nc.sync.dma_start(xT_t[:, 4:8, 0:512], xTr[:, 4:8, 0:512])
                nc.sync.dma_start(wq_t[:, :, 0:P], wqr[:, :, 0:P])
                nc.sync.dma_start(xrT_t[:],
                                  io["xrT"].rearrange("(i p) n -> p i n", p=P))
                nc.sync.dma_start(wq_t[:, :, P:512], wqr[:, :, P:512])
                nc.sync.dma_start(wv_t[:, :, 0:512], wvr[:, :, 0:512])
                early_dmas()
                nc.sync.dma_start(xT_t[:, :, 512:1024], xTr[:, :, 512:1024])
                nc.sync.dma_start(wk_t[:, :, P:512], wkr[:, :, P:512])
                nc.sync.dma_start(xT_t[:, :, 1024:1536], xTr[:, :, 1024:1536])
                nc.sync.dma_start(xT_t[:, :, 1536:2048], xTr[:, :, 1536:2048])
                nc.sync.dma_start(wk_t[:, :, 512:1024], wkr[:, :, 512:1024])
                nc.sync.dma_start(wq_t[:, :, 512:1024], wqr[:, :, 512:1024])
                const_dmas()
                nc.sync.dma_start(wv_t[:, :, 512:1024], wvr[:, :, 512:1024])

                def k_proj(oc):
                    for tcc in range(4):
                        ps = ppsum.tile([P, 512], F32, tag="proj")
                        for i2 in range(ICH):
                            nc.tensor.matmul(
                                ps[:],
                                wk_t[:, 2 * i2:2 * i2 + 2, oc * P:(oc + 1) * P],
                                xT_t[:, 2 * i2:2 * i2 + 2,
                                     tcc * 512:(tcc + 1) * 512],
                                start=(i2 == 0), stop=(i2 == ICH - 1),
                                perf_mode=DR)
                            yield
                        nc.vector.tensor_scalar(
                            out=kT[:, oc, tcc * 512:(tcc + 1) * 512],
                            in0=ps[:], scalar1=1.0 / WQK_SCALE,
                            scalar2=bk_t[:, oc:oc + 1],
                            op0=ALU.mult, op1=ALU.add)

                def q_proj(oc):
                    ps = ppsum.tile([P, 512], F32, tag="proj")
                    for i2 in range(ICH):
                        nc.tensor.matmul(
                            ps[:],
                            wq_t[:, 2 * i2:2 * i2 + 2, oc * P:(oc + 1) * P],
                            xrT_t[:, 2 * i2:2 * i2 + 2, :],
                            start=(i2 == 0), stop=(i2 == ICH - 1),
                            perf_mode=DR)
                        yield
                    nc.vector.tensor_scalar(
                        out=qT[:, oc, :], in0=ps[:], scalar1=1.0 / WQK_SCALE,
                        scalar2=bq_t[:, oc:oc + 1], op0=ALU.mult, op1=ALU.add)

                def v_proj(tcc, hf):
                    ps = ppsum.tile([P, 512], F32, tag="proj")
                    for i2 in range(ICH):
                        nc.tensor.matmul(
                            ps[:],
                            xT_t[:, 2 * i2:2 * i2 + 2, tcc * P:(tcc + 1) * P],
                            wv_t[:, 2 * i2:2 * i2 + 2,
                                 hf * 512:(hf + 1) * 512],
                            start=(i2 == 0), stop=(i2 == ICH - 1),
                            perf_mode=DR)
                        yield
                    nc.vector.tensor_tensor(
                        out=v8[:, tcc, hf * 8:(hf + 1) * 8, :HD],
                        in0=ps.rearrange("p (h d) -> p h d", d=HD),
                        in1=bv_t[:, hf * 512:(hf + 1) * 512]
                        .rearrange("p (h d) -> p h d", d=HD),
                        op=ALU.add)

                def adv(gen, n):
                    for _ in range(n):
                        try:
                            next(gen)
                        except StopIteration:
                            return

                ks = [k_proj(oc) for oc in range(IC)]
                qs = [q_proj(oc) for oc in range(IC)]
                v0s = [v_proj(tcc, 0) for tcc in range(TC)]
                v1s = [v_proj(tcc, 1) for tcc in range(TC)]
                # upfront, ordered to match serial DMA arrival.  All of V0
                # must be EMITTED before pair 0's AV flush (tile deps track
                # emission order), so V0 is not deferred.
                adv(ks[0], 4)                    # K0.tcc0 (wk0+xT0)
                for oc in range(4):
                    adv(qs[oc], 5)               # Q0-3 (wq0+xrT)
                adv(ks[0], 100)                  # K0 rest (xT1-3)
                for tcc in range(TC):
                    adv(v0s[tcc], 5)             # V0 (wv0+xT)
                # deferred: rest drained inside the attention pair loop.
                # Deadlines (6 drains/kcp, 48/pair): k1 by pair 1, v1 fully
                # emitted before pair 4's AV flush, k6/k7 by pairs 6/7.
                proj_stream.append(ks[1])
                proj_stream.extend(qs[4:8])
                proj_stream.append(ks[2])
                proj_stream.extend(v1s[0:4])
                proj_stream.append(ks[3])
                proj_stream.extend(v1s[4:8])
                proj_stream.append(ks[4])
                proj_stream.extend(v1s[8:12])
                proj_stream.append(ks[5])
                proj_stream.extend(v1s[12:16])
                proj_stream.extend([ks[6], ks[7]])

                prev_epi = prev_flush = None
                for pair in range(4):
                    prev_epi, prev_flush = attention(pair, prev_epi,
                                                     prev_flush)
                # mid-attention: prefetch fc1 weights (SBUF freed by Q release
                # is modest; w1h/w1l fit alongside the attention working set)
                w1_pool = tc.alloc_tile_pool(name="w1_pool", bufs=1,
                                             side="right")
                w1h_t = w1_pool.tile([P, IC, DFF], F8E4)
                w1r_h = io["w1h"].rearrange("(i p) n -> p i n", p=P)
                for c in range(4):
                    nc.sync.dma_start(
                        w1h_t[:, :, c * 1024:(c + 1) * 1024],
                        w1r_h[:, :, c * 1024:(c + 1) * 1024])
                late_pools["w1h"] = (w1_pool, w1h_t)
                for pair in range(4, 6):
                    prev_epi, prev_flush = attention(pair, prev_epi,
                                                     prev_flush)
                w1l_pool = tc.alloc_tile_pool(name="w1l_pool", bufs=1,
                                              side="right")
                w1l_t = w1l_pool.tile([P, IC, DFF], F8E5)
                w1r_l = io["w1l"].rearrange("(i p) n -> p i n", p=P)
                for c in range(4):
                    nc.sync.dma_start(
                        w1l_t[:, :, c * 1024:(c + 1) * 1024],
                        w1r_l[:, :, c * 1024:(c + 1) * 1024])
                late_pools["w1l"] = (w1l_pool, w1l_t)
                prev_epi, prev_flush = attention(6, prev_epi, prev_flush)
                drain_proj(1 << 30)

            # x_pool freed: prefetch xr + wo + w2h under attn 7 (right side)
            xrr_pool = tc.alloc_tile_pool(name="xrr_pool", bufs=1, side="right")
            xr_nat = xrr_pool.tile([P, 4, D], F32)
            nc.sync.dma_start(xr_nat[:],
                              io["xr"].rearrange("(rc p) d -> p rc d", p=P))
            wo_pool = tc.alloc_tile_pool(name="wo_pool", bufs=1, side="right")
            wo_t = wo_pool.tile([P, IC, D], F8E4)
            wor = io["wo"].rearrange("(i p) n -> p i n", p=P)
            nc.sync.dma_start(wo_t[:], wor[:])

            prev_epi, prev_flush = attention(7, prev_epi, prev_flush)
            prev_flush()
            prev_epi()

            # free the attention pools (non-LIFO: wo/w1 stay live)
            avpsum.release()
            spsum.release()
            rtile.release()
            ptile.release()
            kv_pool.release()

            w1h_t = late_pools["w1h"][1]
            w1l_t = late_pools["w1l"][1]

            if True:
                # ---- out-proj + LN1 + transpose (hi/lo split) ----
                # The critical path transposes the RAW normalized u (g1/be1
                # are folded into W1h/W1l/b1 host-side); the affine tbf
                # (= x1 + b2, the LN2 residual) is computed off-path.
                with tc.tile_pool(name="t_pool", bufs=1) as t_pool:
                    ubf = t_pool.tile([P, 4, D], BF16)     # LN1 u (pre-affine)
                    tbf = t_pool.tile([P, 4, D], BF16)     # x1 + b2 (residual)
                    tTh = t_pool.tile([P, IC, 512], F8E4)  # u^T hi
                    tTl = t_pool.tile([P, IC, 512], F8E5)  # u^T lo

                    # fc2 weights fit once the attention tiles are gone;
                    # DMA'd in fcp order so fc2 matmuls chase the transfers
                    w2h_pool = tc.alloc_tile_pool(name="w2h_pool", bufs=1)
                    w2h_t = w2h_pool.tile([P, FC, D], F8E4)
                    w2r_h = io["w2h"].rearrange("(f p) n -> p f n", p=P)
                    for grp in range(4):
                        nc.sync.dma_start(
                            w2h_t[:, grp * 8:(grp + 1) * 8, :],
                            w2r_h[:, grp * 8:(grp + 1) * 8, :])
                    w2l_pool = tc.alloc_tile_pool(name="w2l_pool", bufs=1)
                    w2l_t = w2l_pool.tile([P, FC, D], F8E5)
                    w2r_l = io["w2l"].rearrange("(f p) n -> p f n", p=P)
                    for grp in range(4):
                        nc.sync.dma_start(
                            w2l_t[:, grp * 8:(grp + 1) * 8, :],
                            w2r_l[:, grp * 8:(grp + 1) * 8, :])

                    with (
                        tc.tile_pool(name="lnt", bufs=3) as lnt,
                        tc.tile_pool(name="opsum", bufs=4, space="PSUM") as opsum,
                        tc.tile_pool(name="trpsum", bufs=4, space="PSUM") as trpsum,
                    ):
                        for rc in range(4):
                            acc = lnt.tile([P, D], F32, tag="acc")
                            for n2 in range(2):
                                pso = opsum.tile([P, 512], F32, tag="o")
                                for i2 in range(ICH):
                                    nc.tensor.matmul(
                                        pso[:],
                                        aoT8[:, 2 * i2:2 * i2 + 2,
                                             rc * P:(rc + 1) * P],
                                        wo_t[:, 2 * i2:2 * i2 + 2,
                                             n2 * 512:(n2 + 1) * 512],
                                        start=(i2 == 0), stop=(i2 == ICH - 1),
                                        perf_mode=DR)
                                nc.vector.tensor_tensor(
                                    out=acc[:, n2 * 512:(n2 + 1) * 512],
                                    in0=pso[:],
                                    in1=xr_nat[:, rc, n2 * 512:(n2 + 1) * 512],
                                    op=ALU.add)
                            _ln_u(nc, lnt, acc, eps_t, ubf[:, rc, :])
                            # critical path: transpose + hi/lo split of u
                            # (hi casts split ACT/Pool to keep ACT free for
                            # the next rc's u-pass)
                            for ic in range(IC):
                                pst = trpsum.tile([P, P], BF16, tag="tr")
                                nc.tensor.transpose(
                                    pst[:], ubf[:, rc, ic * P:(ic + 1) * P],
                                    ident[:])
                                th = tTh[:, ic, rc * P:(rc + 1) * P]
                                if ic % 2 == 0:
                                    nc.scalar.copy(th, pst[:])
                                else:
                                    nc.vector.tensor_copy(out=th, in_=pst[:])
                                nc.vector.tensor_tensor(
                                    out=tTl[:, ic, rc * P:(rc + 1) * P],
                                    in0=pst[:],
                                    in1=th,
                                    op=ALU.subtract)
                        # off-path: residual tbf = u*g1 + (be1 + b2)
                        for rc in range(4):
                            nc.gpsimd.tensor_tensor(
                                out=tbf[:, rc, :], in0=ubf[:, rc, :],
                                in1=g1_t[:, :], op=ALU.mult)
                            nc.vector.tensor_tensor(
                                out=tbf[:, rc, :], in0=tbf[:, rc, :],
                                in1=be1_t[:, :], op=ALU.add)

                    wo_pool.release()
                    xrr_pool.release()

                    # ================= FFN =================
                    with (
                        tc.tile_pool(name="h_pool", bufs=1) as h_pool,
                        tc.tile_pool(name="tb_pool", bufs=2) as tb_pool,
                        tc.tile_pool(name="fpsum", bufs=2, space="PSUM") as fpsum,
                        tc.tile_pool(name="ypsum", bufs=3, space="PSUM") as ypsum,
                    ):
                        hh = h_pool.tile([P, FC, 512], F8E4)
                        hl = h_pool.tile([P, FC, 512], F8E5)
                        psy = {}
                        stop_tracker = {}

                        def fc2_mms(fcp, rcs, hx, wx, term):
                            for rc in rcs:
                                for n2 in range(2):
                                    key = (rc, n2)
                                    start = key not in stop_tracker
                                    stop_tracker[key] = True
                                    nc.tensor.matmul(
                                        psy[rc][:, n2, :],
                                        hx[:, 2 * fcp:2 * fcp + 2,
                                           rc * P:(rc + 1) * P],
                                        wx[:, 2 * fcp:2 * fcp + 2,
                                           n2 * 512:(n2 + 1) * 512],
                                        start=start, stop=False,
                                        perf_mode=DR, skip_group_check=True)

                        def fc2_mms_last(fcp, rcs):
                            for rc in rcs:
                                for n2 in range(2):
                                    nc.tensor.matmul(
                                        psy[rc][:, n2, :],
                                        hh[:, 2 * fcp:2 * fcp + 2,
                                           rc * P:(rc + 1) * P],
                                        w2l_t[:, 2 * fcp:2 * fcp + 2,
                                              n2 * 512:(n2 + 1) * 512],
                                        start=False, stop=True,
                                        perf_mode=DR, skip_group_check=True)

                        finbox = {}

                        def epilogue(rc):
                            fin = finbox["p"]
                            acc = fin.tile([P, D], F32, tag="acc2", bufs=2)
                            for n2 in range(2):
                                nc.vector.scalar_tensor_tensor(
                                    out=acc[:, n2 * 512:(n2 + 1) * 512],
                                    in0=psy[rc][:, n2, :],
                                    scalar=1.0 / W2_SCALE,
                                    in1=tbf[:, rc, n2 * 512:(n2 + 1) * 512],
                                    op0=ALU.mult, op1=ALU.add)
                            res = fin.tile([P, D], BF16, tag="res", bufs=2)
                            _layernorm(nc, fin, acc, eps_t, g2_t, be2_t,
                                       res[:], g_eng=nc.vector,
                                       b_eng=nc.vector)
                            nc.sync.dma_start(
                                out.rearrange("(rc p) d -> p rc d", p=P)[:, rc, :],
                                res[:])

                        # pass 1: fc1 + fc2 for rc 0,1,2 interleaved per fc;
                        # term3 (hh @ w2l) lags 6 fcp behind so the w2l DMA
                        # (which only starts after the attention pools free)
                        # has landed.
                        psy[0] = ypsum.tile([P, 2, 512], F32, tag="y", name="psy0")
                        psy[1] = ypsum.tile([P, 2, 512], F32, tag="y", name="psy1")
                        psy[2] = ypsum.tile([P, 2, 512], F32, tag="y", name="psy2")
                        LAG = 6
                        for fc in range(FC):
                            ps = fpsum.tile([P, 512], F32, tag="f1")
                            # rc-halves: the first half's operands (tT cols
                            # 0:256 = row chunks 0-1) are ready before the
                            # second, so fc1 can start while LN1/transpose
                            # of rc2-3 is still in flight.
                            for rh in range(2):
                                cols = slice(rh * 256, rh * 256 + 256)
                                h_first = True
                                for wt, xt in ((w1h_t, tTh), (w1h_t, tTl),
                                               (w1l_t, tTh)):
                                    for i2 in range(ICH):
                                        last = (xt is tTh and wt is w1l_t
                                                and i2 == ICH - 1)
                                        nc.tensor.matmul(
                                            ps[:, cols],
                                            wt[:, 2 * i2:2 * i2 + 2,
                                               fc * P:(fc + 1) * P],
                                            xt[:, 2 * i2:2 * i2 + 2, cols],
                                            start=h_first, stop=last,
                                            perf_mode=DR)
                                        h_first = False
                            tb = tb_pool.tile([P, 512], BF16, tag="tb")
                            nc.scalar.activation(out=tb[:], in_=ps[:],
                                                 func=AF.Gelu,
                                                 bias=b1_t[:, fc:fc + 1],
                                                 scale=1.0 / W1_SCALE)
                            nc.gpsimd.tensor_copy(out=hh[:, fc, :], in_=tb[:])
                            nc.vector.tensor_tensor(out=hl[:, fc, :],
                                                    in0=tb[:],
                                                    in1=hh[:, fc, :],
                                                    op=ALU.subtract)
                            if fc % 2 == 1:
                                fcp = fc // 2
                                fc2_mms(fcp, (0, 1, 2), hh, w2h_t, 1)
                                fc2_mms(fcp, (0, 1, 2), hl, w2h_t, 2)
                                if fcp >= LAG:
                                    if fcp - LAG == FCH - 1:
                                        fc2_mms_last(fcp - LAG, (0, 1, 2))
                                    else:
                                        fc2_mms(fcp - LAG, (0, 1, 2), hh,
                                                w2l_t, 3)
                        # fc1 weights are done with: free before the LN2
                        # epilogue scratch allocates (LIFO on the right
                        # stack: w1l, then w1h, then ao)
                        late_pools["w1l"][0].release()
                        late_pools["w1h"][0].release()
                        ao_pool.release()
                        for fcp in range(FCH - LAG, FCH):
                            if fcp == FCH - 1:
                                fc2_mms_last(fcp, (0, 1, 2))
                            else:
                                fc2_mms(fcp, (0, 1, 2), hh, w2l_t, 3)
                        finbox["p"] = tc.alloc_tile_pool(name="fin", bufs=1)
                        epilogue(0)
                        epilogue(1)
                        epilogue(2)
                        # pass 2: fc2 for rc3 (everything resident now)
                        psy[3] = ypsum.tile([P, 2, 512], F32, tag="y", name="psy3")
                        for fcp in range(FCH):
                            fc2_mms(fcp, (3,), hh, w2h_t, 1)
                            fc2_mms(fcp, (3,), hl, w2h_t, 2)
                            if fcp == FCH - 1:
                                fc2_mms_last(fcp, (3,))
                            else:
                                fc2_mms(fcp, (3,), hh, w2l_t, 3)
                        epilogue(3)
                        finbox["p"].release()
                    w2l_pool.release()
                    w2h_pool.release()


def _row_index(g):
    idx = np.empty(512, dtype=np.int64)
    r = 0
    for p in range(2):
        for s in range(2):
            j = 2 * p + s
            base = j * 512 + g * 128
            idx[r:r + 128] = np.arange(base, base + 128)
            r += 128
    return idx


def _mask_mq(g):
    """Causal indicator for the mask matmul: mq[m, i, d, q] = 1 iff the
    static -240*[k >= m] stationary, contracted against this column, yields
    -240*[k > q + (g - i)*128] (the masked region of the n0 block)."""
    mq = np.zeros((P, 4, 2, P), dtype=np.float32)
    for i in range(4):
        t = (g - i) * 128
        for q in range(P):
            tgt = q + t + 1
            if tgt < 0:
                tgt = 0
            if tgt <= P - 1:
                mq[tgt, i, :, q] = 1.0
    return mq


def _mask_mk():
    m = np.arange(P)[:, None]
    k = np.arange(P)[None, :]
    return np.where(k >= m, -240.0, 0.0).astype(np.float32)


def kernel(**inputs):
    if "nc" not in _CACHE:
        _CACHE["nc"] = _build()
    nc = _CACHE["nc"]

    bf = ml_dtypes.bfloat16
    e4 = ml_dtypes.float8_e4m3
    e5 = ml_dtypes.float8_e5m2
    x = np.asarray(inputs["x"], dtype=np.float32)

    def f32(k):
        return np.asarray(inputs[k], dtype=np.float32)

    wq8 = np.ascontiguousarray((WQK_SCALE * f32("Wq")).astype(e4))
    wk8 = np.ascontiguousarray((WQK_SCALE * f32("Wk")).astype(e4))
    wv8 = np.ascontiguousarray(f32("Wv").astype(e4))
    wo8 = np.ascontiguousarray(f32("Wo").astype(e4))
    vecs = {k: f32(k) for k in ("bq", "bk", "bv", "bo", "b1", "b2", "g1",
                                "be1", "g2", "be2")}
    # fold LN1's affine into the fc1 weights/bias: x1 @ W1 = u @ (g1*W1)
    # + be1 @ W1 (the raw normalized u is what gets transposed on-chip)
    w1s = W1_SCALE * (vecs["g1"][:, None] * f32("W1"))
    w1h = w1s.astype(e4)
    w1l = (w1s - w1h.astype(np.float32)).astype(e5)
    w1h, w1l = np.ascontiguousarray(w1h), np.ascontiguousarray(w1l)
    b1f = vecs["b1"] + vecs["be1"] @ f32("W1")
    w2s = W2_SCALE * f32("W2")
    w2h = w2s.astype(e4)
    w2l = (w2s - w2h.astype(np.float32)).astype(e5)
    w2h, w2l = np.ascontiguousarray(w2h), np.ascontiguousarray(w2l)
    # tbf on-chip computes u*g1 + be1f where be1f = be1 + b2 (the fc2 bias
    # rides along with the LN2 residual)
    be1f = vecs["be1"] + vecs["b2"]
    mk = _mask_mk().astype(bf)

    in_maps = []
    for c in range(N_CORES):
        b, g = c // 4, c % 4
        idx = _row_index(g)
        xb = x[b]
        xrows = xb[idx]
        in_maps.append({
            "xT": np.ascontiguousarray(xb.T.astype(e4)),
            "xrT": np.ascontiguousarray(xrows.T.astype(e4)),
            "xr": np.ascontiguousarray(xrows + vecs["bo"][None, :]),
            "wq": wq8, "wk": wk8, "wv": wv8, "wo": wo8,
            "w1h": w1h, "w1l": w1l, "w2h": w2h, "w2l": w2l,
            "bq": vecs["bq"], "bk": vecs["bk"],
            "bv": vecs["bv"].astype(bf),
            "b1": b1f,
            "g1": vecs["g1"].astype(bf), "be1": be1f.astype(bf),
            "g2": vecs["g2"].astype(bf), "be2": vecs["be2"].astype(bf),
            "mq": _mask_mq(g).astype(bf),
            "mk": mk,
        })

    res = run_bass_kernel_spmd(nc, in_maps, core_ids=list(range(N_CORES)))
    _CACHE["last_result"] = res

    outp = np.empty((B, L, D), dtype=np.float32)
    for c in range(N_CORES):
        b, g = c // 4, c % 4
        outp[b][_row_index(g)] = res.results[c]["out"].astype(np.float32)
    return outp


# revision 46
# speedup vs baseline: 1.4084x; 1.0177x over previous
"""Trainium2 Bass kernel for AttentionFFNBlock (B=2, L=2048, D=1024, H=16, FF=4096).

Sharding (8 cores, zero cross-core communication):
  core c -> batch b = c//4, group slot g = c%4.
  Each core owns 512 query rows of its batch, interleaved in 128-row blocks
  for causal load balance: global row = (2p+s)*512 + g*128 + i for local row
  r = p*256 + s*128 + i.  The core computes K/V for the full sequence
  (replicated inside the batch group), attention for its rows over all 16
  heads, then out-proj + LN1 + FFN + LN2 for its rows only.

FP8 design (cost model: DoubleRow fp8 matmul = 0.5 cycles/row with 2x128
contraction -> 4x bf16 MAC throughput):
  - Q/K/V/out projections run as fp8e4m3 DoubleRow matmuls. wq/wk are scaled
    16x host-side (their sigma=1/32 sits in e4m3's subnormal range); the
    1/16 descale folds into the psum-drain tensor_scalar for free.
  - Scores stay bf16 (kT/qT bf16).  Causality is enforced PRE-exp by one
    extra bf16 matmul per (pair, kc): a static lower-triangular [k>=m]*-240
    stationary against a per-core indicator moving operand adds -240 exactly
    where key > query.  No per-element mask multiplies on DVE/Pool at all.
  - Softmax: pt = exp(s/8 - 2) written by ACT directly as fp8e4m3 (max logit
    ~6.5 -> max pt ~95 < 240).  The denominator comes from the ones column of
    v8 through the same AV matmul, so quantization of pt largely cancels.
  - AV and out-proj are fp8 DoubleRow (v8 / aoT8 in e4m3).
  - FFN is 3-term split fp8: W ~ (Wh + Wl)/s with Wh=e4m3(s*W) and
    Wl=e5m2(s*W - Wh) (s=16 for W1, 64 for W2 - avoids e4m3 subnormal
    flush), activations split hi=e4m3(a), lo=e5m2(a - hi). Terms
    ah@Wh + al@Wh + ah@Wl accumulate in one psum group: 0.75x the bf16
    cost with ~bf16 accuracy.  Descale 1/16 folds into the Gelu activation
    scale; 1/64 into the fc2 drain tensor_scalar.

Measured numpy end-to-end rel err of this exact scheme: 7.8e-3 (gate 2e-2).
"""

import numpy as np
import ml_dtypes

import concourse.bass as bass
import concourse.mybir as mybir
import concourse.tile as tile
from concourse import bacc
from concourse.bass_utils import run_bass_kernel_spmd
from concourse.masks import make_identity

F32 = mybir.dt.float32
BF16 = mybir.dt.bfloat16
F8E4 = mybir.dt.float8e4
F8E5 = mybir.dt.float8e5
AF = mybir.ActivationFunctionType
ALU = mybir.AluOpType
DR = mybir.MatmulPerfMode.DoubleRow

N_CORES = 8
B, L, D = 2, 2048, 1024
H, HD = 16, 64
DFF = 4096
EPS = 1e-5
P = 128

IC = D // P        # 8 contraction chunks of the model dim
ICH = IC // 2      # 4 DoubleRow chunks (256 contraction each)
TC = L // P        # 16 token chunks
FC = DFF // P      # 32 ff chunks
FCH = FC // 2      # 16 DoubleRow ff chunks
NPAIR = 8          # head pairs (= oc chunks)

WQK_SCALE = 16.0   # wq/wk quantized from 16*W
W1_SCALE = 16.0
W2_SCALE = 64.0

_CACHE = {}


def _build():
    nc = bacc.Bacc("TRN2", target_bir_lowering=False, debug=False,
                   num_devices=N_CORES)

    def din(name, shape, dt=F32):
        return nc.dram_tensor(name, shape, dt, kind="ExternalInput").ap()

    io = dict(
        xT=din("xT", [D, L], F8E4),               # x[b]^T (K/V source)
        xrT=din("xrT", [D, 512], F8E4),           # owned rows^T (Q source)
        xr=din("xr", [512, D], F32),              # owned rows (residual)
        wq=din("wq", [D, D], F8E4), wk=din("wk", [D, D], F8E4),
        wv=din("wv", [D, D], F8E4), wo=din("wo", [D, D], F8E4),
        w1h=din("w1h", [D, DFF], F8E4), w1l=din("w1l", [D, DFF], F8E5),
        w2h=din("w2h", [DFF, D], F8E4), w2l=din("w2l", [DFF, D], F8E5),
        bq=din("bq", [D]), bk=din("bk", [D]), bv=din("bv", [D], BF16),
        b1=din("b1", [DFF]),
        g1=din("g1", [D], BF16), be1=din("be1", [D], BF16),
        g2=din("g2", [D], BF16), be2=din("be2", [D], BF16),
        mq=din("mq", [P, TC, P], BF16),           # causal indicator (per-core)
        mk=din("mk", [P, P], BF16),               # static -240 * [k >= m]
        out=nc.dram_tensor("out", [512, D], BF16, kind="ExternalOutput").ap(),
    )

    with tile.TileContext(nc) as tc:
        _emit(nc, tc, io)
    nc.compile()
    return nc


def _ln_u(nc, pool, acc, eps_t, out_u):
    """Normalize (no affine) over the free axis of acc [128, 1024] -> out_u."""
    stats = pool.tile([P, 2, 6], F32, tag="ln_stats")
    for sg in range(2):
        nc.vector.bn_stats(out=stats[:, sg, :], in_=acc[:, sg * 512:(sg + 1) * 512])
    mv = pool.tile([P, 2], F32, tag="ln_mv")
    nc.vector.bn_aggr(out=mv[:], in_=stats[:])
    rstd = pool.tile([P, 1], F32, tag="ln_rstd")
    nc.scalar.activation(out=rstd[:], in_=mv[:, 1:2], func=AF.Sqrt,
                         bias=eps_t[:], scale=1.0)
    nc.vector.reciprocal(out=rstd[:], in_=rstd[:])
    nmr = pool.tile([P, 1], F32, tag="ln_nmr")
    nc.vector.tensor_scalar(out=nmr[:], in0=mv[:, 0:1], scalar1=rstd[:],
                            scalar2=-1.0, op0=ALU.mult, op1=ALU.mult)
    nc.scalar.activation(out=out_u, in_=acc[:], func=AF.Identity,
                         bias=nmr[:], scale=rstd[:])


def _layernorm(nc, pool, acc, eps_t, g_t, b_t, out_ap, g_eng=None,
               b_eng=None):
    """LayerNorm over the free axis (D=1024) of acc [128, 1024] -> out_ap."""
    u = pool.tile([P, D], BF16, tag="ln_u")
    _ln_u(nc, pool, acc, eps_t, u[:])
    (g_eng or nc.gpsimd).tensor_tensor(out=u[:], in0=u[:], in1=g_t[:, :],
                                       op=ALU.mult)
    (b_eng or nc.vector).tensor_tensor(out=out_ap, in0=u[:], in1=b_t[:, :],
                                       op=ALU.add)


def _emit(nc, tc, io):
    out = io["out"]

    with tc.tile_pool(name="const", bufs=1) as const:
        ao_pool = tc.alloc_tile_pool(name="ao_pool", bufs=1, side="right")
        # ---- constants / biases (tiles now; DMAs deferred past wk/xT) ----
        bq_t = const.tile([P, IC], F32)
        bk_t = const.tile([P, IC], F32)
        b1_t = const.tile([P, FC], F32)
        # bo is folded into xr host-side; b2 into be1 (tbf = x1 + b2);
        # g1/be1 into W1h/W1l/b1 for the fc1 path.
        row_vecs = {}
        for nm in ("bv", "g1", "be1", "g2", "be2"):
            rv = const.tile([P, D], BF16, name=f"cv_{nm}")
            row_vecs[nm] = rv
        bv_t = row_vecs["bv"]
        g1_t, be1_t = row_vecs["g1"], row_vecs["be1"]
        g2_t, be2_t = row_vecs["g2"], row_vecs["be2"]
        mq_t = const.tile([P, TC, P], BF16)
        mk_t = const.tile([P, P], BF16)
        eps_t = const.tile([P, 1], F32)
        neg2_t = const.tile([P, 1], F32)
        ident = const.tile([P, P], BF16)

        def tiny_dmas():
            nc.sync.dma_start(bk_t[:], io["bk"].rearrange("(o p) -> p o", p=P))
            nc.sync.dma_start(bq_t[:], io["bq"].rearrange("(o p) -> p o", p=P))
            nc.sync.dma_start(mq_t[:], io["mq"])
            nc.sync.dma_start(mk_t[:], io["mk"])
            nc.vector.memset(eps_t[:], EPS)
            nc.vector.memset(neg2_t[:], -2.0)

        def early_dmas():
            nc.sync.dma_start(b1_t[:], io["b1"].rearrange("(f p) -> p f", p=P))
            nc.sync.dma_start(row_vecs["bv"][:],
                              io["bv"][None, :].to_broadcast([P, D]))

        def const_dmas():
            for nm in ("g1", "be1", "g2", "be2"):
                nc.sync.dma_start(row_vecs[nm][:],
                                  io[nm][None, :].to_broadcast([P, D]))
            make_identity(nc, ident[:])

        aoT8 = ao_pool.tile([P, IC, 512], F8E4)   # attention output^T (fp8)

        kv_pool = tc.alloc_tile_pool(name="kv_pool", bufs=1)
        ptile = tc.alloc_tile_pool(name="ptile", bufs=7)
        rtile = tc.alloc_tile_pool(name="rtile", bufs=2)
        spsum = tc.alloc_tile_pool(name="spsum", bufs=2, space="PSUM")
        avpsum = tc.alloc_tile_pool(name="avpsum", bufs=1, space="PSUM")
        if True:
            kT = kv_pool.tile([P, IC, L], BF16)
            v8 = kv_pool.tile([P, TC, H, HD + 1], F8E4)
            qT = kv_pool.tile([P, IC, 512], BF16)
            nc.vector.memset(v8[:, :, :, HD:], 1.0)

            proj_stream = []   # deferred (emit_mms, epilogue) generators

            def drain_proj(n):
                """Emit up to n deferred projection matmuls."""
                while n > 0 and proj_stream:
                    gen = proj_stream[0]
                    try:
                        next(gen)
                        n -= 1
                    except StopIteration:
                        proj_stream.pop(0)

            # prefetch pools for FFN weights, allocated mid-attention
            late_pools = {}

            def attention(pair, prev_epi=None, prev_flush=None):
                hA, hB = 2 * pair, 2 * pair + 1
                pavA = avpsum.tile([HD + 1, 512], F32, tag="avA")
                pavB = avpsum.tile([HD + 1, 512], F32, tag="avB")
                pts = []

                def emit_av(ent, last):
                    pkcp, pn0, ppt = ent
                    for j, (h, pav) in enumerate(((hA, pavA), (hB, pavB))):
                        nc.tensor.matmul(
                            pav[:, pn0:512],
                            v8[:, 2 * pkcp:2 * pkcp + 2, h, :],
                            ppt[:, j, :, pn0:512],
                            start=(pkcp == 0), stop=last,
                            perf_mode=DR, skip_group_check=True)

                # exp-feeding matmuls are emitted densely (scores+masks for
                # both kc of the pair back to back) so ACT never waits on
                # drain/AV filler sitting in the in-order PE queue.
                for kcp in range(8):
                    n0 = kcp * 64
                    pt = ptile.tile([P, 2, 2, 512], F8E4, tag="p")
                    for t in range(2):
                        kc = 2 * kcp + t
                        ps = spsum.tile([P, 2, 512], F32, tag="s")
                        nc.tensor.matmul(
                            ps[:, 0, n0:512],
                            kT[0:HD, pair, kc * P:(kc + 1) * P],
                            qT[0:HD, pair, n0:512], start=True, stop=True)
                        nc.tensor.matmul(
                            ps[:, 1, n0:512],
                            kT[HD:P, pair, kc * P:(kc + 1) * P],
                            qT[HD:P, pair, n0:512], start=True, stop=True)
                        mw = min(P, 512 - n0)
                        for j in range(2):
                            nc.tensor.matmul(
                                ps[:, j, n0:n0 + mw], mk_t[:],
                                mq_t[:, kc, 0:mw], start=False,
                                stop=False, skip_group_check=True)
                        nc.scalar.activation(out=pt[:, :, t, n0:512],
                                             in_=ps[:, :, n0:512],
                                             func=AF.Exp, scale=0.125,
                                             bias=neg2_t[:])
                    pts.append((kcp, n0, pt))
                    if kcp == 0 and prev_flush is not None:
                        prev_flush()
                    if kcp == 1 and prev_epi is not None:
                        prev_epi()
                    if len(pts) >= 4:
                        emit_av(pts.pop(0), last=False)
                    drain_proj(5 if pair < 6 else 2)

                def flush():
                    while pts:
                        emit_av(pts.pop(0), last=(not pts))

                def epi():
                    for hp, pav in ((0, pavA), (HD, pavB)):
                        rec = rtile.tile([1, 512], F32, tag="rec")
                        nc.vector.reciprocal(rec[:], pav[HD:HD + 1, :])
                        rec_b = rtile.tile([HD, 512], F32, tag="rec_b")
                        nc.gpsimd.partition_broadcast(rec_b[:], rec[0:1, :])
                        nc.vector.tensor_tensor(
                            out=aoT8[hp:hp + HD, pair, :],
                            in0=pav[:HD, :], in1=rec_b[:], op=ALU.mult)
                return epi, flush

            # ---- projections (pairs 0..6 overlap with x_pool live) ----
            with (
                tc.tile_pool(name="x_pool", bufs=1) as x_pool,
                tc.tile_pool(name="ppsum", bufs=2, space="PSUM") as ppsum,
            ):
                wk_t = x_pool.tile([P, IC, D], F8E4)
                xT_t = x_pool.tile([P, IC, L], F8E4)
                wq_t = x_pool.tile([P, IC, D], F8E4)
                xrT_t = x_pool.tile([P, IC, 512], F8E4)
                wv_t = x_pool.tile([P, IC, D], F8E4)
                wkr = io["wk"].rearrange("(i p) n -> p i n", p=P)
                wqr = io["wq"].rearrange("(i p) n -> p i n", p=P)
                wvr = io["wv"].rearrange("(i p) n -> p i n", p=P)
                xTr = io["xT"].rearrange("(i p) n -> p i n", p=P)
                nc.sync.dma_start(wk_t[:, :, 0:P], wkr[:, :, 0:P])
                tiny_dmas()
                nc.sync.dma_start(xT_t[:, 0:4, 0:512], xTr[:, 0:4, 0:512])
                nc.sync.dma_start(xT_t[:, 4:8, 0:512], xTr[:, 4:8, 0:512])
                nc.sync.dma_start(wq_t[:, :, 0:P], wqr[:, :, 0:P])
                nc.sync.dma_start(xrT_t[:],
                                  io["xrT"].rearrange("(i p) n -> p i n", p=P))
                nc.sync.dma_start(wq_t[:, :, P:512], wqr[:, :, P:512])
                nc.sync.dma_start(wv_t[:, :, 0:512], wvr[:, :, 0:512])
                early_dmas()
                nc.sync.dma_start(xT_t[:, :, 512:1024], xTr[:, :, 512:1024])
                nc.sync.dma_start(wk_t[:, :, P:512], wkr[:, :, P:512])
                nc.sync.dma_start(xT_t[:, :, 1024:1536], xTr[:, :, 1024:1536])
                nc.sync.dma_start(xT_t[:, :, 1536:2048], xTr[:, :, 1536:2048])
                nc.sync.dma_start(wk_t[:, :, 512:1024], wkr[:, :, 512:1024])
                nc.sync.dma_start(wq_t[:, :, 512:1024], wqr[:, :, 512:1024])
                const_dmas()
                nc.sync.dma_start(wv_t[:, :, 512:1024], wvr[:, :, 512:1024])

                def k_proj(oc):
                    for tcc in range(4):
                        ps = ppsum.tile([P, 512], F32, tag="proj")
                        for i2 in range(ICH):
                            nc.tensor.matmul(
                                ps[:],
                                wk_t[:, 2 * i2:2 * i2 + 2, oc * P:(oc + 1) * P],
                                xT_t[:, 2 * i2:2 * i2 + 2,
                                     tcc * 512:(tcc + 1) * 512],
                                start=(i2 == 0), stop=(i2 == ICH - 1),
                                perf_mode=DR)
                            yield
                        nc.vector.tensor_scalar(
                            out=kT[:, oc, tcc * 512:(tcc + 1) * 512],
                            in0=ps[:], scalar1=1.0 / WQK_SCALE,
                            scalar2=bk_t[:, oc:oc + 1],
                            op0=ALU.mult, op1=ALU.add)

                def q_proj(oc):
                    ps = ppsum.tile([P, 512], F32, tag="proj")
                    for i2 in range(ICH):
                        nc.tensor.matmul(
                            ps[:],
                            wq_t[:, 2 * i2:2 * i2 + 2, oc * P:(oc + 1) * P],
                            xrT_t[:, 2 * i2:2 * i2 + 2, :],
                            start=(i2 == 0), stop=(i2 == ICH - 1),
                            perf_mode=DR)
                        yield
                    nc.vector.tensor_scalar(
                        out=qT[:, oc, :], in0=ps[:], scalar1=1.0 / WQK_SCALE,
                        scalar2=bq_t[:, oc:oc + 1], op0=ALU.mult, op1=ALU.add)

                def v_proj(tcc, hf):
                    ps = ppsum.tile([P, 512], F32, tag="proj")
                    for i2 in range(ICH):
                        nc.tensor.matmul(
                            ps[:],
                            xT_t[:, 2 * i2:2 * i2 + 2, tcc * P:(tcc + 1) * P],
                            wv_t[:, 2 * i2:2 * i2 + 2,
                                 hf * 512:(hf + 1) * 512],
                            start=(i2 == 0), stop=(i2 == ICH - 1),
                            perf_mode=DR)
                        yield
                    nc.vector.tensor_tensor(
                        out=v8[:, tcc, hf * 8:(hf + 1) * 8, :HD],
                        in0=ps.rearrange("p (h d) -> p h d", d=HD),
                        in1=bv_t[:, hf * 512:(hf + 1) * 512]
                        .rearrange("p (h d) -> p h d", d=HD),
                        op=ALU.add)

                def adv(gen, n):
                    for _ in range(n):
                        try:
                            next(gen)
                        except StopIteration:
                            return

                ks = [k_proj(oc) for oc in range(IC)]
                qs = [q_proj(oc) for oc in range(IC)]
                v0s = [v_proj(tcc, 0) for tcc in range(TC)]
                v1s = [v_proj(tcc, 1) for tcc in range(TC)]
                # upfront, ordered to match serial DMA arrival.  All of V0
                # must be EMITTED before pair 0's AV flush (tile deps track
                # emission order), so V0 is not deferred.
                adv(ks[0], 4)                    # K0.tcc0 (wk0+xT0)
                for oc in range(4):
                    adv(qs[oc], 5)               # Q0-3 (wq0+xrT)
                adv(ks[0], 100)                  # K0 rest (xT1-3)
                for tcc in range(TC):
                    adv(v0s[tcc], 5)             # V0 (wv0+xT)
                # deferred: rest drained inside the attention pair loop.
                # Deadlines (6 drains/kcp, 48/pair): k1 by pair 1, v1 fully
                # emitted before pair 4's AV flush, k6/k7 by pairs 6/7.
                proj_stream.append(ks[1])
                proj_stream.extend(qs[4:8])
                proj_stream.append(ks[2])
                proj_stream.extend(v1s[0:4])
                proj_stream.append(ks[3])
                proj_stream.extend(v1s[4:8])
                proj_stream.append(ks[4])
                proj_stream.extend(v1s[8:12])
                proj_stream.append(ks[5])
                proj_stream.extend(v1s[12:16])
                proj_stream.extend([ks[6], ks[7]])

                prev_epi = prev_flush = None
                for pair in range(4):
                    prev_epi, prev_flush = attention(pair, prev_epi,
                                                     prev_flush)
                # mid-attention: prefetch fc1 weights (SBUF freed by Q release
                # is modest; w1h/w1l fit alongside the attention working set)
                w1_pool = tc.alloc_tile_pool(name="w1_pool", bufs=1,
                                             side="right")
                w1h_t = w1_pool.tile([P, IC, DFF], F8E4)
                w1r_h = io["w1h"].rearrange("(i p) n -> p i n", p=P)
                for c in range(4):
                    nc.sync.dma_start(
                        w1h_t[:, :, c * 1024:(c + 1) * 1024],
                        w1r_h[:, :, c * 1024:(c + 1) * 1024])
                late_pools["w1h"] = (w1_pool, w1h_t)
                for pair in range(4, 6):
                    prev_epi, prev_flush = attention(pair, prev_epi,
                                                     prev_flush)
                w1l_pool = tc.alloc_tile_pool(name="w1l_pool", bufs=1,
                                              side="right")
                w1l_t = w1l_pool.tile([P, IC, DFF], F8E5)
                w1r_l = io["w1l"].rearrange("(i p) n -> p i n", p=P)
                for c in range(4):
                    nc.sync.dma_start(
                        w1l_t[:, :, c * 1024:(c + 1) * 1024],
                        w1r_l[:, :, c * 1024:(c + 1) * 1024])
                late_pools["w1l"] = (w1l_pool, w1l_t)
                prev_epi, prev_flush = attention(6, prev_epi, prev_flush)
                drain_proj(1 << 30)

            # x_pool freed: prefetch xr + wo + w2h under attn 7 (right side)
            xrr_pool = tc.alloc_tile_pool(name="xrr_pool", bufs=1, side="right")
            xr_nat = xrr_pool.tile([P, 4, D], F32)
            nc.sync.dma_start(xr_nat[:],
                              io["xr"].rearrange("(rc p) d -> p rc d", p=P))
            wo_pool = tc.alloc_tile_pool(name="wo_pool", bufs=1, side="right")
            wo_t = wo_pool.tile([P, IC, D], F8E4)
            wor = io["wo"].rearrange("(i p) n -> p i n", p=P)
            nc.sync.dma_start(wo_t[:], wor[:])

            prev_epi, prev_flush = attention(7, prev_epi, prev_flush)
            prev_flush()
            prev_epi()

            # free the attention pools (non-LIFO: wo/w1 stay live)
            avpsum.release()
            spsum.release()
            rtile.release()
            ptile.release()
            kv_pool.release()

            w1h_t = late_pools["w1h"][1]
            w1l_t = late_pools["w1l"][1]

            if True:
                # ---- out-proj + LN1 + transpose (hi/lo split) ----
                # The critical path transposes the RAW normalized u (g1/be1
                # are folded into W1h/W1l/b1 host-side); the affine tbf
                # (= x1 + b2, the LN2 residual) is computed off-path.
                with tc.tile_pool(name="t_pool", bufs=1) as t_pool:
                    ubf = t_pool.tile([P, 4, D], BF16)     # LN1 u (pre-affine)
                    tbf = t_pool.tile([P, 4, D], BF16)     # x1 + b2 (residual)
                    tTh = t_pool.tile([P, IC, 512], F8E4)  # u^T hi
                    tTl = t_pool.tile([P, IC, 512], F8E5)  # u^T lo

                    # fc2 weights fit once the attention tiles are gone;
                    # DMA'd in fcp order so fc2 matmuls chase the transfers
                    w2h_pool = tc.alloc_tile_pool(name="w2h_pool", bufs=1)
                    w2h_t = w2h_pool.tile([P, FC, D], F8E4)
                    w2r_h = io["w2h"].rearrange("(f p) n -> p f n", p=P)
                    for grp in range(4):
                        nc.sync.dma_start(
                            w2h_t[:, grp * 8:(grp + 1) * 8, :],
                            w2r_h[:, grp * 8:(grp + 1) * 8, :])
                    w2l_pool = tc.alloc_tile_pool(name="w2l_pool", bufs=1)
                    w2l_t = w2l_pool.tile([P, FC, D], F8E5)
                    w2r_l = io["w2l"].rearrange("(f p) n -> p f n", p=P)
                    for grp in range(4):
                        nc.sync.dma_start(
                            w2l_t[:, grp * 8:(grp + 1) * 8, :],
                            w2r_l[:, grp * 8:(grp + 1) * 8, :])

                    with (
                        tc.tile_pool(name="lnt", bufs=3) as lnt,
                        tc.tile_pool(name="opsum", bufs=4, space="PSUM") as opsum,
                        tc.tile_pool(name="trpsum", bufs=4, space="PSUM") as trpsum,
                    ):
                        for rc in range(4):
                            acc = lnt.tile([P, D], F32, tag="acc")
                            for n2 in range(2):
                                pso = opsum.tile([P, 512], F32, tag="o")
                                for i2 in range(ICH):
                                    nc.tensor.matmul(
                                        pso[:],
                                        aoT8[:, 2 * i2:2 * i2 + 2,
                                             rc * P:(rc + 1) * P],
                                        wo_t[:, 2 * i2:2 * i2 + 2,
                                             n2 * 512:(n2 + 1) * 512],
                                        start=(i2 == 0), stop=(i2 == ICH - 1),
                                        perf_mode=DR)
                                nc.vector.tensor_tensor(
                                    out=acc[:, n2 * 512:(n2 + 1) * 512],
                                    in0=pso[:],
                                    in1=xr_nat[:, rc, n2 * 512:(n2 + 1) * 512],
                                    op=ALU.add)
                            _ln_u(nc, lnt, acc, eps_t, ubf[:, rc, :])
                            # critical path: transpose + hi/lo split of u
                            # (hi casts split ACT/Pool to keep ACT free for
                            # the next rc's u-pass)
                            for ic in range(IC):
                                pst = trpsum.tile([P, P], BF16, tag="tr")
                                nc.tensor.transpose(
                                    pst[:], ubf[:, rc, ic * P:(ic + 1) * P],
                                    ident[:])
                                th = tTh[:, ic, rc * P:(rc + 1) * P]
                                if ic % 2 == 0:
                                    nc.scalar.copy(th, pst[:])
                                else:
                                    nc.vector.tensor_copy(out=th, in_=pst[:])
                                nc.vector.tensor_tensor(
                                    out=tTl[:, ic, rc * P:(rc + 1) * P],
                                    in0=pst[:],
                                    in1=th,
                                    op=ALU.subtract)
                        # off-path: residual tbf = u*g1 + (be1 + b2)
                        for rc in range(4):
                            nc.gpsimd.tensor_tensor(
                                out=tbf[:, rc, :], in0=ubf[:, rc, :],
                                in1=g1_t[:, :], op=ALU.mult)
                            nc.vector.tensor_tensor(
                                out=tbf[:, rc, :], in0=tbf[:, rc, :],
                                in1=be1_t[:, :], op=ALU.add)

                    wo_pool.release()
                    xrr_pool.release()

                    # ================= FFN =================
                    with (
                        tc.tile_pool(name="h_pool", bufs=1) as h_pool,
                        tc.tile_pool(name="tb_pool", bufs=2) as tb_pool,
                        tc.tile_pool(name="fpsum", bufs=2, space="PSUM") as fpsum,
                        tc.tile_pool(name="ypsum", bufs=3, space="PSUM") as ypsum,
                    ):
                        hh = h_pool.tile([P, FC, 512], F8E4)
                        hl = h_pool.tile([P, FC, 512], F8E5)
                        psy = {}
                        stop_tracker = {}

                        def fc2_mms(fcp, rcs, hx, wx, term):
                            for rc in rcs:
                                for n2 in range(2):
                                    key = (rc, n2)
                                    start = key not in stop_tracker
                                    stop_tracker[key] = True
                                    nc.tensor.matmul(
                                        psy[rc][:, n2, :],
                                        hx[:, 2 * fcp:2 * fcp + 2,
                                           rc * P:(rc + 1) * P],
                                        wx[:, 2 * fcp:2 * fcp + 2,
                                           n2 * 512:(n2 + 1) * 512],
                                        start=start, stop=False,
                                        perf_mode=DR, skip_group_check=True)

                        def fc2_mms_last(fcp, rcs):
                            for rc in rcs:
                                for n2 in range(2):
                                    nc.tensor.matmul(
                                        psy[rc][:, n2, :],
                                        hh[:, 2 * fcp:2 * fcp + 2,
                                           rc * P:(rc + 1) * P],
                                        w2l_t[:, 2 * fcp:2 * fcp + 2,
                                              n2 * 512:(n2 + 1) * 512],
                                        start=False, stop=True,
                                        perf_mode=DR, skip_group_check=True)

                        finbox = {}

                        def epilogue(rc):
                            fin = finbox["p"]
                            acc = fin.tile([P, D], F32, tag="acc2", bufs=2)
                            for n2 in range(2):
                                nc.vector.scalar_tensor_tensor(
                                    out=acc[:, n2 * 512:(n2 + 1) * 512],
                                    in0=psy[rc][:, n2, :],
                                    scalar=1.0 / W2_SCALE,
                                    in1=tbf[:, rc, n2 * 512:(n2 + 1) * 512],
                                    op0=ALU.mult, op1=ALU.add)
                            res = fin.tile([P, D], BF16, tag="res", bufs=2)
                            _layernorm(nc, fin, acc, eps_t, g2_t, be2_t,
                                       res[:], g_eng=nc.vector,
                                       b_eng=nc.vector)
                            nc.sync.dma_start(
                                out.rearrange("(rc p) d -> p rc d", p=P)[:, rc, :],
                                res[:])

                        # pass 1: fc1 + fc2 for rc 0,1,2 interleaved per fc;
                        # term3 (hh @ w2l) lags 6 fcp behind so the w2l DMA
                        # (which only starts after the attention pools free)
                        # has landed.
                        psy[0] = ypsum.tile([P, 2, 512], F32, tag="y", name="psy0")
                        psy[1] = ypsum.tile([P, 2, 512], F32, tag="y", name="psy1")
                        psy[2] = ypsum.tile([P, 2, 512], F32, tag="y", name="psy2")
                        LAG = 6
                        for fc in range(FC):
                            ps = fpsum.tile([P, 512], F32, tag="f1")
                            # rc-halves: the first half's operands (tT cols
                            # 0:256 = row chunks 0-1) are ready before the
                            # second, so fc1 can start while LN1/transpose
                            # of rc2-3 is still in flight.
                            for rh in range(2):
                                cols = slice(rh * 256, rh * 256 + 256)
                                h_first = True
                                for wt, xt in ((w1h_t, tTh), (w1h_t, tTl),
                                               (w1l_t, tTh)):
                                    for i2 in range(ICH):
                                        last = (xt is tTh and wt is w1l_t
                                                and i2 == ICH - 1)
                                        nc.tensor.matmul(
                                            ps[:, cols],
                                            wt[:, 2 * i2:2 * i2 + 2,
                                               fc * P:(fc + 1) * P],
                                            xt[:, 2 * i2:2 * i2 + 2, cols],
                                            start=h_first, stop=last,
                                            perf_mode=DR)
                                        h_first = False
                            tb = tb_pool.tile([P, 512], BF16, tag="tb")
                            nc.scalar.activation(out=tb[:], in_=ps[:],
                                                 func=AF.Gelu,
                                                 bias=b1_t[:, fc:fc + 1],
                                                 scale=1.0 / W1_SCALE)
                            nc.gpsimd.tensor_copy(out=hh[:, fc, :], in_=tb[:])
                            nc.vector.tensor_tensor(out=hl[:, fc, :],
                                                    in0=tb[:],
                                                    in1=hh[:, fc, :],
                                                    op=ALU.subtract)
                            if fc % 2 == 1:
                                fcp = fc // 2
                                fc2_mms(fcp, (0, 1, 2), hh, w2h_t, 1)
                                fc2_mms(fcp, (0, 1, 2), hl, w2h_t, 2)
                                if fcp >= LAG:
                                    if fcp - LAG == FCH - 1:
                                        fc2_mms_last(fcp - LAG, (0, 1, 2))
                                    else:
                                        fc2_mms(fcp - LAG, (0, 1, 2), hh,
                                                w2l_t, 3)
                        # fc1 weights are done with: free before the LN2
                        # epilogue scratch allocates (LIFO on the right
                        # stack: w1l, then w1h, then ao)
                        late_pools["w1l"][0].release()
                        late_pools["w1h"][0].release()
                        ao_pool.release()
                        for fcp in range(FCH - LAG, FCH):
                            if fcp == FCH - 1:
                                fc2_mms_last(fcp, (0, 1, 2))
                            else:
                                fc2_mms(fcp, (0, 1, 2), hh, w2l_t, 3)
                        finbox["p"] = tc.alloc_tile_pool(name="fin", bufs=1)
                        epilogue(0)
                        epilogue(1)
                        epilogue(2)
                        # pass 2: fc2 for rc3 (everything resident now)
                        psy[3] = ypsum.tile([P, 2, 512], F32, tag="y", name="psy3")
                        for fcp in range(FCH):
                            fc2_mms(fcp, (3,), hh, w2h_t, 1)
                            fc2_mms(fcp, (3,), hl, w2h_t, 2)
                            if fcp == FCH - 1:
                                fc2_mms_last(fcp, (3,))
                            else:
                                fc2_mms(fcp, (3,), hh, w2l_t, 3)
                        epilogue(3)
                        finbox["p"].release()
                    w2l_pool.release()
                    w2h_pool.release()


def _blocks(g):
    """64-row blocks owned by core g: {8m+g, 8m+7-g} - exactly balanced
    causal load across the 4 cores of a batch group."""
    return sorted(b for m in range(4) for b in (8 * m + g, 8 * m + 7 - g))


def _row_index(g):
    idx = np.empty(512, dtype=np.int64)
    for v, w in enumerate(_blocks(g)):
        idx[v * 64:(v + 1) * 64] = np.arange(w * 64, w * 64 + 64)
    return idx


def _mask_mq(g):
    """Causal indicator for the mask matmul: column q of block kc selects
    the row m of the static -240*[k >= m] stationary such that the product
    adds -240 exactly where global key > global query."""
    blocks = _blocks(g)
    mq = np.zeros((P, TC, P), dtype=np.float32)
    for kc in range(TC):
        n0 = 64 * (kc // 2)
        for qq in range(P):
            q = n0 + qq
            if q >= 512:
                break
            qg = 64 * blocks[q // 64] + q % 64
            thr = qg - P * kc          # mask iff key k > thr
            tgt = max(thr + 1, 0)
            if tgt <= P - 1:
                mq[tgt, kc, qq] = 1.0
    return mq


def _mask_mk():
    m = np.arange(P)[:, None]
    k = np.arange(P)[None, :]
    return np.where(k >= m, -240.0, 0.0).astype(np.float32)


def kernel(**inputs):
    if "nc" not in _CACHE:
        _CACHE["nc"] = _build()
    nc = _CACHE["nc"]

    bf = ml_dtypes.bfloat16
    e4 = ml_dtypes.float8_e4m3
    e5 = ml_dtypes.float8_e5m2
    x = np.asarray(inputs["x"], dtype=np.float32)

    def f32(k):
        return np.asarray(inputs[k], dtype=np.float32)

    wq8 = np.ascontiguousarray((WQK_SCALE * f32("Wq")).astype(e4))
    wk8 = np.ascontiguousarray((WQK_SCALE * f32("Wk")).astype(e4))
    wv8 = np.ascontiguousarray(f32("Wv").astype(e4))
    wo8 = np.ascontiguousarray(f32("Wo").astype(e4))
    vecs = {k: f32(k) for k in ("bq", "bk", "bv", "bo", "b1", "b2", "g1",
                                "be1", "g2", "be2")}
    # fold LN1's affine into the fc1 weights/bias: x1 @ W1 = u @ (g1*W1)
    # + be1 @ W1 (the raw normalized u is what gets transposed on-chip)
    w1s = W1_SCALE * (vecs["g1"][:, None] * f32("W1"))
    w1h = w1s.astype(e4)
    w1l = (w1s - w1h.astype(np.float32)).astype(e5)
    w1h, w1l = np.ascontiguousarray(w1h), np.ascontiguousarray(w1l)
    b1f = vecs["b1"] + vecs["be1"] @ f32("W1")
    w2s = W2_SCALE * f32("W2")
    w2h = w2s.astype(e4)
    w2l = (w2s - w2h.astype(np.float32)).astype(e5)
    w2h, w2l = np.ascontiguousarray(w2h), np.ascontiguousarray(w2l)
    # tbf on-chip computes u*g1 + be1f where be1f = be1 + b2 (the fc2 bias
    # rides along with the LN2 residual)
    be1f = vecs["be1"] + vecs["b2"]
    mk = _mask_mk().astype(bf)

    in_maps = []
    for c in range(N_CORES):
        b, g = c // 4, c % 4
        idx = _row_index(g)
        xb = x[b]
        xrows = xb[idx]
        in_maps.append({
            "xT": np.ascontiguousarray(xb.T.astype(e4)),
            "xrT": np.ascontiguousarray(xrows.T.astype(e4)),
            "xr": np.ascontiguousarray(xrows + vecs["bo"][None, :]),
            "wq": wq8, "wk": wk8, "wv": wv8, "wo": wo8,
            "w1h": w1h, "w1l": w1l, "w2h": w2h, "w2l": w2l,
            "bq": vecs["bq"], "bk": vecs["bk"],
            "bv": vecs["bv"].astype(bf),
            "b1": b1f,
            "g1": vecs["g1"].astype(bf), "be1": be1f.astype(bf),
            "g2": vecs["g2"].astype(bf), "be2": vecs["be2"].astype(bf),
            "mq": _mask_mq(g).astype(bf),
            "mk": mk,
        })

    res = run_bass_kernel_spmd(nc, in_maps, core_ids=list(range(N_CORES)))
    _CACHE["last_result"] = res

    outp = np.empty((B, L, D), dtype=np.float32)
    for c in range(N_CORES):
        b, g = c // 4, c % 4
        outp[b][_row_index(g)] = res.results[c]["out"].astype(np.float32)
    return outp


# revision 54
# speedup vs baseline: 1.4311x; 1.0161x over previous
"""Trainium2 Bass kernel for AttentionFFNBlock (B=2, L=2048, D=1024, H=16, FF=4096).

Sharding (8 cores, zero cross-core communication):
  core c -> batch b = c//4, group slot g = c%4.
  Each core owns 512 query rows of its batch, interleaved in 128-row blocks
  for causal load balance: global row = (2p+s)*512 + g*128 + i for local row
  r = p*256 + s*128 + i.  The core computes K/V for the full sequence
  (replicated inside the batch group), attention for its rows over all 16
  heads, then out-proj + LN1 + FFN + LN2 for its rows only.

FP8 design (cost model: DoubleRow fp8 matmul = 0.5 cycles/row with 2x128
contraction -> 4x bf16 MAC throughput):
  - Q/K/V/out projections run as fp8e4m3 DoubleRow matmuls. wq/wk are scaled
    16x host-side (their sigma=1/32 sits in e4m3's subnormal range); the
    1/16 descale folds into the psum-drain tensor_scalar for free.
  - Scores stay bf16 (kT/qT bf16).  Causality is enforced PRE-exp by one
    extra bf16 matmul per (pair, kc): a static lower-triangular [k>=m]*-240
    stationary against a per-core indicator moving operand adds -240 exactly
    where key > query.  No per-element mask multiplies on DVE/Pool at all.
  - Softmax: pt = exp(s/8 - 2) written by ACT directly as fp8e4m3 (max logit
    ~6.5 -> max pt ~95 < 240).  The denominator comes from the ones column of
    v8 through the same AV matmul, so quantization of pt largely cancels.
  - AV and out-proj are fp8 DoubleRow (v8 / aoT8 in e4m3).
  - FFN is 3-term split fp8: W ~ (Wh + Wl)/s with Wh=e4m3(s*W) and
    Wl=e5m2(s*W - Wh) (s=16 for W1, 64 for W2 - avoids e4m3 subnormal
    flush), activations split hi=e4m3(a), lo=e5m2(a - hi). Terms
    ah@Wh + al@Wh + ah@Wl accumulate in one psum group: 0.75x the bf16
    cost with ~bf16 accuracy.  Descale 1/16 folds into the Gelu activation
    scale; 1/64 into the fc2 drain tensor_scalar.

Measured numpy end-to-end rel err of this exact scheme: 7.8e-3 (gate 2e-2).
"""

import numpy as np
import ml_dtypes

import concourse.bass as bass
import concourse.mybir as mybir
import concourse.tile as tile
from concourse import bacc
from concourse.bass_utils import run_bass_kernel_spmd
from concourse.masks import make_identity

F32 = mybir.dt.float32
BF16 = mybir.dt.bfloat16
F8E4 = mybir.dt.float8e4
F8E5 = mybir.dt.float8e5
AF = mybir.ActivationFunctionType
ALU = mybir.AluOpType
DR = mybir.MatmulPerfMode.DoubleRow

N_CORES = 8
B, L, D = 2, 2048, 1024
H, HD = 16, 64
DFF = 4096
EPS = 1e-5
P = 128

IC = D // P        # 8 contraction chunks of the model dim
ICH = IC // 2      # 4 DoubleRow chunks (256 contraction each)
TC = L // P        # 16 token chunks
FC = DFF // P      # 32 ff chunks
FCH = FC // 2      # 16 DoubleRow ff chunks
NPAIR = 8          # head pairs (= oc chunks)

WQK_SCALE = 16.0   # wq/wk quantized from 16*W
W1_SCALE = 16.0
W2_SCALE = 64.0

_CACHE = {}


def _build():
    nc = bacc.Bacc("TRN2", target_bir_lowering=False, debug=False,
                   num_devices=N_CORES)

    def din(name, shape, dt=F32):
        return nc.dram_tensor(name, shape, dt, kind="ExternalInput").ap()

    io = dict(
        xT=din("xT", [D, L], F8E4),               # x[b]^T (K/V source)
        xrT=din("xrT", [D, 512], F8E4),           # owned rows^T (Q source)
        xr=din("xr", [512, D], F32),              # owned rows (residual)
        wq=din("wq", [D, D], F8E4), wk=din("wk", [D, D], F8E4),
        wv=din("wv", [D, D], F8E4), wo=din("wo", [D, D], F8E4),
        w1h=din("w1h", [D, DFF], F8E4), w1l=din("w1l", [D, DFF], F8E5),
        w2h=din("w2h", [DFF, D], F8E4), w2l=din("w2l", [DFF, D], F8E5),
        bq=din("bq", [D]), bk=din("bk", [D]), bv=din("bv", [D], BF16),
        b1=din("b1", [DFF]),
        g1=din("g1", [D], BF16), be1=din("be1", [D], BF16),
        g2=din("g2", [D], BF16), be2=din("be2", [D], BF16),
        mq=din("mq", [P, TC, P], BF16),           # causal indicator (per-core)
        mk=din("mk", [P, P], BF16),               # static -240 * [k >= m]
        out=nc.dram_tensor("out", [512, D], BF16, kind="ExternalOutput").ap(),
    )

    with tile.TileContext(nc) as tc:
        _emit(nc, tc, io)
    nc.compile()
    return nc


def _ln_u(nc, pool, acc, eps_t, out_u):
    """Normalize (no affine) over the free axis of acc [128, 1024] -> out_u."""
    stats = pool.tile([P, 2, 6], F32, tag="ln_stats")
    for sg in range(2):
        nc.vector.bn_stats(out=stats[:, sg, :], in_=acc[:, sg * 512:(sg + 1) * 512])
    mv = pool.tile([P, 2], F32, tag="ln_mv")
    nc.vector.bn_aggr(out=mv[:], in_=stats[:])
    rstd = pool.tile([P, 1], F32, tag="ln_rstd")
    nc.scalar.activation(out=rstd[:], in_=mv[:, 1:2], func=AF.Sqrt,
                         bias=eps_t[:], scale=1.0)
    nc.vector.reciprocal(out=rstd[:], in_=rstd[:])
    nmr = pool.tile([P, 1], F32, tag="ln_nmr")
    nc.vector.tensor_scalar(out=nmr[:], in0=mv[:, 0:1], scalar1=rstd[:],
                            scalar2=-1.0, op0=ALU.mult, op1=ALU.mult)
    nc.scalar.activation(out=out_u, in_=acc[:], func=AF.Identity,
                         bias=nmr[:], scale=rstd[:])


def _layernorm(nc, pool, acc, eps_t, g_t, b_t, out_ap, g_eng=None,
               b_eng=None):
    """LayerNorm over the free axis (D=1024) of acc [128, 1024] -> out_ap."""
    u = pool.tile([P, D], BF16, tag="ln_u")
    _ln_u(nc, pool, acc, eps_t, u[:])
    (g_eng or nc.gpsimd).tensor_tensor(out=u[:], in0=u[:], in1=g_t[:, :],
                                       op=ALU.mult)
    (b_eng or nc.vector).tensor_tensor(out=out_ap, in0=u[:], in1=b_t[:, :],
                                       op=ALU.add)


def _emit(nc, tc, io):
    out = io["out"]

    with tc.tile_pool(name="const", bufs=1) as const:
        ao_pool = tc.alloc_tile_pool(name="ao_pool", bufs=1, side="right")
        # ---- constants / biases (tiles now; DMAs deferred past wk/xT) ----
        bq_t = const.tile([P, IC], F32)
        bk_t = const.tile([P, IC], F32)
        b1_t = const.tile([P, FC], F32)
        # bo is folded into xr host-side; b2 into be1 (tbf = x1 + b2);
        # g1/be1 into W1h/W1l/b1 for the fc1 path.
        row_vecs = {}
        for nm in ("bv", "g1", "be1", "g2", "be2"):
            rv = const.tile([P, D], BF16, name=f"cv_{nm}")
            row_vecs[nm] = rv
        bv_t = row_vecs["bv"]
        g1_t, be1_t = row_vecs["g1"], row_vecs["be1"]
        g2_t, be2_t = row_vecs["g2"], row_vecs["be2"]
        mq_t = const.tile([P, TC, P], BF16)
        mk_t = const.tile([P, P], BF16)
        eps_t = const.tile([P, 1], F32)
        neg2_t = const.tile([P, 1], F32)
        ident = const.tile([P, P], BF16)

        def tiny_dmas():
            nc.sync.dma_start(bk_t[:], io["bk"].rearrange("(o p) -> p o", p=P))
            nc.sync.dma_start(bq_t[:], io["bq"].rearrange("(o p) -> p o", p=P))
            nc.sync.dma_start(mq_t[:], io["mq"])
            nc.sync.dma_start(mk_t[:], io["mk"])
            nc.vector.memset(eps_t[:], EPS)
            nc.vector.memset(neg2_t[:], -2.0)

        def early_dmas():
            nc.sync.dma_start(b1_t[:], io["b1"].rearrange("(f p) -> p f", p=P))
            nc.sync.dma_start(row_vecs["bv"][:],
                              io["bv"][None, :].to_broadcast([P, D]))

        def const_dmas():
            for nm in ("g1", "be1", "g2", "be2"):
                nc.sync.dma_start(row_vecs[nm][:],
                                  io[nm][None, :].to_broadcast([P, D]))
            make_identity(nc, ident[:])

        aoT8 = ao_pool.tile([P, IC, 512], F8E4)   # attention output^T (fp8)

        kv_pool = tc.alloc_tile_pool(name="kv_pool", bufs=1)
        ptile = tc.alloc_tile_pool(name="ptile", bufs=7)
        rtile = tc.alloc_tile_pool(name="rtile", bufs=2)
        spsum = tc.alloc_tile_pool(name="spsum", bufs=2, space="PSUM")
        avpsum = tc.alloc_tile_pool(name="avpsum", bufs=1, space="PSUM")
        if True:
            kT = kv_pool.tile([P, IC, L], BF16)
            v8 = kv_pool.tile([P, TC, H, HD + 1], F8E4)
            qT = kv_pool.tile([P, IC, 512], BF16)
            nc.vector.memset(v8[:, :, :, HD:], 1.0)

            proj_stream = []   # deferred (emit_mms, epilogue) generators

            def drain_proj(n):
                """Emit up to n deferred projection matmuls."""
                while n > 0 and proj_stream:
                    gen = proj_stream[0]
                    try:
                        next(gen)
                        n -= 1
                    except StopIteration:
                        proj_stream.pop(0)

            # prefetch pools for FFN weights, allocated mid-attention
            late_pools = {}

            def attention(pair, prev_epi=None, prev_flush=None):
                hA, hB = 2 * pair, 2 * pair + 1
                pavA = avpsum.tile([HD + 1, 512], F32, tag="avA")
                pavB = avpsum.tile([HD + 1, 512], F32, tag="avB")
                pts = []

                def emit_av(ent, last):
                    pkcp, pn0, ppt = ent
                    for j, (h, pav) in enumerate(((hA, pavA), (hB, pavB))):
                        nc.tensor.matmul(
                            pav[:, pn0:512],
                            v8[:, 2 * pkcp:2 * pkcp + 2, h, :],
                            ppt[:, j, :, pn0:512],
                            start=(pkcp == 0), stop=last,
                            perf_mode=DR, skip_group_check=True)

                # exp-feeding matmuls are emitted densely (scores+masks for
                # both kc of the pair back to back) so ACT never waits on
                # drain/AV filler sitting in the in-order PE queue.
                for kcp in range(8):
                    n0 = kcp * 64
                    pt = ptile.tile([P, 2, 2, 512], F8E4, tag="p")
                    for t in range(2):
                        kc = 2 * kcp + t
                        ps = spsum.tile([P, 2, 512], F32, tag="s")
                        nc.tensor.matmul(
                            ps[:, 0, n0:512],
                            kT[0:HD, pair, kc * P:(kc + 1) * P],
                            qT[0:HD, pair, n0:512], start=True, stop=True)
                        nc.tensor.matmul(
                            ps[:, 1, n0:512],
                            kT[HD:P, pair, kc * P:(kc + 1) * P],
                            qT[HD:P, pair, n0:512], start=True, stop=True)
                        mw = min(P, 512 - n0)
                        for j in range(2):
                            nc.tensor.matmul(
                                ps[:, j, n0:n0 + mw], mk_t[:],
                                mq_t[:, kc, 0:mw], start=False,
                                stop=False, skip_group_check=True)
                        nc.scalar.activation(out=pt[:, :, t, n0:512],
                                             in_=ps[:, :, n0:512],
                                             func=AF.Exp, scale=0.125,
                                             bias=neg2_t[:])
                    pts.append((kcp, n0, pt))
                    if kcp == 0 and prev_flush is not None:
                        prev_flush()
                    if kcp == 1 and prev_epi is not None:
                        prev_epi()
                    if len(pts) >= 4:
                        emit_av(pts.pop(0), last=False)
                    drain_proj(5 if pair < 6 else 2)

                def flush():
                    while pts:
                        emit_av(pts.pop(0), last=(not pts))

                def epi():
                    for hp, pav in ((0, pavA), (HD, pavB)):
                        rec = rtile.tile([1, 512], F32, tag="rec")
                        nc.vector.reciprocal(rec[:], pav[HD:HD + 1, :])
                        rec_b = rtile.tile([HD, 512], F32, tag="rec_b")
                        nc.gpsimd.partition_broadcast(rec_b[:], rec[0:1, :])
                        nc.vector.tensor_tensor(
                            out=aoT8[hp:hp + HD, pair, :],
                            in0=pav[:HD, :], in1=rec_b[:], op=ALU.mult)
                return epi, flush

            # ---- projections (pairs 0..6 overlap with x_pool live) ----
            with (
                tc.tile_pool(name="x_pool", bufs=1) as x_pool,
                tc.tile_pool(name="ppsum", bufs=2, space="PSUM") as ppsum,
            ):
                wk_t = x_pool.tile([P, IC, D], F8E4)
                xT_t = x_pool.tile([P, IC, L], F8E4)
                wq_t = x_pool.tile([P, IC, D], F8E4)
                xrT_t = x_pool.tile([P, IC, 512], F8E4)
                wv_t = x_pool.tile([P, IC, D], F8E4)
                wkr = io["wk"].rearrange("(i p) n -> p i n", p=P)
                wqr = io["wq"].rearrange("(i p) n -> p i n", p=P)
                wvr = io["wv"].rearrange("(i p) n -> p i n", p=P)
                xTr = io["xT"].rearrange("(i p) n -> p i n", p=P)
                nc.sync.dma_start(wk_t[:, :, 0:P], wkr[:, :, 0:P])
                tiny_dmas()
                nc.sync.dma_start(xT_t[:, 0:4, 0:512], xTr[:, 0:4, 0:512])
                nc.sync.dma_start(xT_t[:, 4:8, 0:512], xTr[:, 4:8, 0:512])
                nc.sync.dma_start(wq_t[:, :, 0:P], wqr[:, :, 0:P])
                nc.sync.dma_start(xrT_t[:],
                                  io["xrT"].rearrange("(i p) n -> p i n", p=P))
                nc.sync.dma_start(wq_t[:, :, P:512], wqr[:, :, P:512])
                nc.sync.dma_start(wv_t[:, :, 0:512], wvr[:, :, 0:512])
                early_dmas()
                nc.sync.dma_start(xT_t[:, :, 512:1024], xTr[:, :, 512:1024])
                nc.sync.dma_start(wk_t[:, :, P:512], wkr[:, :, P:512])
                nc.sync.dma_start(xT_t[:, :, 1024:1536], xTr[:, :, 1024:1536])
                nc.sync.dma_start(xT_t[:, :, 1536:2048], xTr[:, :, 1536:2048])
                nc.sync.dma_start(wk_t[:, :, 512:1024], wkr[:, :, 512:1024])
                nc.sync.dma_start(wq_t[:, :, 512:1024], wqr[:, :, 512:1024])
                const_dmas()
                nc.sync.dma_start(wv_t[:, :, 512:1024], wvr[:, :, 512:1024])

                def k_proj(oc):
                    for tcc in range(4):
                        ps = ppsum.tile([P, 512], F32, tag="proj")
                        for i2 in range(ICH):
                            nc.tensor.matmul(
                                ps[:],
                                wk_t[:, 2 * i2:2 * i2 + 2, oc * P:(oc + 1) * P],
                                xT_t[:, 2 * i2:2 * i2 + 2,
                                     tcc * 512:(tcc + 1) * 512],
                                start=(i2 == 0), stop=(i2 == ICH - 1),
                                perf_mode=DR)
                            yield
                        nc.vector.tensor_scalar(
                            out=kT[:, oc, tcc * 512:(tcc + 1) * 512],
                            in0=ps[:], scalar1=1.0 / WQK_SCALE,
                            scalar2=bk_t[:, oc:oc + 1],
                            op0=ALU.mult, op1=ALU.add)

                def q_proj(oc):
                    ps = ppsum.tile([P, 512], F32, tag="proj")
                    for i2 in range(ICH):
                        nc.tensor.matmul(
                            ps[:],
                            wq_t[:, 2 * i2:2 * i2 + 2, oc * P:(oc + 1) * P],
                            xrT_t[:, 2 * i2:2 * i2 + 2, :],
                            start=(i2 == 0), stop=(i2 == ICH - 1),
                            perf_mode=DR)
                        yield
                    nc.vector.tensor_scalar(
                        out=qT[:, oc, :], in0=ps[:], scalar1=1.0 / WQK_SCALE,
                        scalar2=bq_t[:, oc:oc + 1], op0=ALU.mult, op1=ALU.add)

                def v_proj(tcc, hf):
                    ps = ppsum.tile([P, 512], F32, tag="proj")
                    for i2 in range(ICH):
                        nc.tensor.matmul(
                            ps[:],
                            xT_t[:, 2 * i2:2 * i2 + 2, tcc * P:(tcc + 1) * P],
                            wv_t[:, 2 * i2:2 * i2 + 2,
                                 hf * 512:(hf + 1) * 512],
                            start=(i2 == 0), stop=(i2 == ICH - 1),
                            perf_mode=DR)
                        yield
                    nc.vector.tensor_tensor(
                        out=v8[:, tcc, hf * 8:(hf + 1) * 8, :HD],
                        in0=ps.rearrange("p (h d) -> p h d", d=HD),
                        in1=bv_t[:, hf * 512:(hf + 1) * 512]
                        .rearrange("p (h d) -> p h d", d=HD),
                        op=ALU.add)

                def adv(gen, n):
                    for _ in range(n):
                        try:
                            next(gen)
                        except StopIteration:
                            return

                ks = [k_proj(oc) for oc in range(IC)]
                qs = [q_proj(oc) for oc in range(IC)]
                v0s = [v_proj(tcc, 0) for tcc in range(TC)]
                v1s = [v_proj(tcc, 1) for tcc in range(TC)]
                # upfront, ordered to match serial DMA arrival.  All of V0
                # must be EMITTED before pair 0's AV flush (tile deps track
                # emission order), so V0 is not deferred.
                adv(ks[0], 4)                    # K0.tcc0 (wk0+xT0)
                for oc in range(4):
                    adv(qs[oc], 5)               # Q0-3 (wq0+xrT)
                adv(ks[0], 100)                  # K0 rest (xT1-3)
                for tcc in range(TC):
                    adv(v0s[tcc], 5)             # V0 (wv0+xT)
                # deferred: rest drained inside the attention pair loop.
                # K projections lead (their DVE epilogues gate the next
                # pair's scores); deadlines at 5 drains/kcp: v1 t12-15
                # emitted by pair-4's carried flush (pos ~200), k6/k7 by
                # pairs 6/7.
                proj_stream.append(ks[1])
                proj_stream.append(ks[2])
                proj_stream.extend(qs[4:8])
                proj_stream.append(ks[3])
                proj_stream.extend(v1s[0:4])
                proj_stream.append(ks[4])
                proj_stream.extend(v1s[4:12])
                proj_stream.append(ks[5])
                proj_stream.extend(v1s[12:16])
                proj_stream.extend([ks[6], ks[7]])

                prev_epi = prev_flush = None
                for pair in range(4):
                    prev_epi, prev_flush = attention(pair, prev_epi,
                                                     prev_flush)
                # mid-attention: prefetch fc1 weights (SBUF freed by Q release
                # is modest; w1h/w1l fit alongside the attention working set)
                w1_pool = tc.alloc_tile_pool(name="w1_pool", bufs=1,
                                             side="right")
                w1h_t = w1_pool.tile([P, IC, DFF], F8E4)
                w1r_h = io["w1h"].rearrange("(i p) n -> p i n", p=P)
                for c in range(4):
                    nc.sync.dma_start(
                        w1h_t[:, :, c * 1024:(c + 1) * 1024],
                        w1r_h[:, :, c * 1024:(c + 1) * 1024])
                late_pools["w1h"] = (w1_pool, w1h_t)
                for pair in range(4, 6):
                    prev_epi, prev_flush = attention(pair, prev_epi,
                                                     prev_flush)
                w1l_pool = tc.alloc_tile_pool(name="w1l_pool", bufs=1,
                                              side="right")
                w1l_t = w1l_pool.tile([P, IC, DFF], F8E5)
                w1r_l = io["w1l"].rearrange("(i p) n -> p i n", p=P)
                for c in range(4):
                    nc.sync.dma_start(
                        w1l_t[:, :, c * 1024:(c + 1) * 1024],
                        w1r_l[:, :, c * 1024:(c + 1) * 1024])
                late_pools["w1l"] = (w1l_pool, w1l_t)
                prev_epi, prev_flush = attention(6, prev_epi, prev_flush)
                drain_proj(1 << 30)

            # x_pool freed: prefetch xr + wo + w2h under attn 7 (right side)
            xrr_pool = tc.alloc_tile_pool(name="xrr_pool", bufs=1, side="right")
            xr_nat = xrr_pool.tile([P, 4, D], F32)
            nc.sync.dma_start(xr_nat[:],
                              io["xr"].rearrange("(rc p) d -> p rc d", p=P))
            wo_pool = tc.alloc_tile_pool(name="wo_pool", bufs=1, side="right")
            wo_t = wo_pool.tile([P, IC, D], F8E4)
            wor = io["wo"].rearrange("(i p) n -> p i n", p=P)
            nc.sync.dma_start(wo_t[:], wor[:])

            prev_epi, prev_flush = attention(7, prev_epi, prev_flush)
            prev_flush()
            prev_epi()

            # free the attention pools (non-LIFO: wo/w1 stay live)
            avpsum.release()
            spsum.release()
            rtile.release()
            ptile.release()
            kv_pool.release()

            w1h_t = late_pools["w1h"][1]
            w1l_t = late_pools["w1l"][1]

            if True:
                # ---- out-proj + LN1 + transpose (hi/lo split) ----
                # The critical path transposes the RAW normalized u (g1/be1
                # are folded into W1h/W1l/b1 host-side); the affine tbf
                # (= x1 + b2, the LN2 residual) is computed off-path.
                with tc.tile_pool(name="t_pool", bufs=1) as t_pool:
                    tbf = t_pool.tile([P, 4, D], BF16)     # x1 + b2 (residual)
                    tTh = t_pool.tile([P, IC, 512], F8E4)  # u^T hi
                    tTl = t_pool.tile([P, IC, 512], F8E5)  # u^T lo

                    # fc2 weights fit once the attention tiles are gone;
                    # DMA'd in fcp order so fc2 matmuls chase the transfers
                    w2h_pool = tc.alloc_tile_pool(name="w2h_pool", bufs=1)
                    w2h_t = w2h_pool.tile([P, FC, D], F8E4)
                    w2r_h = io["w2h"].rearrange("(f p) n -> p f n", p=P)
                    for grp in range(4):
                        nc.sync.dma_start(
                            w2h_t[:, grp * 8:(grp + 1) * 8, :],
                            w2r_h[:, grp * 8:(grp + 1) * 8, :])
                    w2l_pool = tc.alloc_tile_pool(name="w2l_pool", bufs=1)
                    w2l_t = w2l_pool.tile([P, FC, D], F8E5)
                    w2r_l = io["w2l"].rearrange("(f p) n -> p f n", p=P)
                    for grp in range(4):
                        nc.sync.dma_start(
                            w2l_t[:, grp * 8:(grp + 1) * 8, :],
                            w2r_l[:, grp * 8:(grp + 1) * 8, :])

                    with (
                        tc.tile_pool(name="lnt", bufs=2) as lnt,
                        tc.tile_pool(name="opsum", bufs=4, space="PSUM") as opsum,
                        tc.tile_pool(name="trpsum", bufs=2, space="PSUM") as trpsum,
                    ):
                        for rc in range(4):
                            acc = lnt.tile([P, D], F32, tag="acc")
                            for n2 in range(2):
                                pso = opsum.tile([P, 512], F32, tag="o")
                                for i2 in range(ICH):
                                    nc.tensor.matmul(
                                        pso[:],
                                        aoT8[:, 2 * i2:2 * i2 + 2,
                                             rc * P:(rc + 1) * P],
                                        wo_t[:, 2 * i2:2 * i2 + 2,
                                             n2 * 512:(n2 + 1) * 512],
                                        start=(i2 == 0), stop=(i2 == ICH - 1),
                                        perf_mode=DR)
                                nc.vector.tensor_tensor(
                                    out=acc[:, n2 * 512:(n2 + 1) * 512],
                                    in0=pso[:],
                                    in1=xr_nat[:, rc, n2 * 512:(n2 + 1) * 512],
                                    op=ALU.add)
                            ubf = lnt.tile([P, D], BF16, tag="ubf")
                            _ln_u(nc, lnt, acc, eps_t, ubf[:])
                            # bf16 transposes packed 4 to a psum bank, then
                            # one wide ACT cast (hi) + one wide DVE
                            # subtract (lo) per group
                            for a in range(2):
                                pst4 = trpsum.tile([P, 4, P], BF16, tag="tr")
                                for j in range(4):
                                    ic = 4 * a + j
                                    nc.tensor.transpose(
                                        pst4[:, j, :],
                                        ubf[:, ic * P:(ic + 1) * P],
                                        ident[:])
                                th = tTh[:, 4 * a:4 * a + 4,
                                         rc * P:(rc + 1) * P]
                                nc.scalar.copy(th, pst4[:])
                                nc.vector.tensor_tensor(
                                    out=tTl[:, 4 * a:4 * a + 4,
                                            rc * P:(rc + 1) * P],
                                    in0=pst4[:], in1=th, op=ALU.subtract)
                            # off-path: residual tbf = u*g1 + (be1 + b2)
                            # (bf16 all-SBUF -> DVE 2x mode, cheap)
                            nc.vector.tensor_tensor(
                                out=tbf[:, rc, :], in0=ubf[:],
                                in1=g1_t[:, :], op=ALU.mult)
                            nc.vector.tensor_tensor(
                                out=tbf[:, rc, :], in0=tbf[:, rc, :],
                                in1=be1_t[:, :], op=ALU.add)

                    wo_pool.release()
                    xrr_pool.release()

                    # ================= FFN =================
                    with (
                        tc.tile_pool(name="h_pool", bufs=1) as h_pool,
                        tc.tile_pool(name="tb_pool", bufs=2) as tb_pool,
                        tc.tile_pool(name="fpsum", bufs=2, space="PSUM") as fpsum,
                        tc.tile_pool(name="ypsum", bufs=3, space="PSUM") as ypsum,
                    ):
                        hh = h_pool.tile([P, FC, 512], F8E4)
                        hl = h_pool.tile([P, FC, 512], F8E5)
                        psy = {}
                        stop_tracker = {}

                        def fc2_mms(fcp, rcs, hx, wx, term):
                            for rc in rcs:
                                for n2 in range(2):
                                    key = (rc, n2)
                                    start = key not in stop_tracker
                                    stop_tracker[key] = True
                                    nc.tensor.matmul(
                                        psy[rc][:, n2, :],
                                        hx[:, 2 * fcp:2 * fcp + 2,
                                           rc * P:(rc + 1) * P],
                                        wx[:, 2 * fcp:2 * fcp + 2,
                                           n2 * 512:(n2 + 1) * 512],
                                        start=start, stop=False,
                                        perf_mode=DR, skip_group_check=True)

                        def fc2_mms_last(fcp, rcs):
                            for rc in rcs:
                                for n2 in range(2):
                                    nc.tensor.matmul(
                                        psy[rc][:, n2, :],
                                        hh[:, 2 * fcp:2 * fcp + 2,
                                           rc * P:(rc + 1) * P],
                                        w2l_t[:, 2 * fcp:2 * fcp + 2,
                                              n2 * 512:(n2 + 1) * 512],
                                        start=False, stop=True,
                                        perf_mode=DR, skip_group_check=True)

                        finbox = {}

                        def epilogue(rc):
                            fin = finbox["p"]
                            acc = fin.tile([P, D], F32, tag="acc2", bufs=2)
                            for n2 in range(2):
                                nc.vector.scalar_tensor_tensor(
                                    out=acc[:, n2 * 512:(n2 + 1) * 512],
                                    in0=psy[rc][:, n2, :],
                                    scalar=1.0 / W2_SCALE,
                                    in1=tbf[:, rc, n2 * 512:(n2 + 1) * 512],
                                    op0=ALU.mult, op1=ALU.add)
                            res = fin.tile([P, D], BF16, tag="res", bufs=2)
                            _layernorm(nc, fin, acc, eps_t, g2_t, be2_t,
                                       res[:], g_eng=nc.vector,
                                       b_eng=nc.vector)
                            nc.sync.dma_start(
                                out.rearrange("(rc p) d -> p rc d", p=P)[:, rc, :],
                                res[:])

                        # pass 1: fc1 + fc2 for rc 0,1,2 interleaved per fc;
                        # term3 (hh @ w2l) lags 6 fcp behind so the w2l DMA
                        # (which only starts after the attention pools free)
                        # has landed.
                        psy[0] = ypsum.tile([P, 2, 512], F32, tag="y", name="psy0")
                        psy[1] = ypsum.tile([P, 2, 512], F32, tag="y", name="psy1")
                        psy[2] = ypsum.tile([P, 2, 512], F32, tag="y", name="psy2")
                        LAG = 6
                        for fc in range(FC):
                            ps = fpsum.tile([P, 512], F32, tag="f1")
                            # rc-halves: the first half's operands (tT cols
                            # 0:256 = row chunks 0-1) are ready before the
                            # second, so fc1 can start while LN1/transpose
                            # of rc2-3 is still in flight.
                            for rh in range(2):
                                cols = slice(rh * 256, rh * 256 + 256)
                                h_first = True
                                for wt, xt in ((w1h_t, tTh), (w1h_t, tTl),
                                               (w1l_t, tTh)):
                                    for i2 in range(ICH):
                                        last = (xt is tTh and wt is w1l_t
                                                and i2 == ICH - 1)
                                        nc.tensor.matmul(
                                            ps[:, cols],
                                            wt[:, 2 * i2:2 * i2 + 2,
                                               fc * P:(fc + 1) * P],
                                            xt[:, 2 * i2:2 * i2 + 2, cols],
                                            start=h_first, stop=last,
                                            perf_mode=DR)
                                        h_first = False
                            tb = tb_pool.tile([P, 512], BF16, tag="tb")
                            nc.scalar.activation(out=tb[:], in_=ps[:],
                                                 func=AF.Gelu,
                                                 bias=b1_t[:, fc:fc + 1],
                                                 scale=1.0 / W1_SCALE)
                            nc.gpsimd.tensor_copy(out=hh[:, fc, :], in_=tb[:])
                            nc.vector.tensor_tensor(out=hl[:, fc, :],
                                                    in0=tb[:],
                                                    in1=hh[:, fc, :],
                                                    op=ALU.subtract)
                            if fc % 2 == 1:
                                fcp = fc // 2
                                fc2_mms(fcp, (0, 1, 2), hh, w2h_t, 1)
                                fc2_mms(fcp, (0, 1, 2), hl, w2h_t, 2)
                                if fcp >= LAG:
                                    if fcp - LAG == FCH - 1:
                                        fc2_mms_last(fcp - LAG, (0, 1, 2))
                                    else:
                                        fc2_mms(fcp - LAG, (0, 1, 2), hh,
                                                w2l_t, 3)
                        # fc1 weights are done with: free before the LN2
                        # epilogue scratch allocates (LIFO on the right
                        # stack: w1l, then w1h, then ao)
                        late_pools["w1l"][0].release()
                        late_pools["w1h"][0].release()
                        ao_pool.release()
                        for fcp in range(FCH - LAG, FCH):
                            if fcp == FCH - 1:
                                fc2_mms_last(fcp, (0, 1, 2))
                            else:
                                fc2_mms(fcp, (0, 1, 2), hh, w2l_t, 3)
                        finbox["p"] = tc.alloc_tile_pool(name="fin", bufs=1)
                        epilogue(0)
                        epilogue(1)
                        epilogue(2)
                        # pass 2: fc2 for rc3 (everything resident now)
                        psy[3] = ypsum.tile([P, 2, 512], F32, tag="y", name="psy3")
                        for fcp in range(FCH):
                            fc2_mms(fcp, (3,), hh, w2h_t, 1)
                            fc2_mms(fcp, (3,), hl, w2h_t, 2)
                            if fcp == FCH - 1:
                                fc2_mms_last(fcp, (3,))
                            else:
                                fc2_mms(fcp, (3,), hh, w2l_t, 3)
                        epilogue(3)
                        finbox["p"].release()
                    w2l_pool.release()
                    w2h_pool.release()


def _blocks(g):
    """64-row blocks owned by core g: {8m+g, 8m+7-g} - exactly balanced
    causal load across the 4 cores of a batch group."""
    return sorted(b for m in range(4) for b in (8 * m + g, 8 * m + 7 - g))


def _row_index(g):
    idx = np.empty(512, dtype=np.int64)
    for v, w in enumerate(_blocks(g)):
        idx[v * 64:(v + 1) * 64] = np.arange(w * 64, w * 64 + 64)
    return idx


def _mask_mq(g):
    """Causal indicator for the mask matmul: column q of block kc selects
    the row m of the static -240*[k >= m] stationary such that the product
    adds -240 exactly where global key > global query."""
    blocks = _blocks(g)
    mq = np.zeros((P, TC, P), dtype=np.float32)
    for kc in range(TC):
        n0 = 64 * (kc // 2)
        for qq in range(P):
            q = n0 + qq
            if q >= 512:
                break
            qg = 64 * blocks[q // 64] + q % 64
            thr = qg - P * kc          # mask iff key k > thr
            tgt = max(thr + 1, 0)
            if tgt <= P - 1:
                mq[tgt, kc, qq] = 1.0
    return mq


def _mask_mk():
    m = np.arange(P)[:, None]
    k = np.arange(P)[None, :]
    return np.where(k >= m, -240.0, 0.0).astype(np.float32)


def kernel(**inputs):
    if "nc" not in _CACHE:
        _CACHE["nc"] = _build()
    nc = _CACHE["nc"]

    bf = ml_dtypes.bfloat16
    e4 = ml_dtypes.float8_e4m3
    e5 = ml_dtypes.float8_e5m2
    x = np.asarray(inputs["x"], dtype=np.float32)

    def f32(k):
        return np.asarray(inputs[k], dtype=np.float32)

    wq8 = np.ascontiguousarray((WQK_SCALE * f32("Wq")).astype(e4))
    wk8 = np.ascontiguousarray((WQK_SCALE * f32("Wk")).astype(e4))
    wv8 = np.ascontiguousarray(f32("Wv").astype(e4))
    wo8 = np.ascontiguousarray(f32("Wo").astype(e4))
    vecs = {k: f32(k) for k in ("bq", "bk", "bv", "bo", "b1", "b2", "g1",
                                "be1", "g2", "be2")}
    # fold LN1's affine into the fc1 weights/bias: x1 @ W1 = u @ (g1*W1)
    # + be1 @ W1 (the raw normalized u is what gets transposed on-chip)
    w1s = W1_SCALE * (vecs["g1"][:, None] * f32("W1"))
    w1h = w1s.astype(e4)
    w1l = (w1s - w1h.astype(np.float32)).astype(e5)
    w1h, w1l = np.ascontiguousarray(w1h), np.ascontiguousarray(w1l)
    b1f = vecs["b1"] + vecs["be1"] @ f32("W1")
    w2s = W2_SCALE * f32("W2")
    w2h = w2s.astype(e4)
    w2l = (w2s - w2h.astype(np.float32)).astype(e5)
    w2h, w2l = np.ascontiguousarray(w2h), np.ascontiguousarray(w2l)
    # tbf on-chip computes u*g1 + be1f where be1f = be1 + b2 (the fc2 bias
    # rides along with the LN2 residual)
    be1f = vecs["be1"] + vecs["b2"]
    mk = _mask_mk().astype(bf)

    in_maps = []
    for c in range(N_CORES):
        b, g = c // 4, c % 4
        idx = _row_index(g)
        xb = x[b]
        xrows = xb[idx]
        in_maps.append({
            "xT": np.ascontiguousarray(xb.T.astype(e4)),
            "xrT": np.ascontiguousarray(xrows.T.astype(e4)),
            "xr": np.ascontiguousarray(xrows + vecs["bo"][None, :]),
            "wq": wq8, "wk": wk8, "wv": wv8, "wo": wo8,
            "w1h": w1h, "w1l": w1l, "w2h": w2h, "w2l": w2l,
            "bq": vecs["bq"], "bk": vecs["bk"],
            "bv": vecs["bv"].astype(bf),
            "b1": b1f,
            "g1": vecs["g1"].astype(bf), "be1": be1f.astype(bf),
            "g2": vecs["g2"].astype(bf), "be2": vecs["be2"].astype(bf),
            "mq": _mask_mq(g).astype(bf),
            "mk": mk,
        })

    res = run_bass_kernel_spmd(nc, in_maps, core_ids=list(range(N_CORES)))
    _CACHE["last_result"] = res

    outp = np.empty((B, L, D), dtype=np.float32)
    for c in range(N_CORES):
        b, g = c // 4, c % 4
        outp[b][_row_index(g)] = res.results[c]["out"].astype(np.float32)
    return outp
